# revision 1
# baseline (speedup 1.0000x reference)
"""Trainium2 Bass kernel for nn_CIFARViT (spectral group-attention ViT).

kernel(**inputs) takes the FULL unsharded inputs (keys as in setup_inputs),
shards the batch over 8 NeuronCores (pure data parallel, 4 images/core),
runs one Bass program per core via run_bass_kernel_spmd, and gathers the
full [32, 10] output.
"""
import sys
import os

if '/opt/trn_rl_repo' not in sys.path:
    sys.path.insert(0, '/opt/trn_rl_repo')
os.environ.setdefault("NEURON_RT_RESET_CORES", "1")

import numpy as np

import concourse.bass as bass
import concourse.mybir as mybir
from concourse import bacc
from concourse.tile import TileContext
from concourse.bass_utils import run_bass_kernel_spmd

F32 = mybir.dt.float32
F32R = mybir.dt.float32r
AF = mybir.ActivationFunctionType
ALU = mybir.AluOpType
AX = mybir.AxisListType

D = 8
C = 768
H = 8
B = 32
BL = 4
NCORES = 8
N = 256
CH = 96
N_EIGS = 5
NCLS = 10
FC = C // 128
SCALE_QK = CH ** -0.5

K = 16
ROUNDS = [8, 8, 3, 3, 3, 3, 3, 3]
if os.environ.get("KERNEL_R1"):
    ROUNDS = [int(os.environ["KERNEL_R1"])] * 8
D_CHEB = 2
A_INT, B_INT = 0.95, 1.35
C_CENT = (B_INT + A_INT) / 2.0
CHEB_S = 4.0 / (B_INT - A_INT)
ETA = 1e-2
NS_MID = (3e-3, 13)
NS_F1 = (1e-3, 14)
NS_F2 = (1e-5, 8)
DIAG_ITERS = 16
DIAG_CAP = 0.45
DIAG_DAMP = 0.4
TIE_D = 1e-5
GM = 16
NG4 = 4

_SEED = 1234


def _host_consts():
    rng = np.random.RandomState(_SEED)
    X0 = rng.randn(N, K).astype(np.float32)
    R0 = (rng.randn(N, K).astype(np.float32) / np.sqrt(N)) * ETA
    cs = {}
    cs["x0c"] = np.ascontiguousarray(X0.reshape(2, 128, K).transpose(1, 0, 2))
    cs["r0c"] = np.ascontiguousarray(R0.reshape(2, 128, K).transpose(1, 0, 2))
    cs["i128"] = np.eye(128, dtype=np.float32)
    cs["ni128"] = -np.eye(128, dtype=np.float32)
    cs["onesrow"] = np.ones((1, 128), dtype=np.float32)
    cs["ones128"] = np.ones((128, 1), dtype=np.float32)
    dm = np.zeros((2, 128, 256), dtype=np.float32)
    for r in range(2):
        for p in range(128):
            dm[r, p, 128 * r + p] = 1.0
    cs["dmask"] = np.ascontiguousarray(dm.transpose(1, 0, 2))
    cs["vmask"] = np.ascontiguousarray(1.0 - dm.transpose(1, 0, 2))
    import itertools
    combs = np.array(list(itertools.product([1.0, -1.0], repeat=N_EIGS)),
                     dtype=np.float32)
    c4 = np.zeros((128, 32), dtype=np.float32)
    for r in range(4):
        c4[32 * r:32 * r + N_EIGS, :] = combs.T
    cs["combs4"] = c4
    eye = np.zeros((128, 512), dtype=np.float32)
    blk = np.zeros((128, 512), dtype=np.float32)
    iot = np.full((128, 512), 1e9, dtype=np.float32)
    for g4 in range(4):
        for r in range(4):
            for i in range(K):
                eye[32 * r + i, 128 * g4 + 32 * r + i] = 1.0
                blk[32 * r + i, 128 * g4 + 32 * r:128 * g4 + 32 * r + K] = 1.0
                for j in range(N_EIGS):
                    iot[32 * r + i, 128 * g4 + 32 * r + j] = float(j)
    cs["sm_eye"] = eye
    cs["sm_eye15"] = 1.5 * eye
    cs["sm_blk"] = blk
    cs["sm_offblk"] = blk - eye
    cs["sm_iotasel"] = iot
    cs["sm_tie"] = (np.arange(128, dtype=np.float32) % 32 * TIE_D).reshape(128, 1)
    e32 = np.zeros((4, 128), dtype=np.float32)
    for r in range(4):
        e32[r, 32 * r:32 * (r + 1)] = 1.0
    cs["exp32"] = e32
    return cs


CONSTS = _host_consts()


def _prep_weights(inputs):
    w = {}
    qkv_w = np.asarray(inputs["qkv_w"], dtype=np.float32)
    w["wq"] = np.ascontiguousarray(qkv_w[:, :, 0:C]).reshape(D, FC, 128, C)
    w["wv"] = np.ascontiguousarray(qkv_w[:, :, 2 * C:3 * C]).reshape(D, FC, 128, C)
    w["wproj"] = np.ascontiguousarray(np.asarray(inputs["proj_w"], dtype=np.float32))
    w["bproj"] = np.asarray(inputs["proj_b"], dtype=np.float32).reshape(D, FC, 128, 1).copy()
    w["lns"] = np.asarray(inputs["g_ln_s"], dtype=np.float32).reshape(D, FC, 128, 1).copy()
    w["lnb"] = np.asarray(inputs["g_ln_b"], dtype=np.float32).reshape(D, FC, 128, 1).copy()
    w["w1"] = np.ascontiguousarray(np.asarray(inputs["mlp_w1"], dtype=np.float32)).reshape(D, FC, 128, 4 * C)
    w["b1"] = np.asarray(inputs["mlp_b1"], dtype=np.float32).reshape(D, 24, 128, 1).copy()
    w["w2"] = np.ascontiguousarray(np.asarray(inputs["mlp_w2"], dtype=np.float32)).reshape(D, 24, 128, C)
    w["b2"] = np.asarray(inputs["mlp_b2"], dtype=np.float32).reshape(D, FC, 128, 1).copy()
    w["lns2"] = np.asarray(inputs["ln_s"], dtype=np.float32).reshape(FC, 128, 1).copy()
    w["lnb2"] = np.asarray(inputs["ln_b"], dtype=np.float32).reshape(FC, 128, 1).copy()
    w["whead"] = np.asarray(inputs["head_w"], dtype=np.float32).reshape(FC, 128, NCLS).copy()
    w["bhead"] = np.asarray(inputs["head_b"], dtype=np.float32).reshape(1, NCLS).copy()
    pw = np.asarray(inputs["patch_w"], dtype=np.float32).reshape(C, 12)
    w["pwT"] = np.ascontiguousarray(pw.T)
    w["pbias"] = np.asarray(inputs["patch_b"], dtype=np.float32).reshape(FC, 128, 1).copy()
    pos = np.asarray(inputs["pos_emb"], dtype=np.float32).reshape(N, C)
    w["posT"] = np.ascontiguousarray(pos.T).reshape(FC, 128, N)
    return w


def _prep_x(x):
    Bb = x.shape[0]
    xp = np.asarray(x, dtype=np.float32).reshape(Bb, 3, 16, 2, 16, 2)
    xp = xp.transpose(0, 2, 4, 1, 3, 5).reshape(Bb, N, 12)
    return np.ascontiguousarray(np.swapaxes(xp, 1, 2))


# ====================== device program ======================

def build_nc(n_layers=D, tap_layer=None):
    nc = bacc.Bacc("TRN2", target_bir_lowering=False, debug=False)
    dram = {}

    def din(name, shape):
        dram[name] = nc.dram_tensor(name, list(shape), F32, kind="ExternalInput")

    din("xpt", (BL, 12, N))
    din("pwT", (12, C))
    din("pbias", (FC, 128, 1))
    din("posT", (FC, 128, N))
    din("wq", (D, FC, 128, C))
    din("wv", (D, FC, 128, C))
    din("wproj", (D, CH, C))
    din("bproj", (D, FC, 128, 1))
    din("lns", (D, FC, 128, 1))
    din("lnb", (D, FC, 128, 1))
    din("w1", (D, FC, 128, 4 * C))
    din("b1", (D, 24, 128, 1))
    din("w2", (D, 24, 128, C))
    din("b2", (D, FC, 128, 1))
    din("lns2", (FC, 128, 1))
    din("lnb2", (FC, 128, 1))
    din("whead", (FC, 128, NCLS))
    din("bhead", (1, NCLS))
    for cn, arr in CONSTS.items():
        din(cn, arr.shape)

    out = nc.dram_tensor("out", [BL, NCLS], F32, kind="ExternalOutput")
    tap = None
    if tap_layer is not None:
        tap = nc.dram_tensor("tap", [128, FC, BL, N], F32, kind="ExternalOutput")
    if os.environ.get("KERNEL_DEBUG_TAPS"):
        nc._dbg = {
            "d_emb": nc.dram_tensor("d_emb", [128, FC, BL, N], F32, kind="ExternalOutput"),
            "d_S": nc.dram_tensor("d_S", [128, GM, 2, N], F32, kind="ExternalOutput"),
            "d_V": nc.dram_tensor("d_V", [128, GM, 2, CH], F32, kind="ExternalOutput"),
            "d_u0": nc.dram_tensor("d_u0", [128, GM, 2, 1], F32, kind="ExternalOutput"),
            "d_X": nc.dram_tensor("d_X", [128, GM, 2, K], F32, kind="ExternalOutput"),
            "d_B": nc.dram_tensor("d_B", [128, 512], F32, kind="ExternalOutput"),
            "d_Ut": nc.dram_tensor("d_Ut", [128, NG4, N], F32, kind="ExternalOutput"),
            "d_att": nc.dram_tensor("d_att", [128, FC, BL, N], F32, kind="ExternalOutput"),
        }
    else:
        nc._dbg = {}

    with TileContext(nc) as tc:
        _emit(nc, tc, dram, out, tap, n_layers, tap_layer)
    nc.compile()
    return nc


def _emit(nc, tc, dram, out, tap, n_layers, tap_layer):
    import contextlib
    es = contextlib.ExitStack()
    with es:
        persist = es.enter_context(tc.tile_pool(name="persist", bufs=1))
        wpool = es.enter_context(tc.tile_pool(name="wpool", bufs=1))
        wstr = es.enter_context(tc.tile_pool(name="wstr", bufs=2))
        wqrt = es.enter_context(tc.tile_pool(name="wqrt", bufs=1))
        spool = es.enter_context(tc.tile_pool(name="spool", bufs=1))
        epool = es.enter_context(tc.tile_pool(name="epool", bufs=1))
        scr = es.enter_context(tc.tile_pool(name="scr", bufs=1))
        rowp = es.enter_context(tc.tile_pool(name="rowp", bufs=1))
        ps1 = es.enter_context(tc.tile_pool(name="ps1", bufs=2, space="PSUM"))
        ps2 = es.enter_context(tc.tile_pool(name="ps2", bufs=2, space="PSUM"))
        ps3 = es.enter_context(tc.tile_pool(name="ps3", bufs=2, space="PSUM"))

        cst = {}
        for cn, arr in CONSTS.items():
            t = persist.tile(list(arr.shape), F32, tag="c_" + cn)
            cst[cn] = t
            nc.sync.dma_start(t[:], dram[cn][:])

        def smc(name):
            return cst[name][:].rearrange("p (a b) -> p a b", a=4)

        hT = persist.tile([128, FC, BL, N], F32, tag="hT")

        def r32(ap):
            return ap

        TT = nc.vector.tensor_tensor
        TS = nc.vector.tensor_scalar
        STT = nc.vector.scalar_tensor_tensor
        CP = nc.vector.tensor_copy
        MM = nc.tensor.matmul

        # ============ patch embed ============
        xpt = persist.tile([12, BL, N], F32, tag="xpt")
        nc.sync.dma_start(xpt[:], dram["xpt"][:].rearrange("b k n -> k b n"))
        pwT = persist.tile([12, C], F32, tag="pwT")
        nc.sync.dma_start(pwT[:], dram["pwT"][:])
        pbias = persist.tile([128, FC, 1], F32, tag="pbias")
        nc.sync.dma_start(pbias[:], dram["pbias"][:].rearrange("f p o -> p f o"))
        posT = persist.tile([128, FC, N], F32, tag="posT")
        nc.sync.dma_start(posT[:], dram["posT"][:].rearrange("f p n -> p f n"))

        for b in range(BL):
            for ch in range(FC):
                ps = ps1.tile([128, 512], F32, tag="ps1")
                MM(ps[:, 0:N], r32(pwT[:, 128 * ch:128 * (ch + 1)]),
                   r32(xpt[:, b]), start=True, stop=True)
                tmp = scr.tile([128, N], F32, tag="ge")
                TS(tmp[:], ps[:, 0:N], pbias[:, ch], None, op0=ALU.add)
                TT(hT[:, ch, b], tmp[:], posT[:, ch], op=ALU.add)

        if nc._dbg:
            nc.sync.dma_start(nc._dbg["d_emb"][:], hT[:])
        # ===================== layers =====================
        for li in range(n_layers):
            wq = wpool.tile([128, FC, C], F32, tag="wq")
            nc.sync.dma_start(wq[:], dram["wq"][li].rearrange("f p c -> p f c"))
            wproj = wpool.tile([CH, C], F32, tag="wproj")
            nc.sync.dma_start(wproj[:], dram["wproj"][li])
            bproj = wpool.tile([128, FC, 1], F32, tag="bproj")
            nc.sync.dma_start(bproj[:], dram["bproj"][li].rearrange("f p o -> p f o"))
            lns = wpool.tile([128, FC, 1], F32, tag="lns")
            nc.sync.dma_start(lns[:], dram["lns"][li].rearrange("f p o -> p f o"))
            lnb = wpool.tile([128, FC, 1], F32, tag="lnb")
            nc.sync.dma_start(lnb[:], dram["lnb"][li].rearrange("f p o -> p f o"))
            b1t = wpool.tile([128, 24, 1], F32, tag="b1t")
            nc.sync.dma_start(b1t[:], dram["b1"][li].rearrange("j p o -> p j o"))
            b2t = wpool.tile([128, FC, 1], F32, tag="b2t")
            nc.sync.dma_start(b2t[:], dram["b2"][li].rearrange("f p o -> p f o"))

            for g in range(2):
                bidx = [2 * g, 2 * g + 1]
                Sg = spool.tile([128, GM, 2, N], F32, tag="Sg")
                Vg = spool.tile([128, GM, 2, CH], F32, tag="Vg")
                U0g = spool.tile([128, GM, 2, 1], F32, tag="U0g")
                U0r = spool.tile([128, NG4, N], F32, tag="U0r")

                # ---- v = hT^T Wv per matrix (wv streamed, SBUF-accumulate) --
                for ch in range(FC):
                    wvc = wstr.tile([128, C], F32, tag="wvc")
                    nc.sync.dma_start(wvc[:], dram["wv"][li, ch])
                    for m in range(GM):
                        b = bidx[m // H]
                        h = m % H
                        for rb in range(2):
                            psv = ps3.tile([128, 256], F32, tag="ps3")
                            MM(psv[:, 0:CH],
                               r32(hT[:, ch, b, 128 * rb:128 * (rb + 1)]),
                               r32(wvc[:, CH * h:CH * (h + 1)]),
                               start=True, stop=True)
                            if ch == 0:
                                CP(Vg[:, m, rb], psv[:, 0:CH])
                            else:
                                TT(Vg[:, m, rb], Vg[:, m, rb], psv[:, 0:CH],
                                   op=ALU.add)

                # ---- phase A (exp table): logits -> G into Sg ----
                for m in range(GM):
                    b = bidx[m // H]
                    h = m % H
                    qT = scr.tile([CH, N], F32, tag="qT")
                    psq = ps1.tile([128, 512], F32, tag="ps1")
                    for ch in range(FC):
                        MM(psq[0:CH, 0:N],
                           r32(wq[:, ch, CH * h:CH * (h + 1)]),
                           r32(hT[:, ch, b]),
                           start=(ch == 0), stop=(ch == FC - 1))
                    CP(qT[:], psq[0:CH, 0:N])
                    psl = ps2.tile([128, 512], F32, tag="ps2")
                    for rb in range(2):
                        MM(psl[:, 256 * rb:256 * (rb + 1)],
                           r32(qT[:, 128 * rb:128 * (rb + 1)]),
                           r32(qT[:]), start=True, stop=True)
                    rm = scr.tile([128, 1], F32, tag="rm")
                    nc.vector.reduce_max(rm[:], psl[:], axis=AX.X)
                    pst = ps3.tile([128, 256], F32, tag="ps3")
                    MM(pst[0:1, 0:128], rm[:], cst["i128"][:], start=True, stop=True)
                    mv = scr.tile([1, 1], F32, tag="mv")
                    nc.vector.reduce_max(mv[:], pst[0:1, 0:128], axis=AX.X)
                    pbc = ps3.tile([128, 256], F32, tag="ps3")
                    MM(pbc[:, 0:1], cst["onesrow"][:], mv[:], start=True, stop=True)
                    negm = scr.tile([128, 1], F32, tag="negm")
                    nc.scalar.mul(negm[:], pbc[:, 0:1], -SCALE_QK)
                    for rb in range(2):
                        ge = scr.tile([128, N], F32, tag="ge")
                        nc.scalar.activation(ge[:], psl[:, 256 * rb:256 * (rb + 1)],
                                             AF.Exp, bias=negm[:, 0:1],
                                             scale=SCALE_QK)
                        TT(Sg[:, m, rb], ge[:], cst["vmask"][:, rb], op=ALU.mult)

                # ---- phase B (sqrt table): G -> Ahat, u0 ----
                for m in range(GM):
                    g4, rr_ = m // 4, m % 4
                    rsum = scr.tile([128, 2], F32, tag="rsum")
                    for rb in range(2):
                        nc.vector.reduce_sum(rsum[:, rb:rb + 1], Sg[:, m, rb],
                                             axis=AX.X)
                    invr = scr.tile([128, 2], F32, tag="invr")
                    nc.vector.reciprocal(invr[:], rsum[:])
                    ptv = ps3.tile([128, 256], F32, tag="ps3")
                    for ob in range(2):
                        for kb in range(2):
                            MM(ptv[:, ob:ob + 1],
                               Sg[:, m, kb, 128 * ob:128 * (ob + 1)],
                               invr[:, kb:kb + 1],
                               start=(kb == 0), stop=(kb == 1))
                    deg = scr.tile([128, 2], F32, tag="deg")
                    TS(deg[:], ptv[:, 0:2], 0.5, 0.5, op0=ALU.mult, op1=ALU.add)
                    sd = scr.tile([128, 2], F32, tag="sd")
                    nc.scalar.activation(sd[:], deg[:], AF.Sqrt)
                    wv_ = scr.tile([128, 2], F32, tag="wv_")
                    nc.vector.reciprocal(wv_[:], sd[:])
                    alpha = scr.tile([128, 2], F32, tag="alpha")
                    TT(alpha[:], wv_[:], invr[:], op=ALU.mult)
                    nc.scalar.mul(alpha[:], alpha[:], 0.5)
                    psn = ps3.tile([128, 256], F32, tag="ps3")
                    for kb in range(2):
                        MM(psn[0:1, 0:1], deg[:, kb:kb + 1], cst["ones128"][:],
                           start=(kb == 0), stop=(kb == 1))
                    nrm = scr.tile([1, 1], F32, tag="nrm")
                    nc.scalar.activation(nrm[:], psn[0:1, 0:1], AF.Sqrt)
                    rn = scr.tile([1, 1], F32, tag="rn")
                    nc.vector.reciprocal(rn[:], nrm[:])
                    pbc = ps3.tile([128, 256], F32, tag="ps3")
                    MM(pbc[:, 0:1], cst["onesrow"][:], rn[:], start=True, stop=True)
                    rnb = scr.tile([128, 1], F32, tag="rnb")
                    CP(rnb[:], pbc[:, 0:1])
                    for kb in range(2):
                        TS(U0g[:, m, kb], sd[:, kb:kb + 1], rnb[:, 0:1], None,
                           op0=ALU.mult)
                    arow = rowp.tile([1, N], F32, tag="arow")
                    brow = rowp.tile([1, N], F32, tag="brow")
                    for (row_t, col_t) in ((arow, alpha), (brow, wv_)):
                        ptr2 = ps3.tile([128, 256], F32, tag="ps3")
                        for kb in range(2):
                            MM(ptr2[0:1, 128 * kb:128 * (kb + 1)],
                               col_t[:, kb:kb + 1], cst["i128"][:],
                               start=True, stop=True)
                        CP(row_t[:], ptr2[0:1, :])
                    ptr3 = ps3.tile([128, 256], F32, tag="ps3")
                    for kb in range(2):
                        MM(ptr3[32 * rr_:32 * rr_ + 1, 128 * kb:128 * (kb + 1)],
                           U0g[:, m, kb], cst["i128"][:],
                           start=True, stop=True, tile_position=(0, 32 * rr_))
                    CP(U0r[32 * rr_:32 * rr_ + 1, g4], ptr3[32 * rr_:32 * rr_ + 1, :])
                    for rb in range(2):
                        pso = ps1.tile([128, 512], F32, tag="ps1")
                        MM(pso[:, 0:N], arow[:, 128 * rb:128 * (rb + 1)],
                           brow[:], start=True, stop=False)
                        MM(pso[:, 0:N], brow[:, 128 * rb:128 * (rb + 1)],
                           arow[:], start=False, stop=True)
                        go = scr.tile([128, N], F32, tag="ge")
                        TT(go[:], Sg[:, m, rb], pso[:, 0:N], op=ALU.mult)
                        STT(Sg[:, m, rb], go[:], -CHEB_S, cst["dmask"][:, rb],
                            op0=ALU.mult, op1=ALU.bypass)
                        STT(Sg[:, m, rb], cst["dmask"][:, rb],
                            CHEB_S * (1.0 - C_CENT), Sg[:, m, rb],
                            op0=ALU.mult, op1=ALU.add)

                if nc._dbg and li == 0 and g == 0:
                    nc.sync.dma_start(nc._dbg["d_S"][:], Sg[:])
                    nc.sync.dma_start(nc._dbg["d_V"][:], Vg[:])
                    nc.sync.dma_start(nc._dbg["d_u0"][:], U0g[:])
                _eigensolve_and_attend(
                    nc, tc, cst, smc, dram, hT, Sg, Vg, U0g, U0r,
                    wproj, bproj, epool, spool, scr, ps1, ps2, ps3,
                    TT, TS, STT, CP, MM, r32, li, g, bidx)

            if nc._dbg and li == 0:
                nc.sync.dma_start(nc._dbg["d_att"][:], hT[:])
            _mlp_block(nc, tc, cst, dram, hT, lns, lnb, b1t, b2t,
                       wqrt, spool, scr, ps1, ps2, ps3,
                       TT, TS, STT, CP, MM, r32, li)

            if tap_layer is not None and li == tap_layer:
                nc.sync.dma_start(tap[:], hT[:])

        # ============ pool + final LN + head ============
        lns2 = wpool.tile([128, FC, 1], F32, tag="lns")
        nc.sync.dma_start(lns2[:], dram["lns2"][:].rearrange("f p o -> p f o"))
        lnb2 = wpool.tile([128, FC, 1], F32, tag="lnb")
        nc.sync.dma_start(lnb2[:], dram["lnb2"][:].rearrange("f p o -> p f o"))
        whead = wpool.tile([128, FC, NCLS], F32, tag="whead")
        nc.sync.dma_start(whead[:], dram["whead"][:].rearrange("f p c -> p f c"))
        bhead = wpool.tile([1, NCLS], F32, tag="bhead")
        nc.sync.dma_start(bhead[:], dram["bhead"][:])

        for b in range(BL):
            pooled = scr.tile([128, FC], F32, tag="pooled")
            for ch in range(FC):
                nc.vector.reduce_sum(pooled[:, ch:ch + 1], hT[:, ch, b],
                                     axis=AX.X)
            nc.scalar.mul(pooled[:], pooled[:], 1.0 / N)
            psa = ps3.tile([128, 256], F32, tag="ps3")
            for ch in range(FC):
                MM(psa[0:1, 0:1], pooled[:, ch:ch + 1], cst["ones128"][:],
                   start=(ch == 0), stop=(ch == FC - 1))
            sq = scr.tile([128, FC], F32, tag="poolsq")
            nc.scalar.activation(sq[:], pooled[:], AF.Square)
            psb = ps3.tile([128, 256], F32, tag="ps3")
            for ch in range(FC):
                MM(psb[0:1, 0:1], sq[:, ch:ch + 1], cst["ones128"][:],
                   start=(ch == 0), stop=(ch == FC - 1))
            mean = scr.tile([1, 1], F32, tag="fmean")
            nc.scalar.mul(mean[:], psa[0:1, 0:1], 1.0 / C)
            msq = scr.tile([1, 1], F32, tag="fmsq")
            nc.scalar.mul(msq[:], psb[0:1, 0:1], 1.0 / C)
            m2 = scr.tile([1, 1], F32, tag="fm2")
            nc.scalar.activation(m2[:], mean[:], AF.Square)
            var = scr.tile([1, 1], F32, tag="fvar")
            TT(var[:], msq[:], m2[:], op=ALU.subtract)
            TS(var[:], var[:], 1e-5, None, op0=ALU.add)
            sdv = scr.tile([1, 1], F32, tag="fsdv")
            nc.scalar.activation(sdv[:], var[:], AF.Sqrt)
            rstd = scr.tile([1, 1], F32, tag="frstd")
            nc.vector.reciprocal(rstd[:], sdv[:])
            two = scr.tile([1, 2], F32, tag="ftwo")
            CP(two[:, 0:1], mean[:])
            CP(two[:, 1:2], rstd[:])
            psc = ps3.tile([128, 256], F32, tag="ps3")
            MM(psc[:, 0:2], cst["onesrow"][:], two[:], start=True, stop=True)
            meanb = scr.tile([128, 1], F32, tag="fmeanb")
            rstdb = scr.tile([128, 1], F32, tag="frstdb")
            CP(meanb[:], psc[:, 0:1])
            CP(rstdb[:], psc[:, 1:2])
            pnorm = scr.tile([128, FC], F32, tag="pnorm")
            TS(pnorm[:], pooled[:], meanb[:, 0:1], None, op0=ALU.subtract)
            TS(pnorm[:], pnorm[:], rstdb[:, 0:1], None, op0=ALU.mult)
            for ch in range(FC):
                TS(pnorm[:, ch:ch + 1], pnorm[:, ch:ch + 1],
                   lns2[:, ch], lnb2[:, ch], op0=ALU.mult, op1=ALU.add)
            psh = ps3.tile([128, 256], F32, tag="ps3")
            for ch in range(FC):
                MM(psh[0:1, 0:NCLS], pnorm[:, ch:ch + 1], whead[:, ch],
                   start=(ch == 0), stop=(ch == FC - 1))
            ologit = scr.tile([1, NCLS], F32, tag="ologit")
            TT(ologit[:], psh[0:1, 0:NCLS], bhead[:], op=ALU.add)
            nc.sync.dma_start(out[b:b + 1, :], ologit[:])


def _eigensolve_and_attend(nc, tc, cst, smc, dram, hT, Sg, Vg, U0g, U0r,
                           wproj, bproj, epool, spool, scr, ps1, ps2, ps3,
                           TT, TS, STT, CP, MM, r32, li, g, bidx):
    Xg = epool.tile([128, GM, 2, K], F32, tag="Xg")
    for m in range(GM):
        for kb in range(2):
            CP(Xg[:, m, kb], cst["x0c"][:, kb])
    Xt = epool.tile([128, NG4, N], F32, tag="Xt")

    def cheb_round():
        for m in range(GM):
            Tp = epool.tile([128, 2, K], F32, tag="Tp")
            Tc = epool.tile([128, 2, K], F32, tag="Tc")
            for kb in range(2):
                CP(Tp[:, kb], Xg[:, m, kb])
            psx = ps2.tile([128, 512], F32, tag="ps2")
            for ob in range(2):
                for kb in range(2):
                    MM(psx[:, K * ob:K * (ob + 1)],
                       Sg[:, m, kb, 128 * ob:128 * (ob + 1)],
                       Tp[:, kb], start=(kb == 0), stop=(kb == 1))
            for kb in range(2):
                nc.scalar.mul(Tc[:, kb], psx[:, K * kb:K * (kb + 1)], 0.5)
            for _ in range(D_CHEB - 1):
                psy = ps2.tile([128, 512], F32, tag="ps2")
                for ob in range(2):
                    for kb in range(2):
                        MM(psy[:, K * ob:K * (ob + 1)],
                           Sg[:, m, kb, 128 * ob:128 * (ob + 1)],
                           Tc[:, kb], start=(kb == 0), stop=False)
                    MM(psy[:, K * ob:K * (ob + 1)], cst["ni128"][:], Tp[:, ob],
                       start=False, stop=True)
                for kb in range(2):
                    CP(Tp[:, kb], Tc[:, kb])
                    CP(Tc[:, kb], psy[:, K * kb:K * (kb + 1)])
            for kb in range(2):
                CP(Xg[:, m, kb], Tc[:, kb])

    def deflate():
        for m in range(GM):
            g4, r = m // 4, m % 4
            psp = ps3.tile([128, 256], F32, tag="ps3")
            for kb in range(2):
                MM(psp[32 * r:32 * r + 1, 0:K], U0g[:, m, kb], Xg[:, m, kb],
                   start=(kb == 0), stop=(kb == 1), tile_position=(0, 32 * r))
            pr = scr.tile([128, K], F32, tag="pr")
            nc.scalar.mul(pr[32 * r:32 * r + 1, :], psp[32 * r:32 * r + 1, 0:K], -1.0)
            psd = ps2.tile([128, 512], F32, tag="ps2")
            for kb in range(2):
                MM(psd[:, K * kb:K * (kb + 1)], cst["i128"][:], Xg[:, m, kb],
                   start=True, stop=False)
                MM(psd[:, K * kb:K * (kb + 1)],
                   U0r[32 * r:32 * r + 1, g4, 128 * kb:128 * (kb + 1)],
                   pr[32 * r:32 * r + 1, :],
                   start=False, stop=True, tile_position=(32 * r, 0))
            for kb in range(2):
                CP(Xg[:, m, kb], psd[:, K * kb:K * (kb + 1)])

    def small_mm(dst, lhsT, rhs):
        psb = ps2.tile([128, 512], F32, tag="ps2")
        pv = psb[:].rearrange("p (a b) -> p a b", a=4)
        for g4 in range(NG4):
            MM(pv[:, g4], lhsT[:, g4], rhs[:, g4], start=True, stop=True)
        CP(dst[:], pv[:])

    def build_xt():
        for m in range(GM):
            g4, r = m // 4, m % 4
            ptx = ps2.tile([128, 512], F32, tag="ps2")
            for kb in range(2):
                MM(ptx[32 * r:32 * r + K, 128 * kb:128 * (kb + 1)],
                   Xg[:, m, kb], cst["i128"][:],
                   start=True, stop=True, tile_position=(0, 32 * r))
            CP(Xt[32 * r:32 * r + K, g4], ptx[32 * r:32 * r + K, 0:256])

    def bcast_permat(v44):
        ptq = ps3.tile([128, 256], F32, tag="ps3")
        MM(ptq[0:4, 0:4], v44[:], cst["i128"][0:4, 0:4], start=True, stop=True)
        v44t = scr.tile([4, 4], F32, tag="v44t")
        CP(v44t[:], ptq[0:4, 0:4])
        ptw = ps3.tile([128, 256], F32, tag="ps3")
        MM(ptw[:, 0:4], cst["exp32"][:], v44t[:], start=True, stop=True)
        ob = scr.tile([128, 4], F32, tag="permat")
        CP(ob[:], ptw[:, 0:4])
        return ob

    def permat_max(src):
        ptm = ps3.tile([128, 256], F32, tag="ps3")
        MM(ptm[0:4, 0:128], src[:], cst["i128"][:], start=True, stop=True)
        tr = scr.tile([4, 128], F32, tag="tr44")
        CP(tr[:], ptm[0:4, 0:128])
        mx = scr.tile([4, 4], F32, tag="mx44")
        nc.vector.reduce_max(mx[:], tr[:].rearrange("p (a b) -> p a b", a=4),
                             axis=AX.X)
        return mx

    def whiten_pass(eps, steps):
        Gt = epool.tile([128, 4, 128], F32, tag="Gt")
        psg = ps2.tile([128, 512], F32, tag="ps2")
        pvg = psg[:].rearrange("p (a b) -> p a b", a=4)
        for m in range(GM):
            g4, r = m // 4, m % 4
            for kb in range(2):
                MM(psg[32 * r:32 * r + K,
                       128 * g4 + 32 * r:128 * g4 + 32 * r + K],
                   Xg[:, m, kb], Xg[:, m, kb],
                   start=(kb == 0), stop=(kb == 1), tile_position=(0, 32 * r))
        TT(Gt[:], pvg[:], smc("sm_blk"), op=ALU.mult)
        dg = scr.tile([128, 4, 128], F32, tag="smA")
        TT(dg[:], Gt[:], smc("sm_eye"), op=ALU.mult)
        dgt = scr.tile([128, 4], F32, tag="dgt")
        nc.vector.reduce_sum(dgt[:], dg[:], axis=AX.X)
        TS(dgt[:], dgt[:], 1e-12, None, op0=ALU.add)
        sq = scr.tile([128, 4], F32, tag="sq")
        nc.scalar.activation(sq[:], dgt[:], AF.Sqrt)
        srec = scr.tile([128, 4], F32, tag="srec")
        nc.vector.reciprocal(srec[:], sq[:])
        for g4 in range(NG4):
            TS(Gt[:, g4], Gt[:, g4], srec[:, g4:g4 + 1], None, op0=ALU.mult)
        GtT = epool.tile([128, 4, 128], F32, tag="GtT")
        small_mm(GtT, Gt, smc("sm_eye"))
        for g4 in range(NG4):
            TS(GtT[:, g4], GtT[:, g4], srec[:, g4:g4 + 1], None, op0=ALU.mult)
        STT(Gt[:], smc("sm_eye"), eps, GtT[:], op0=ALU.mult, op1=ALU.add)
        brs = scr.tile([128, 4], F32, tag="brs4")
        nc.vector.tensor_reduce(brs[:], Gt[:], axis=AX.X, op=ALU.add,
                                apply_absolute_value=True)
        mx = permat_max(brs)
        rec = scr.tile([4, 4], F32, tag="rec44")
        nc.vector.reciprocal(rec[:], mx[:])
        bre = bcast_permat(rec)
        for g4 in range(NG4):
            TS(Gt[:, g4], Gt[:, g4], bre[:, g4:g4 + 1], None, op0=ALU.mult)
        Yt = epool.tile([128, 4, 128], F32, tag="Yt")
        Zt = epool.tile([128, 4, 128], F32, tag="Zt")
        Tt = epool.tile([128, 4, 128], F32, tag="Tt")
        CP(Yt[:], Gt[:])
        CP(Zt[:], smc("sm_eye"))
        for _ in range(steps):
            psb = ps2.tile([128, 512], F32, tag="ps2")
            pv2 = psb[:].rearrange("p (a b) -> p a b", a=4)
            for g4 in range(NG4):
                MM(pv2[:, g4], Zt[:, g4], Yt[:, g4], start=True, stop=True)
            STT(Tt[:], pv2[:], -0.5, smc("sm_eye15"), op0=ALU.mult, op1=ALU.add)
            small_mm(Yt, Yt, Tt)
            small_mm(Zt, Tt, Zt)
        sqb = scr.tile([4, 4], F32, tag="sqb44")
        nc.scalar.activation(sqb[:], mx[:], AF.Sqrt)
        nc.vector.reciprocal(sqb[:], sqb[:])
        sbe = bcast_permat(sqb)
        for g4 in range(NG4):
            TS(Zt[:, g4], Zt[:, g4], srec[:, g4:g4 + 1], None, op0=ALU.mult)
            TS(Zt[:, g4], Zt[:, g4], sbe[:, g4:g4 + 1], None, op0=ALU.mult)
        build_xt()
        for m in range(GM):
            g4, r = m // 4, m % 4
            psx = ps2.tile([128, 512], F32, tag="ps2")
            for kb in range(2):
                MM(psx[:, K * kb:K * (kb + 1)],
                   Xt[32 * r:32 * r + K, g4, 128 * kb:128 * (kb + 1)],
                   Zt[32 * r:32 * r + K, g4, 32 * r:32 * r + K],
                   start=True, stop=True, tile_position=(32 * r, 0))
            for kb in range(2):
                CP(Xg[:, m, kb], psx[:, K * kb:K * (kb + 1)])

    # ---------------- rounds ----------------
    for rr in range(ROUNDS[li]):
        cheb_round()
        deflate()
        tc.strict_bb_all_engine_barrier()
        whiten_pass(*NS_MID)
        tc.strict_bb_all_engine_barrier()
        for m in range(GM):
            for kb in range(2):
                TT(Xg[:, m, kb], Xg[:, m, kb], cst["r0c"][:, kb], op=ALU.add)
    deflate()
    tc.strict_bb_all_engine_barrier()
    whiten_pass(*NS_F1)
    tc.strict_bb_all_engine_barrier()
    whiten_pass(*NS_F2)
    tc.strict_bb_all_engine_barrier()
    if nc._dbg and li == 0 and g == 0:
        nc.sync.dma_start(nc._dbg["d_X"][:], Xg[:])

    # ---------------- B build ----------------
    Bt = epool.tile([128, 4, 128], F32, tag="Bt")
    Wt = epool.tile([128, 4, 128], F32, tag="Wt")
    psgB = ps2.tile([128, 512], F32, tag="ps2")
    pvB = psgB[:].rearrange("p (a b) -> p a b", a=4)
    for m in range(GM):
        g4, r = m // 4, m % 4
        psz = ps1.tile([128, 512], F32, tag="ps1")
        for ob in range(2):
            for kb in range(2):
                MM(psz[:, K * ob:K * (ob + 1)],
                   Sg[:, m, kb, 128 * ob:128 * (ob + 1)],
                   Xg[:, m, kb], start=(kb == 0), stop=(kb == 1))
        Zc = epool.tile([128, 2, K], F32, tag="Zc")
        for kb in range(2):
            CP(Zc[:, kb], psz[:, K * kb:K * (kb + 1)])
        for kb in range(2):
            MM(psgB[32 * r:32 * r + K,
                    128 * g4 + 32 * r:128 * g4 + 32 * r + K],
               Xg[:, m, kb], Zc[:, kb],
               start=(kb == 0), stop=(kb == 1), tile_position=(0, 32 * r))
    TT(Bt[:], pvB[:], smc("sm_blk"), op=ALU.mult)
    BtT = epool.tile([128, 4, 128], F32, tag="GtT")
    small_mm(BtT, Bt, smc("sm_eye"))
    TT(Bt[:], Bt[:], BtT[:], op=ALU.add)
    nc.scalar.mul(Bt[:], Bt[:], 0.5)
    CP(Wt[:], smc("sm_eye"))
    if nc._dbg and li == 0 and g == 0:
        nc.sync.dma_start(nc._dbg["d_B"][:], Bt[:].rearrange("p a b -> p (a b)"))
    build_xt()

    # ---------------- all-pairs diagonalizer ----------------
    Et = epool.tile([128, 4, 128], F32, tag="Et")
    Qt = epool.tile([128, 4, 128], F32, tag="Qt")
    QtT = epool.tile([128, 4, 128], F32, tag="QtT")
    M1 = epool.tile([128, 4, 128], F32, tag="M1")
    for it in range(DIAG_ITERS):
        dmat = scr.tile([128, 4, 128], F32, tag="smA")
        TT(dmat[:], Bt[:], smc("sm_eye"), op=ALU.mult)
        dcol = scr.tile([128, 4], F32, tag="dcol")
        nc.vector.reduce_sum(dcol[:], dmat[:], axis=AX.X)
        Drow = epool.tile([128, 4, 128], F32, tag="GtT")
        small_mm(Drow, smc("sm_blk"), dmat)
        dd = scr.tile([128, 4, 128], F32, tag="smB")
        for g4 in range(NG4):
            STT(dd[:, g4], Drow[:, g4], dcol[:, g4:g4 + 1],
                smc("sm_blk")[:, g4], op0=ALU.subtract, op1=ALU.mult)
        sgn = scr.tile([128, 4, 128], F32, tag="smC")
        TS(sgn[:], dd[:], 0.0, None, op0=ALU.is_ge)
        STT(dd[:], sgn[:], 2e-9, dd[:], op0=ALU.mult, op1=ALU.add)
        TS(dd[:], dd[:], -1e-9, None, op0=ALU.add)
        nc.vector.reciprocal(dd[:], dd[:])
        TT(Et[:], Bt[:], dd[:], op=ALU.mult)
        TT(Et[:], Et[:], smc("sm_offblk"), op=ALU.mult)
        TS(Et[:], Et[:], DIAG_CAP, None, op0=ALU.min)
        TS(Et[:], Et[:], -DIAG_CAP, None, op0=ALU.max)
        EtT = scr.tile([128, 4, 128], F32, tag="smA")
        small_mm(EtT, Et, smc("sm_eye"))
        TT(Et[:], Et[:], EtT[:], op=ALU.subtract)
        nc.scalar.mul(Et[:], Et[:], 0.5)
        ern = scr.tile([128, 4], F32, tag="ern4")
        nc.vector.tensor_reduce(ern[:], Et[:], axis=AX.X, op=ALU.add,
                                apply_absolute_value=True)
        emx = permat_max(ern)
        TS(emx[:], emx[:], 1e-9, None, op0=ALU.add)
        esc = scr.tile([4, 4], F32, tag="esc44")
        nc.vector.reciprocal(esc[:], emx[:])
        TS(esc[:], esc[:], DIAG_DAMP, 1.0, op0=ALU.mult, op1=ALU.min)
        ebe = bcast_permat(esc)
        for g4 in range(NG4):
            TS(Et[:, g4], Et[:, g4], ebe[:, g4:g4 + 1], None, op0=ALU.mult)
        TT(Qt[:], smc("sm_eye"), Et[:], op=ALU.add)
        TT(QtT[:], smc("sm_eye"), Et[:], op=ALU.subtract)
        for _ in range(2):
            P_ = scr.tile([128, 4, 128], F32, tag="smA")
            small_mm(P_, Qt, Qt)
            STT(M1[:], P_[:], -0.5, smc("sm_eye15"), op0=ALU.mult, op1=ALU.add)
            Qn = scr.tile([128, 4, 128], F32, tag="smB")
            small_mm(Qn, QtT, M1)
            small_mm(QtT, M1, QtT)
            CP(Qt[:], Qn[:])
        BQ = scr.tile([128, 4, 128], F32, tag="smA")
        small_mm(BQ, Bt, Qt)
        small_mm(Bt, Qt, BQ)
        small_mm(Wt, Qt, Wt)

    # ---------------- rank selection -> Ut ----------------
    dmat = scr.tile([128, 4, 128], F32, tag="smA")
    TT(dmat[:], Bt[:], smc("sm_eye"), op=ALU.mult)
    dcol = scr.tile([128, 4], F32, tag="dcol")
    nc.vector.reduce_sum(dcol[:], dmat[:], axis=AX.X)
    TS(dcol[:], dcol[:], cst["sm_tie"][:, 0:1], None, op0=ALU.add)
    dmat2 = scr.tile([128, 4, 128], F32, tag="smB")
    for g4 in range(NG4):
        TS(dmat2[:, g4], smc("sm_eye")[:, g4], dcol[:, g4:g4 + 1], None,
           op0=ALU.mult)
    Drow = epool.tile([128, 4, 128], F32, tag="GtT")
    small_mm(Drow, smc("sm_blk"), dmat2)
    Cc = scr.tile([128, 4, 128], F32, tag="smC")
    for g4 in range(NG4):
        STT(Cc[:, g4], Drow[:, g4], dcol[:, g4:g4 + 1],
            smc("sm_blk")[:, g4], op0=ALU.is_lt, op1=ALU.mult)
    rnk = scr.tile([128, 4], F32, tag="rnk")
    nc.vector.reduce_sum(rnk[:], Cc[:], axis=AX.X)
    Sel = scr.tile([128, 4, 128], F32, tag="smC")
    for g4 in range(NG4):
        TS(Sel[:, g4], smc("sm_iotasel")[:, g4], rnk[:, g4:g4 + 1], None,
           op0=ALU.is_equal)
    Pt = epool.tile([128, 4, 128], F32, tag="M1")
    small_mm(Pt, Wt, Sel)

    Utg = epool.tile([128, NG4, N], F32, tag="Utg")
    for g4 in range(NG4):
        psu = ps1.tile([128, 512], F32, tag="ps1")
        for r in range(4):
            MM(psu[32 * r:32 * r + N_EIGS, 0:N],
               Pt[32 * r:32 * r + K, g4, 32 * r:32 * r + N_EIGS],
               Xt[32 * r:32 * r + K, g4],
               start=True, stop=True, tile_position=(32 * r, 32 * r))
        CP(Utg[:, g4], psu[:, 0:N])

    un = scr.tile([128, NG4], F32, tag="un")
    for g4 in range(NG4):
        usq = scr.tile([128, N], F32, tag="ge")
        nc.scalar.activation(usq[:], Utg[:, g4], AF.Square)
        nc.vector.reduce_sum(un[:, g4:g4 + 1], usq[:], axis=AX.X)
    TS(un[:], un[:], 1e-30, None, op0=ALU.add)
    uns = scr.tile([128, NG4], F32, tag="uns")
    nc.scalar.activation(uns[:], un[:], AF.Sqrt)
    nc.vector.reciprocal(uns[:], uns[:])
    for g4 in range(NG4):
        TS(Utg[:, g4], Utg[:, g4], uns[:, g4:g4 + 1], None, op0=ALU.mult)

    if nc._dbg and li == 0 and g == 0:
        nc.sync.dma_start(nc._dbg["d_Ut"][:], Utg[:])
    # ---------------- group attention + out + proj ----------------
    aoutT = [spool.tile([CH, N], F32, tag="aoutT" + str(i), name="aoutT" + str(i)) for i in range(2)]
    for m in range(GM):
        g4, r = m // 4, m % 4
        bloc = m // H
        h = m % H
        psga = ps1.tile([128, 512], F32, tag="ps1")
        MM(psga[0:32, 0:N], cst["combs4"][32 * r:32 * r + N_EIGS, :],
           Utg[32 * r:32 * r + N_EIGS, g4],
           start=True, stop=True, tile_position=(32 * r, 0))
        gmx = scr.tile([32, 1], F32, tag="gmx")
        nc.vector.reduce_max(gmx[:], psga[0:32, 0:N], axis=AX.X)
        ngm = scr.tile([32, 1], F32, tag="ngm")
        nc.scalar.mul(ngm[:], gmx[:], -1.0)
        ga = scr.tile([32, N], F32, tag="ga")
        nc.scalar.activation(ga[:], psga[0:32, 0:N], AF.Exp, bias=ngm[:, 0:1])
        gs = scr.tile([32, 1], F32, tag="gs")
        nc.vector.reduce_sum(gs[:], ga[:], axis=AX.X)
        gr = scr.tile([32, 1], F32, tag="gr")
        nc.vector.reciprocal(gr[:], gs[:])
        TS(ga[:], ga[:], gr[:, 0:1], None, op0=ALU.mult)
        gaT = scr.tile([128, 2, 32], F32, tag="gaT")
        psgt = ps3.tile([128, 256], F32, tag="ps3")
        for kb in range(2):
            MM(psgt[:, 32 * kb:32 * (kb + 1)], ga[:, 128 * kb:128 * (kb + 1)],
               cst["i128"][0:32, 0:32], start=True, stop=True)
            CP(gaT[:, kb], psgt[:, 32 * kb:32 * (kb + 1)])
        psoh = ps3.tile([128, 256], F32, tag="ps3")
        for kb in range(2):
            MM(psoh[0:CH, 0:32], Vg[:, m, kb], gaT[:, kb],
               start=(kb == 0), stop=(kb == 1))
        CP(aoutT[bloc][:, 32 * h:32 * (h + 1)], psoh[0:CH, 0:32])

    for bloc in range(2):
        b = bidx[bloc]
        for ch in range(FC):
            psj = ps1.tile([128, 512], F32, tag="ps1")
            MM(psj[:, 0:N], r32(wproj[:, 128 * ch:128 * (ch + 1)]),
               r32(aoutT[bloc][:]), start=True, stop=True)
            TS(hT[:, ch, b], psj[:, 0:N], bproj[:, ch], None, op0=ALU.add)


def _mlp_block(nc, tc, cst, dram, hT, lns, lnb, b1t, b2t,
               wqrt, spool, scr, ps1, ps2, ps3, TT, TS, STT, CP, MM, r32, li):
    for pair in range(2):
        bb = [2 * pair, 2 * pair + 1]
        hnp = spool.tile([128, FC, 2, N], F32, tag="Sg")
        for bloc, b in enumerate(bb):
            pss = ps3.tile([128, 256], F32, tag="ps3")
            ps2s = ps3.tile([128, 256], F32, tag="ps3")
            for ch in range(FC):
                MM(pss[0:1, 0:128], cst["ones128"][:], hT[:, ch, b, 0:128],
                   start=(ch == 0), stop=(ch == FC - 1))
            # NOTE: sums are computed in two half-token blocks (psum free 128)
            for ch in range(FC):
                MM(ps2s[0:1, 0:128], cst["ones128"][:], hT[:, ch, b, 128:256],
                   start=(ch == 0), stop=(ch == FC - 1))
            sums = scr.tile([1, N], F32, tag="lsums")
            CP(sums[:, 0:128], pss[0:1, 0:128])
            CP(sums[:, 128:256], ps2s[0:1, 0:128])
            sq1 = scr.tile([128, N], F32, tag="lt1")
            psq1 = ps3.tile([128, 256], F32, tag="ps3")
            psq2 = ps3.tile([128, 256], F32, tag="ps3")
            for ch in range(FC):
                nc.scalar.activation(sq1[:], hT[:, ch, b], AF.Square)
                MM(psq1[0:1, 0:128], cst["ones128"][:], sq1[:, 0:128],
                   start=(ch == 0), stop=(ch == FC - 1))
                MM(psq2[0:1, 0:128], cst["ones128"][:], sq1[:, 128:256],
                   start=(ch == 0), stop=(ch == FC - 1))
            sqs = scr.tile([1, N], F32, tag="lsqs")
            CP(sqs[:, 0:128], psq1[0:1, 0:128])
            CP(sqs[:, 128:256], psq2[0:1, 0:128])
            mean = scr.tile([1, N], F32, tag="lmean")
            nc.scalar.mul(mean[:], sums[:], 1.0 / C)
            msq = scr.tile([1, N], F32, tag="lmsq")
            nc.scalar.mul(msq[:], sqs[:], 1.0 / C)
            m2 = scr.tile([1, N], F32, tag="lm2")
            nc.scalar.activation(m2[:], mean[:], AF.Square)
            var = scr.tile([1, N], F32, tag="lvar")
            TT(var[:], msq[:], m2[:], op=ALU.subtract)
            TS(var[:], var[:], 1e-5, None, op0=ALU.add)
            sdv = scr.tile([1, N], F32, tag="lsdv")
            nc.scalar.activation(sdv[:], var[:], AF.Sqrt)
            rst = scr.tile([1, N], F32, tag="lrst")
            nc.vector.reciprocal(rst[:], sdv[:])
            mrs = scr.tile([1, N], F32, tag="lmrs")
            TT(mrs[:], mean[:], rst[:], op=ALU.mult)
            psb1 = ps1.tile([128, 512], F32, tag="ps1")
            MM(psb1[:, 0:N], cst["onesrow"][:], rst[:], start=True, stop=True)
            MM(psb1[:, 256:512], cst["onesrow"][:], mrs[:], start=True, stop=True)
            rstB = scr.tile([128, N], F32, tag="lrstB")
            mrsB = scr.tile([128, N], F32, tag="lmrsB")
            CP(rstB[:], psb1[:, 0:N])
            CP(mrsB[:], psb1[:, 256:512])
            for ch in range(FC):
                t1 = scr.tile([128, N], F32, tag="lt1")
                TT(t1[:], hT[:, ch, b], rstB[:], op=ALU.mult)
                TT(t1[:], t1[:], mrsB[:], op=ALU.subtract)
                TS(hnp[:, ch, bloc], t1[:], lns[:, ch], lnb[:, ch],
                   op0=ALU.mult, op1=ALU.add)

        # MLP quarters: a1q = gelu(psum-accum) ; mlp2 accumulates into hT
        for q in range(4):
            w1q = wqrt.tile([128, FC, 768], F32, tag="wqrt")
            nc.sync.dma_start(
                w1q[:], dram["w1"][li, :, :, 768 * q:768 * (q + 1)]
                .rearrange("f p c -> p f c"))
            a1q = spool.tile([128, 6, 2, N], F32, tag="Vg")
            for jc in range(6):
                jg = 6 * q + jc
                psm = ps1.tile([128, 512], F32, tag="ps1")
                for ch in range(FC):
                    MM(psm[:],
                       r32(w1q[:, ch, 128 * jc:128 * (jc + 1)]),
                       r32(hnp[:, ch].rearrange("p a b -> p (a b)")),
                       start=(ch == 0), stop=(ch == FC - 1))
                nc.scalar.activation(
                    a1q[:, jc].rearrange("p a b -> p (a b)"), psm[:],
                    AF.Gelu, bias=b1t[:, jg])
            w2q = wqrt.tile([128, FC, 768], F32, tag="wqrt")
            nc.sync.dma_start(
                w2q[:], dram["w2"][li, 6 * q:6 * (q + 1)]
                .rearrange("j p c -> p j c"))
            for mc in range(FC):
                psm2 = ps1.tile([128, 512], F32, tag="ps1")
                for jc in range(6):
                    MM(psm2[:],
                       r32(w2q[:, jc, 128 * mc:128 * (mc + 1)]),
                       r32(a1q[:, jc].rearrange("p a b -> p (a b)")),
                       start=(jc == 0), stop=(jc == 5))
                hv = hT[:, mc, 2 * pair:2 * pair + 2, :].rearrange("p a b -> p (a b)")
                if q == 0:
                    STT(hv, psm2[:], b2t[:, mc], hv, op0=ALU.add, op1=ALU.add)
                else:
                    TT(hv, hv, psm2[:], op=ALU.add)
            # residual base already in hT (attn out); mlp adds on top


FC_A1 = 6


# ====================== host wrapper ======================

_NC_CACHE = {}


def _get_nc(n_layers=D, tap_layer=None):
    key = (n_layers, tap_layer)
    if key not in _NC_CACHE:
        _NC_CACHE[key] = build_nc(n_layers, tap_layer)
    return _NC_CACHE[key]


def kernel(**inputs):
    x = np.asarray(inputs["x"], dtype=np.float32)
    w = _prep_weights(inputs)
    xp = _prep_x(x)                      # [B, 12, 256]
    nc = _get_nc()
    in_maps = []
    for c in range(NCORES):
        im = {}
        im["xpt"] = np.ascontiguousarray(xp[BL * c:BL * (c + 1)])
        for kname, arr in w.items():
            im[kname] = arr
        for cn, arr in CONSTS.items():
            im[cn] = arr
        in_maps.append(im)
    res = run_bass_kernel_spmd(nc, in_maps, core_ids=list(range(NCORES)))
    outs = [res.results[c]["out"] for c in range(NCORES)]
    return np.concatenate(outs, axis=0).astype(np.float32)


if __name__ == "__main__":
    rng = np.random.RandomState(0)
    print("building 1-layer nc for smoke...")
    import time
    t0 = time.time()
    nc = build_nc(n_layers=1, tap_layer=0)
    print("build+compile took", time.time() - t0)



# revision 3
# speedup vs baseline: 1.2222x; 1.2222x over previous
"""Trainium2 Bass kernel for nn_CIFARViT (spectral group-attention ViT).

kernel(**inputs) takes the FULL unsharded inputs (keys as in setup_inputs),
shards the batch over 8 NeuronCores (pure data parallel, 4 images/core),
runs one Bass program per core via run_bass_kernel_spmd, and gathers the
full [32, 10] output.
"""
import sys
import os

if '/opt/trn_rl_repo' not in sys.path:
    sys.path.insert(0, '/opt/trn_rl_repo')
os.environ.setdefault("NEURON_RT_RESET_CORES", "1")

import numpy as np

import concourse.bass as bass
import concourse.mybir as mybir
from concourse import bacc
from concourse.tile import TileContext
from concourse.bass_utils import run_bass_kernel_spmd

F32 = mybir.dt.float32
F32R = mybir.dt.float32r
AF = mybir.ActivationFunctionType
ALU = mybir.AluOpType
AX = mybir.AxisListType

D = 8
C = 768
H = 8
B = 32
BL = 4
NCORES = 8
N = 256
CH = 96
N_EIGS = 5
NCLS = 10
FC = C // 128
SCALE_QK = CH ** -0.5

K = 16
ROUNDS = [6, 6, 3, 3, 3, 3, 3, 3]
if os.environ.get("KERNEL_R1"):
    ROUNDS = [int(os.environ["KERNEL_R1"])] * 8
D_CHEB = 2
A_INT, B_INT = 0.95, 1.35
C_CENT = (B_INT + A_INT) / 2.0
CHEB_S = 4.0 / (B_INT - A_INT)
ETA = 1e-2
NS_MID = (3e-3, 8)
NS_F1 = (1e-3, 10)
NS_F2 = (1e-5, 6)
DIAG_ITERS = 10
DIAG_CAP = 0.45
DIAG_DAMP = 0.4
TIE_D = 1e-5
GM = 16
NG4 = 4

_SEED = 1234


def _host_consts():
    rng = np.random.RandomState(_SEED)
    X0 = rng.randn(N, K).astype(np.float32)
    R0 = (rng.randn(N, K).astype(np.float32) / np.sqrt(N)) * ETA
    cs = {}
    cs["x0c"] = np.ascontiguousarray(X0.reshape(2, 128, K).transpose(1, 0, 2))
    cs["r0c"] = np.ascontiguousarray(R0.reshape(2, 128, K).transpose(1, 0, 2))
    cs["i128"] = np.eye(128, dtype=np.float32)
    cs["ni128"] = -np.eye(128, dtype=np.float32)
    cs["onesrow"] = np.ones((1, 128), dtype=np.float32)
    cs["ones128"] = np.ones((128, 1), dtype=np.float32)
    dm = np.zeros((2, 128, 256), dtype=np.float32)
    for r in range(2):
        for p in range(128):
            dm[r, p, 128 * r + p] = 1.0
    cs["dmask"] = np.ascontiguousarray(dm.transpose(1, 0, 2))
    cs["vmask"] = np.ascontiguousarray(1.0 - dm.transpose(1, 0, 2))
    import itertools
    combs = np.array(list(itertools.product([1.0, -1.0], repeat=N_EIGS)),
                     dtype=np.float32)
    c4 = np.zeros((128, 32), dtype=np.float32)
    for r in range(4):
        c4[32 * r:32 * r + N_EIGS, :] = combs.T
    cs["combs4"] = c4
    eye = np.zeros((128, 512), dtype=np.float32)
    blk = np.zeros((128, 512), dtype=np.float32)
    iot = np.full((128, 512), 1e9, dtype=np.float32)
    for g4 in range(4):
        for r in range(4):
            for i in range(K):
                eye[32 * r + i, 128 * g4 + 32 * r + i] = 1.0
                blk[32 * r + i, 128 * g4 + 32 * r:128 * g4 + 32 * r + K] = 1.0
                for j in range(N_EIGS):
                    iot[32 * r + i, 128 * g4 + 32 * r + j] = float(j)
    cs["sm_eye"] = eye
    cs["sm_eye15"] = 1.5 * eye
    cs["sm_blk"] = blk
    cs["sm_offblk"] = blk - eye
    cs["sm_iotasel"] = iot
    cs["sm_tie"] = (np.arange(128, dtype=np.float32) % 32 * TIE_D).reshape(128, 1)
    e32 = np.zeros((4, 128), dtype=np.float32)
    for r in range(4):
        e32[r, 32 * r:32 * (r + 1)] = 1.0
    cs["exp32"] = e32
    return cs


CONSTS = _host_consts()


def _prep_weights(inputs):
    w = {}
    qkv_w = np.asarray(inputs["qkv_w"], dtype=np.float32)
    w["wq"] = np.ascontiguousarray(qkv_w[:, :, 0:C]).reshape(D, FC, 128, C)
    w["wv"] = np.ascontiguousarray(qkv_w[:, :, 2 * C:3 * C]).reshape(D, FC, 128, C)
    w["wproj"] = np.ascontiguousarray(np.asarray(inputs["proj_w"], dtype=np.float32))
    w["bproj"] = np.asarray(inputs["proj_b"], dtype=np.float32).reshape(D, FC, 128, 1).copy()
    w["lns"] = np.asarray(inputs["g_ln_s"], dtype=np.float32).reshape(D, FC, 128, 1).copy()
    w["lnb"] = np.asarray(inputs["g_ln_b"], dtype=np.float32).reshape(D, FC, 128, 1).copy()
    w["w1"] = np.ascontiguousarray(np.asarray(inputs["mlp_w1"], dtype=np.float32)).reshape(D, FC, 128, 4 * C)
    w["b1"] = np.asarray(inputs["mlp_b1"], dtype=np.float32).reshape(D, 24, 128, 1).copy()
    w["w2"] = np.ascontiguousarray(np.asarray(inputs["mlp_w2"], dtype=np.float32)).reshape(D, 24, 128, C)
    w["b2"] = np.asarray(inputs["mlp_b2"], dtype=np.float32).reshape(D, FC, 128, 1).copy()
    w["lns2"] = np.asarray(inputs["ln_s"], dtype=np.float32).reshape(FC, 128, 1).copy()
    w["lnb2"] = np.asarray(inputs["ln_b"], dtype=np.float32).reshape(FC, 128, 1).copy()
    w["whead"] = np.asarray(inputs["head_w"], dtype=np.float32).reshape(FC, 128, NCLS).copy()
    w["bhead"] = np.asarray(inputs["head_b"], dtype=np.float32).reshape(1, NCLS).copy()
    pw = np.asarray(inputs["patch_w"], dtype=np.float32).reshape(C, 12)
    w["pwT"] = np.ascontiguousarray(pw.T)
    w["pbias"] = np.asarray(inputs["patch_b"], dtype=np.float32).reshape(FC, 128, 1).copy()
    pos = np.asarray(inputs["pos_emb"], dtype=np.float32).reshape(N, C)
    w["posT"] = np.ascontiguousarray(pos.T).reshape(FC, 128, N)
    return w


def _prep_x(x):
    Bb = x.shape[0]
    xp = np.asarray(x, dtype=np.float32).reshape(Bb, 3, 16, 2, 16, 2)
    xp = xp.transpose(0, 2, 4, 1, 3, 5).reshape(Bb, N, 12)
    return np.ascontiguousarray(np.swapaxes(xp, 1, 2))


# ====================== device program ======================

def build_nc(n_layers=D, tap_layer=None):
    nc = bacc.Bacc("TRN2", target_bir_lowering=False, debug=False)
    dram = {}

    def din(name, shape):
        dram[name] = nc.dram_tensor(name, list(shape), F32, kind="ExternalInput")

    din("xpt", (BL, 12, N))
    din("pwT", (12, C))
    din("pbias", (FC, 128, 1))
    din("posT", (FC, 128, N))
    din("wq", (D, FC, 128, C))
    din("wv", (D, FC, 128, C))
    din("wproj", (D, CH, C))
    din("bproj", (D, FC, 128, 1))
    din("lns", (D, FC, 128, 1))
    din("lnb", (D, FC, 128, 1))
    din("w1", (D, FC, 128, 4 * C))
    din("b1", (D, 24, 128, 1))
    din("w2", (D, 24, 128, C))
    din("b2", (D, FC, 128, 1))
    din("lns2", (FC, 128, 1))
    din("lnb2", (FC, 128, 1))
    din("whead", (FC, 128, NCLS))
    din("bhead", (1, NCLS))
    for cn, arr in CONSTS.items():
        din(cn, arr.shape)

    out = nc.dram_tensor("out", [BL, NCLS], F32, kind="ExternalOutput")
    tap = None
    if tap_layer is not None:
        tap = nc.dram_tensor("tap", [128, FC, BL, N], F32, kind="ExternalOutput")
    if os.environ.get("KERNEL_DEBUG_TAPS"):
        nc._dbg = {
            "d_emb": nc.dram_tensor("d_emb", [128, FC, BL, N], F32, kind="ExternalOutput"),
            "d_S": nc.dram_tensor("d_S", [128, GM, 2, N], F32, kind="ExternalOutput"),
            "d_V": nc.dram_tensor("d_V", [128, GM, 2, CH], F32, kind="ExternalOutput"),
            "d_u0": nc.dram_tensor("d_u0", [128, GM, 2, 1], F32, kind="ExternalOutput"),
            "d_X": nc.dram_tensor("d_X", [128, GM, 2, K], F32, kind="ExternalOutput"),
            "d_B": nc.dram_tensor("d_B", [128, 512], F32, kind="ExternalOutput"),
            "d_Ut": nc.dram_tensor("d_Ut", [128, NG4, N], F32, kind="ExternalOutput"),
            "d_att": nc.dram_tensor("d_att", [128, FC, BL, N], F32, kind="ExternalOutput"),
        }
    else:
        nc._dbg = {}

    with TileContext(nc) as tc:
        _emit(nc, tc, dram, out, tap, n_layers, tap_layer)
    nc.compile()
    return nc


def _emit(nc, tc, dram, out, tap, n_layers, tap_layer):
    import contextlib
    es = contextlib.ExitStack()
    with es:
        persist = es.enter_context(tc.tile_pool(name="persist", bufs=1))
        wpool = es.enter_context(tc.tile_pool(name="wpool", bufs=1))
        wstr = es.enter_context(tc.tile_pool(name="wstr", bufs=2))
        wqrt = es.enter_context(tc.tile_pool(name="wqrt", bufs=1))
        spool = es.enter_context(tc.tile_pool(name="spool", bufs=1))
        epool = es.enter_context(tc.tile_pool(name="epool", bufs=1))
        scr = es.enter_context(tc.tile_pool(name="scr", bufs=1))
        rowp = es.enter_context(tc.tile_pool(name="rowp", bufs=1))
        ps1 = es.enter_context(tc.tile_pool(name="ps1", bufs=2, space="PSUM"))
        ps2 = es.enter_context(tc.tile_pool(name="ps2", bufs=2, space="PSUM"))
        ps3 = es.enter_context(tc.tile_pool(name="ps3", bufs=2, space="PSUM"))

        cst = {}
        for cn, arr in CONSTS.items():
            t = persist.tile(list(arr.shape), F32, tag="c_" + cn)
            cst[cn] = t
            nc.sync.dma_start(t[:], dram[cn][:])

        def smc(name):
            return cst[name][:].rearrange("p (a b) -> p a b", a=4)

        hT = persist.tile([128, FC, BL, N], F32, tag="hT")

        def r32(ap):
            return ap

        TT = nc.vector.tensor_tensor
        TS = nc.vector.tensor_scalar
        STT = nc.vector.scalar_tensor_tensor
        CP = nc.vector.tensor_copy
        MM = nc.tensor.matmul

        # ============ patch embed ============
        xpt = persist.tile([12, BL, N], F32, tag="xpt")
        nc.sync.dma_start(xpt[:], dram["xpt"][:].rearrange("b k n -> k b n"))
        pwT = persist.tile([12, C], F32, tag="pwT")
        nc.sync.dma_start(pwT[:], dram["pwT"][:])
        pbias = persist.tile([128, FC, 1], F32, tag="pbias")
        nc.sync.dma_start(pbias[:], dram["pbias"][:].rearrange("f p o -> p f o"))
        posT = persist.tile([128, FC, N], F32, tag="posT")
        nc.sync.dma_start(posT[:], dram["posT"][:].rearrange("f p n -> p f n"))

        for b in range(BL):
            for ch in range(FC):
                ps = ps1.tile([128, 512], F32, tag="ps1")
                MM(ps[:, 0:N], r32(pwT[:, 128 * ch:128 * (ch + 1)]),
                   r32(xpt[:, b]), start=True, stop=True)
                tmp = scr.tile([128, N], F32, tag="ge")
                TS(tmp[:], ps[:, 0:N], pbias[:, ch], None, op0=ALU.add)
                TT(hT[:, ch, b], tmp[:], posT[:, ch], op=ALU.add)

        if nc._dbg:
            nc.sync.dma_start(nc._dbg["d_emb"][:], hT[:])
        # ===================== layers =====================
        for li in range(n_layers):
            wq = wpool.tile([128, FC, C], F32, tag="wq")
            nc.sync.dma_start(wq[:], dram["wq"][li].rearrange("f p c -> p f c"))
            wproj = wpool.tile([CH, C], F32, tag="wproj")
            nc.sync.dma_start(wproj[:], dram["wproj"][li])
            bproj = wpool.tile([128, FC, 1], F32, tag="bproj")
            nc.sync.dma_start(bproj[:], dram["bproj"][li].rearrange("f p o -> p f o"))
            lns = wpool.tile([128, FC, 1], F32, tag="lns")
            nc.sync.dma_start(lns[:], dram["lns"][li].rearrange("f p o -> p f o"))
            lnb = wpool.tile([128, FC, 1], F32, tag="lnb")
            nc.sync.dma_start(lnb[:], dram["lnb"][li].rearrange("f p o -> p f o"))
            b1t = wpool.tile([128, 24, 1], F32, tag="b1t")
            nc.sync.dma_start(b1t[:], dram["b1"][li].rearrange("j p o -> p j o"))
            b2t = wpool.tile([128, FC, 1], F32, tag="b2t")
            nc.sync.dma_start(b2t[:], dram["b2"][li].rearrange("f p o -> p f o"))

            for g in range(2):
                bidx = [2 * g, 2 * g + 1]
                Sg = spool.tile([128, GM, 2, N], F32, tag="Sg")
                Vg = spool.tile([128, GM, 2, CH], F32, tag="Vg")
                U0g = spool.tile([128, GM, 2, 1], F32, tag="U0g")
                U0r = spool.tile([128, NG4, N], F32, tag="U0r")

                # ---- v = hT^T Wv per matrix (wv streamed, SBUF-accumulate) --
                for ch in range(FC):
                    wvc = wstr.tile([128, C], F32, tag="wvc")
                    nc.sync.dma_start(wvc[:], dram["wv"][li, ch])
                    for m in range(GM):
                        b = bidx[m // H]
                        h = m % H
                        for rb in range(2):
                            psv = ps3.tile([128, 256], F32, tag="ps3")
                            MM(psv[:, 0:CH],
                               r32(hT[:, ch, b, 128 * rb:128 * (rb + 1)]),
                               r32(wvc[:, CH * h:CH * (h + 1)]),
                               start=True, stop=True)
                            if ch == 0:
                                CP(Vg[:, m, rb], psv[:, 0:CH])
                            else:
                                TT(Vg[:, m, rb], Vg[:, m, rb], psv[:, 0:CH],
                                   op=ALU.add)

                # ---- phase A (exp table): logits -> G into Sg ----
                for m in range(GM):
                    b = bidx[m // H]
                    h = m % H
                    qT = scr.tile([CH, N], F32, tag="qT")
                    psq = ps1.tile([128, 512], F32, tag="ps1")
                    for ch in range(FC):
                        MM(psq[0:CH, 0:N],
                           r32(wq[:, ch, CH * h:CH * (h + 1)]),
                           r32(hT[:, ch, b]),
                           start=(ch == 0), stop=(ch == FC - 1))
                    CP(qT[:], psq[0:CH, 0:N])
                    psl = ps2.tile([128, 512], F32, tag="ps2")
                    for rb in range(2):
                        MM(psl[:, 256 * rb:256 * (rb + 1)],
                           r32(qT[:, 128 * rb:128 * (rb + 1)]),
                           r32(qT[:]), start=True, stop=True)
                    rm = scr.tile([128, 1], F32, tag="rm")
                    nc.vector.reduce_max(rm[:], psl[:], axis=AX.X)
                    pst = ps3.tile([128, 256], F32, tag="ps3")
                    MM(pst[0:1, 0:128], rm[:], cst["i128"][:], start=True, stop=True)
                    mv = scr.tile([1, 1], F32, tag="mv")
                    nc.vector.reduce_max(mv[:], pst[0:1, 0:128], axis=AX.X)
                    pbc = ps3.tile([128, 256], F32, tag="ps3")
                    MM(pbc[:, 0:1], cst["onesrow"][:], mv[:], start=True, stop=True)
                    negm = scr.tile([128, 1], F32, tag="negm")
                    nc.scalar.mul(negm[:], pbc[:, 0:1], -SCALE_QK)
                    for rb in range(2):
                        ge = scr.tile([128, N], F32, tag="ge")
                        nc.scalar.activation(ge[:], psl[:, 256 * rb:256 * (rb + 1)],
                                             AF.Exp, bias=negm[:, 0:1],
                                             scale=SCALE_QK)
                        TT(Sg[:, m, rb], ge[:], cst["vmask"][:, rb], op=ALU.mult)

                # ---- phase B (sqrt table): G -> Ahat, u0 ----
                for m in range(GM):
                    g4, rr_ = m // 4, m % 4
                    rsum = scr.tile([128, 2], F32, tag="rsum")
                    for rb in range(2):
                        nc.vector.reduce_sum(rsum[:, rb:rb + 1], Sg[:, m, rb],
                                             axis=AX.X)
                    invr = scr.tile([128, 2], F32, tag="invr")
                    nc.vector.reciprocal(invr[:], rsum[:])
                    ptv = ps3.tile([128, 256], F32, tag="ps3")
                    for ob in range(2):
                        for kb in range(2):
                            MM(ptv[:, ob:ob + 1],
                               Sg[:, m, kb, 128 * ob:128 * (ob + 1)],
                               invr[:, kb:kb + 1],
                               start=(kb == 0), stop=(kb == 1))
                    deg = scr.tile([128, 2], F32, tag="deg")
                    TS(deg[:], ptv[:, 0:2], 0.5, 0.5, op0=ALU.mult, op1=ALU.add)
                    sd = scr.tile([128, 2], F32, tag="sd")
                    nc.scalar.activation(sd[:], deg[:], AF.Sqrt)
                    wv_ = scr.tile([128, 2], F32, tag="wv_")
                    nc.vector.reciprocal(wv_[:], sd[:])
                    alpha = scr.tile([128, 2], F32, tag="alpha")
                    TT(alpha[:], wv_[:], invr[:], op=ALU.mult)
                    nc.scalar.mul(alpha[:], alpha[:], 0.5)
                    psn = ps3.tile([128, 256], F32, tag="ps3")
                    for kb in range(2):
                        MM(psn[0:1, 0:1], deg[:, kb:kb + 1], cst["ones128"][:],
                           start=(kb == 0), stop=(kb == 1))
                    nrm = scr.tile([1, 1], F32, tag="nrm")
                    nc.scalar.activation(nrm[:], psn[0:1, 0:1], AF.Sqrt)
                    rn = scr.tile([1, 1], F32, tag="rn")
                    nc.vector.reciprocal(rn[:], nrm[:])
                    pbc = ps3.tile([128, 256], F32, tag="ps3")
                    MM(pbc[:, 0:1], cst["onesrow"][:], rn[:], start=True, stop=True)
                    rnb = scr.tile([128, 1], F32, tag="rnb")
                    CP(rnb[:], pbc[:, 0:1])
                    for kb in range(2):
                        TS(U0g[:, m, kb], sd[:, kb:kb + 1], rnb[:, 0:1], None,
                           op0=ALU.mult)
                    arow = rowp.tile([1, N], F32, tag="arow")
                    brow = rowp.tile([1, N], F32, tag="brow")
                    for (row_t, col_t) in ((arow, alpha), (brow, wv_)):
                        ptr2 = ps3.tile([128, 256], F32, tag="ps3")
                        for kb in range(2):
                            MM(ptr2[0:1, 128 * kb:128 * (kb + 1)],
                               col_t[:, kb:kb + 1], cst["i128"][:],
                               start=True, stop=True)
                        CP(row_t[:], ptr2[0:1, :])
                    ptr3 = ps3.tile([128, 256], F32, tag="ps3")
                    for kb in range(2):
                        MM(ptr3[32 * rr_:32 * rr_ + 1, 128 * kb:128 * (kb + 1)],
                           U0g[:, m, kb], cst["i128"][:],
                           start=True, stop=True, tile_position=(0, 32 * rr_))
                    CP(U0r[32 * rr_:32 * rr_ + 1, g4], ptr3[32 * rr_:32 * rr_ + 1, :])
                    for rb in range(2):
                        pso = ps1.tile([128, 512], F32, tag="ps1")
                        MM(pso[:, 0:N], arow[:, 128 * rb:128 * (rb + 1)],
                           brow[:], start=True, stop=False)
                        MM(pso[:, 0:N], brow[:, 128 * rb:128 * (rb + 1)],
                           arow[:], start=False, stop=True)
                        go = scr.tile([128, N], F32, tag="ge")
                        TT(go[:], Sg[:, m, rb], pso[:, 0:N], op=ALU.mult)
                        STT(Sg[:, m, rb], go[:], -CHEB_S, cst["dmask"][:, rb],
                            op0=ALU.mult, op1=ALU.bypass)
                        STT(Sg[:, m, rb], cst["dmask"][:, rb],
                            CHEB_S * (1.0 - C_CENT), Sg[:, m, rb],
                            op0=ALU.mult, op1=ALU.add)

                if nc._dbg and li == 0 and g == 0:
                    nc.sync.dma_start(nc._dbg["d_S"][:], Sg[:])
                    nc.sync.dma_start(nc._dbg["d_V"][:], Vg[:])
                    nc.sync.dma_start(nc._dbg["d_u0"][:], U0g[:])
                _eigensolve_and_attend(
                    nc, tc, cst, smc, dram, hT, Sg, Vg, U0g, U0r,
                    wproj, bproj, epool, spool, scr, ps1, ps2, ps3,
                    TT, TS, STT, CP, MM, r32, li, g, bidx)

            if nc._dbg and li == 0:
                nc.sync.dma_start(nc._dbg["d_att"][:], hT[:])
            _mlp_block(nc, tc, cst, dram, hT, lns, lnb, b1t, b2t,
                       wqrt, spool, scr, ps1, ps2, ps3,
                       TT, TS, STT, CP, MM, r32, li)

            if tap_layer is not None and li == tap_layer:
                nc.sync.dma_start(tap[:], hT[:])

        # ============ pool + final LN + head ============
        lns2 = wpool.tile([128, FC, 1], F32, tag="lns")
        nc.sync.dma_start(lns2[:], dram["lns2"][:].rearrange("f p o -> p f o"))
        lnb2 = wpool.tile([128, FC, 1], F32, tag="lnb")
        nc.sync.dma_start(lnb2[:], dram["lnb2"][:].rearrange("f p o -> p f o"))
        whead = wpool.tile([128, FC, NCLS], F32, tag="whead")
        nc.sync.dma_start(whead[:], dram["whead"][:].rearrange("f p c -> p f c"))
        bhead = wpool.tile([1, NCLS], F32, tag="bhead")
        nc.sync.dma_start(bhead[:], dram["bhead"][:])

        for b in range(BL):
            pooled = scr.tile([128, FC], F32, tag="pooled")
            for ch in range(FC):
                nc.vector.reduce_sum(pooled[:, ch:ch + 1], hT[:, ch, b],
                                     axis=AX.X)
            nc.scalar.mul(pooled[:], pooled[:], 1.0 / N)
            psa = ps3.tile([128, 256], F32, tag="ps3")
            for ch in range(FC):
                MM(psa[0:1, 0:1], pooled[:, ch:ch + 1], cst["ones128"][:],
                   start=(ch == 0), stop=(ch == FC - 1))
            sq = scr.tile([128, FC], F32, tag="poolsq")
            nc.scalar.activation(sq[:], pooled[:], AF.Square)
            psb = ps3.tile([128, 256], F32, tag="ps3")
            for ch in range(FC):
                MM(psb[0:1, 0:1], sq[:, ch:ch + 1], cst["ones128"][:],
                   start=(ch == 0), stop=(ch == FC - 1))
            mean = scr.tile([1, 1], F32, tag="fmean")
            nc.scalar.mul(mean[:], psa[0:1, 0:1], 1.0 / C)
            msq = scr.tile([1, 1], F32, tag="fmsq")
            nc.scalar.mul(msq[:], psb[0:1, 0:1], 1.0 / C)
            m2 = scr.tile([1, 1], F32, tag="fm2")
            nc.scalar.activation(m2[:], mean[:], AF.Square)
            var = scr.tile([1, 1], F32, tag="fvar")
            TT(var[:], msq[:], m2[:], op=ALU.subtract)
            TS(var[:], var[:], 1e-5, None, op0=ALU.add)
            sdv = scr.tile([1, 1], F32, tag="fsdv")
            nc.scalar.activation(sdv[:], var[:], AF.Sqrt)
            rstd = scr.tile([1, 1], F32, tag="frstd")
            nc.vector.reciprocal(rstd[:], sdv[:])
            two = scr.tile([1, 2], F32, tag="ftwo")
            CP(two[:, 0:1], mean[:])
            CP(two[:, 1:2], rstd[:])
            psc = ps3.tile([128, 256], F32, tag="ps3")
            MM(psc[:, 0:2], cst["onesrow"][:], two[:], start=True, stop=True)
            meanb = scr.tile([128, 1], F32, tag="fmeanb")
            rstdb = scr.tile([128, 1], F32, tag="frstdb")
            CP(meanb[:], psc[:, 0:1])
            CP(rstdb[:], psc[:, 1:2])
            pnorm = scr.tile([128, FC], F32, tag="pnorm")
            TS(pnorm[:], pooled[:], meanb[:, 0:1], None, op0=ALU.subtract)
            TS(pnorm[:], pnorm[:], rstdb[:, 0:1], None, op0=ALU.mult)
            for ch in range(FC):
                TS(pnorm[:, ch:ch + 1], pnorm[:, ch:ch + 1],
                   lns2[:, ch], lnb2[:, ch], op0=ALU.mult, op1=ALU.add)
            psh = ps3.tile([128, 256], F32, tag="ps3")
            for ch in range(FC):
                MM(psh[0:1, 0:NCLS], pnorm[:, ch:ch + 1], whead[:, ch],
                   start=(ch == 0), stop=(ch == FC - 1))
            ologit = scr.tile([1, NCLS], F32, tag="ologit")
            TT(ologit[:], psh[0:1, 0:NCLS], bhead[:], op=ALU.add)
            nc.sync.dma_start(out[b:b + 1, :], ologit[:])


def _eigensolve_and_attend(nc, tc, cst, smc, dram, hT, Sg, Vg, U0g, U0r,
                           wproj, bproj, epool, spool, scr, ps1, ps2, ps3,
                           TT, TS, STT, CP, MM, r32, li, g, bidx):
    Xg = epool.tile([128, GM, 2, K], F32, tag="Xg")
    for m in range(GM):
        for kb in range(2):
            CP(Xg[:, m, kb], cst["x0c"][:, kb])
    Xt = epool.tile([128, NG4, N], F32, tag="Xt")

    def cheb_round():
        for m in range(GM):
            Tp = epool.tile([128, 2, K], F32, tag="Tp")
            Tc = epool.tile([128, 2, K], F32, tag="Tc")
            for kb in range(2):
                CP(Tp[:, kb], Xg[:, m, kb])
            psx = ps2.tile([128, 512], F32, tag="ps2")
            for ob in range(2):
                for kb in range(2):
                    MM(psx[:, K * ob:K * (ob + 1)],
                       Sg[:, m, kb, 128 * ob:128 * (ob + 1)],
                       Tp[:, kb], start=(kb == 0), stop=(kb == 1))
            for kb in range(2):
                nc.scalar.mul(Tc[:, kb], psx[:, K * kb:K * (kb + 1)], 0.5)
            for _ in range(D_CHEB - 1):
                psy = ps2.tile([128, 512], F32, tag="ps2")
                for ob in range(2):
                    for kb in range(2):
                        MM(psy[:, K * ob:K * (ob + 1)],
                           Sg[:, m, kb, 128 * ob:128 * (ob + 1)],
                           Tc[:, kb], start=(kb == 0), stop=False)
                    MM(psy[:, K * ob:K * (ob + 1)], cst["ni128"][:], Tp[:, ob],
                       start=False, stop=True)
                for kb in range(2):
                    CP(Tp[:, kb], Tc[:, kb])
                    CP(Tc[:, kb], psy[:, K * kb:K * (kb + 1)])
            for kb in range(2):
                CP(Xg[:, m, kb], Tc[:, kb])

    def deflate():
        for m in range(GM):
            g4, r = m // 4, m % 4
            psp = ps3.tile([128, 256], F32, tag="ps3")
            for kb in range(2):
                MM(psp[32 * r:32 * r + 1, 0:K], U0g[:, m, kb], Xg[:, m, kb],
                   start=(kb == 0), stop=(kb == 1), tile_position=(0, 32 * r))
            pr = scr.tile([128, K], F32, tag="pr")
            nc.scalar.mul(pr[32 * r:32 * r + 1, :], psp[32 * r:32 * r + 1, 0:K], -1.0)
            psd = ps2.tile([128, 512], F32, tag="ps2")
            for kb in range(2):
                MM(psd[:, K * kb:K * (kb + 1)], cst["i128"][:], Xg[:, m, kb],
                   start=True, stop=False)
                MM(psd[:, K * kb:K * (kb + 1)],
                   U0r[32 * r:32 * r + 1, g4, 128 * kb:128 * (kb + 1)],
                   pr[32 * r:32 * r + 1, :],
                   start=False, stop=True, tile_position=(32 * r, 0))
            for kb in range(2):
                CP(Xg[:, m, kb], psd[:, K * kb:K * (kb + 1)])

    def small_mm(dst, lhsT, rhs):
        psb = ps2.tile([128, 512], F32, tag="ps2")
        pv = psb[:].rearrange("p (a b) -> p a b", a=4)
        for g4 in range(NG4):
            MM(pv[:, g4], lhsT[:, g4], rhs[:, g4], start=True, stop=True)
        CP(dst[:], pv[:])

    def build_xt():
        for m in range(GM):
            g4, r = m // 4, m % 4
            ptx = ps2.tile([128, 512], F32, tag="ps2")
            for kb in range(2):
                MM(ptx[32 * r:32 * r + K, 128 * kb:128 * (kb + 1)],
                   Xg[:, m, kb], cst["i128"][:],
                   start=True, stop=True, tile_position=(0, 32 * r))
            CP(Xt[32 * r:32 * r + K, g4], ptx[32 * r:32 * r + K, 0:256])

    def bcast_permat(v44):
        ptq = ps3.tile([128, 256], F32, tag="ps3")
        MM(ptq[0:4, 0:4], v44[:], cst["i128"][0:4, 0:4], start=True, stop=True)
        v44t = scr.tile([4, 4], F32, tag="v44t")
        CP(v44t[:], ptq[0:4, 0:4])
        ptw = ps3.tile([128, 256], F32, tag="ps3")
        MM(ptw[:, 0:4], cst["exp32"][:], v44t[:], start=True, stop=True)
        ob = scr.tile([128, 4], F32, tag="permat")
        CP(ob[:], ptw[:, 0:4])
        return ob

    def permat_max(src):
        ptm = ps3.tile([128, 256], F32, tag="ps3")
        MM(ptm[0:4, 0:128], src[:], cst["i128"][:], start=True, stop=True)
        tr = scr.tile([4, 128], F32, tag="tr44")
        CP(tr[:], ptm[0:4, 0:128])
        mx = scr.tile([4, 4], F32, tag="mx44")
        nc.vector.reduce_max(mx[:], tr[:].rearrange("p (a b) -> p a b", a=4),
                             axis=AX.X)
        return mx

    def whiten_pass(eps, steps):
        Gt = epool.tile([128, 4, 128], F32, tag="Gt")
        psg = ps2.tile([128, 512], F32, tag="ps2")
        pvg = psg[:].rearrange("p (a b) -> p a b", a=4)
        for m in range(GM):
            g4, r = m // 4, m % 4
            for kb in range(2):
                MM(psg[32 * r:32 * r + K,
                       128 * g4 + 32 * r:128 * g4 + 32 * r + K],
                   Xg[:, m, kb], Xg[:, m, kb],
                   start=(kb == 0), stop=(kb == 1), tile_position=(0, 32 * r))
        TT(Gt[:], pvg[:], smc("sm_blk"), op=ALU.mult)
        dg = scr.tile([128, 4, 128], F32, tag="smA")
        TT(dg[:], Gt[:], smc("sm_eye"), op=ALU.mult)
        dgt = scr.tile([128, 4], F32, tag="dgt")
        nc.vector.reduce_sum(dgt[:], dg[:], axis=AX.X)
        TS(dgt[:], dgt[:], 1e-12, None, op0=ALU.add)
        sq = scr.tile([128, 4], F32, tag="sq")
        nc.scalar.activation(sq[:], dgt[:], AF.Sqrt)
        srec = scr.tile([128, 4], F32, tag="srec")
        nc.vector.reciprocal(srec[:], sq[:])
        for g4 in range(NG4):
            TS(Gt[:, g4], Gt[:, g4], srec[:, g4:g4 + 1], None, op0=ALU.mult)
        GtT = epool.tile([128, 4, 128], F32, tag="GtT")
        small_mm(GtT, Gt, smc("sm_eye"))
        for g4 in range(NG4):
            TS(GtT[:, g4], GtT[:, g4], srec[:, g4:g4 + 1], None, op0=ALU.mult)
        STT(Gt[:], smc("sm_eye"), eps, GtT[:], op0=ALU.mult, op1=ALU.add)
        brs = scr.tile([128, 4], F32, tag="brs4")
        nc.vector.tensor_reduce(brs[:], Gt[:], axis=AX.X, op=ALU.add,
                                apply_absolute_value=True)
        mx = permat_max(brs)
        rec = scr.tile([4, 4], F32, tag="rec44")
        nc.vector.reciprocal(rec[:], mx[:])
        bre = bcast_permat(rec)
        for g4 in range(NG4):
            TS(Gt[:, g4], Gt[:, g4], bre[:, g4:g4 + 1], None, op0=ALU.mult)
        Yt = epool.tile([128, 4, 128], F32, tag="Yt")
        Zt = epool.tile([128, 4, 128], F32, tag="Zt")
        Tt = epool.tile([128, 4, 128], F32, tag="Tt")
        CP(Yt[:], Gt[:])
        CP(Zt[:], smc("sm_eye"))
        for _ in range(steps):
            psb = ps2.tile([128, 512], F32, tag="ps2")
            pv2 = psb[:].rearrange("p (a b) -> p a b", a=4)
            for g4 in range(NG4):
                MM(pv2[:, g4], Zt[:, g4], Yt[:, g4], start=True, stop=True)
            STT(Tt[:], pv2[:], -0.5, smc("sm_eye15"), op0=ALU.mult, op1=ALU.add)
            small_mm(Yt, Yt, Tt)
            small_mm(Zt, Tt, Zt)
        sqb = scr.tile([4, 4], F32, tag="sqb44")
        nc.scalar.activation(sqb[:], mx[:], AF.Sqrt)
        nc.vector.reciprocal(sqb[:], sqb[:])
        sbe = bcast_permat(sqb)
        for g4 in range(NG4):
            TS(Zt[:, g4], Zt[:, g4], srec[:, g4:g4 + 1], None, op0=ALU.mult)
            TS(Zt[:, g4], Zt[:, g4], sbe[:, g4:g4 + 1], None, op0=ALU.mult)
        build_xt()
        for m in range(GM):
            g4, r = m // 4, m % 4
            psx = ps2.tile([128, 512], F32, tag="ps2")
            for kb in range(2):
                MM(psx[:, K * kb:K * (kb + 1)],
                   Xt[32 * r:32 * r + K, g4, 128 * kb:128 * (kb + 1)],
                   Zt[32 * r:32 * r + K, g4, 32 * r:32 * r + K],
                   start=True, stop=True, tile_position=(32 * r, 0))
            for kb in range(2):
                CP(Xg[:, m, kb], psx[:, K * kb:K * (kb + 1)])

    # ---------------- rounds ----------------
    for rr in range(ROUNDS[li]):
        cheb_round()
        deflate()
        pass  # barrier removed
        whiten_pass(*NS_MID)
        pass  # barrier removed
        for m in range(GM):
            for kb in range(2):
                TT(Xg[:, m, kb], Xg[:, m, kb], cst["r0c"][:, kb], op=ALU.add)
    deflate()
    pass  # barrier removed
    whiten_pass(*NS_F1)
    pass  # barrier removed
    whiten_pass(*NS_F2)
    pass  # barrier removed
    if nc._dbg and li == 0 and g == 0:
        nc.sync.dma_start(nc._dbg["d_X"][:], Xg[:])

    # ---------------- B build ----------------
    Bt = epool.tile([128, 4, 128], F32, tag="Bt")
    Wt = epool.tile([128, 4, 128], F32, tag="Wt")
    psgB = ps2.tile([128, 512], F32, tag="ps2")
    pvB = psgB[:].rearrange("p (a b) -> p a b", a=4)
    for m in range(GM):
        g4, r = m // 4, m % 4
        psz = ps1.tile([128, 512], F32, tag="ps1")
        for ob in range(2):
            for kb in range(2):
                MM(psz[:, K * ob:K * (ob + 1)],
                   Sg[:, m, kb, 128 * ob:128 * (ob + 1)],
                   Xg[:, m, kb], start=(kb == 0), stop=(kb == 1))
        Zc = epool.tile([128, 2, K], F32, tag="Zc")
        for kb in range(2):
            CP(Zc[:, kb], psz[:, K * kb:K * (kb + 1)])
        for kb in range(2):
            MM(psgB[32 * r:32 * r + K,
                    128 * g4 + 32 * r:128 * g4 + 32 * r + K],
               Xg[:, m, kb], Zc[:, kb],
               start=(kb == 0), stop=(kb == 1), tile_position=(0, 32 * r))
    TT(Bt[:], pvB[:], smc("sm_blk"), op=ALU.mult)
    BtT = epool.tile([128, 4, 128], F32, tag="GtT")
    small_mm(BtT, Bt, smc("sm_eye"))
    TT(Bt[:], Bt[:], BtT[:], op=ALU.add)
    nc.scalar.mul(Bt[:], Bt[:], 0.5)
    CP(Wt[:], smc("sm_eye"))
    if nc._dbg and li == 0 and g == 0:
        nc.sync.dma_start(nc._dbg["d_B"][:], Bt[:].rearrange("p a b -> p (a b)"))
    build_xt()

    # ---------------- all-pairs diagonalizer ----------------
    Et = epool.tile([128, 4, 128], F32, tag="Et")
    Qt = epool.tile([128, 4, 128], F32, tag="Qt")
    QtT = epool.tile([128, 4, 128], F32, tag="QtT")
    M1 = epool.tile([128, 4, 128], F32, tag="M1")
    for it in range(DIAG_ITERS):
        dmat = scr.tile([128, 4, 128], F32, tag="smA")
        TT(dmat[:], Bt[:], smc("sm_eye"), op=ALU.mult)
        dcol = scr.tile([128, 4], F32, tag="dcol")
        nc.vector.reduce_sum(dcol[:], dmat[:], axis=AX.X)
        Drow = epool.tile([128, 4, 128], F32, tag="GtT")
        small_mm(Drow, smc("sm_blk"), dmat)
        dd = scr.tile([128, 4, 128], F32, tag="smB")
        for g4 in range(NG4):
            STT(dd[:, g4], Drow[:, g4], dcol[:, g4:g4 + 1],
                smc("sm_blk")[:, g4], op0=ALU.subtract, op1=ALU.mult)
        sgn = scr.tile([128, 4, 128], F32, tag="smC")
        TS(sgn[:], dd[:], 0.0, None, op0=ALU.is_ge)
        STT(dd[:], sgn[:], 2e-9, dd[:], op0=ALU.mult, op1=ALU.add)
        TS(dd[:], dd[:], -1e-9, None, op0=ALU.add)
        nc.vector.reciprocal(dd[:], dd[:])
        TT(Et[:], Bt[:], dd[:], op=ALU.mult)
        TT(Et[:], Et[:], smc("sm_offblk"), op=ALU.mult)
        TS(Et[:], Et[:], DIAG_CAP, None, op0=ALU.min)
        TS(Et[:], Et[:], -DIAG_CAP, None, op0=ALU.max)
        EtT = scr.tile([128, 4, 128], F32, tag="smA")
        small_mm(EtT, Et, smc("sm_eye"))
        TT(Et[:], Et[:], EtT[:], op=ALU.subtract)
        nc.scalar.mul(Et[:], Et[:], 0.5)
        ern = scr.tile([128, 4], F32, tag="ern4")
        nc.vector.tensor_reduce(ern[:], Et[:], axis=AX.X, op=ALU.add,
                                apply_absolute_value=True)
        emx = permat_max(ern)
        TS(emx[:], emx[:], 1e-9, None, op0=ALU.add)
        esc = scr.tile([4, 4], F32, tag="esc44")
        nc.vector.reciprocal(esc[:], emx[:])
        TS(esc[:], esc[:], DIAG_DAMP, 1.0, op0=ALU.mult, op1=ALU.min)
        ebe = bcast_permat(esc)
        for g4 in range(NG4):
            TS(Et[:, g4], Et[:, g4], ebe[:, g4:g4 + 1], None, op0=ALU.mult)
        TT(Qt[:], smc("sm_eye"), Et[:], op=ALU.add)
        TT(QtT[:], smc("sm_eye"), Et[:], op=ALU.subtract)
        for _ in range(2):
            P_ = scr.tile([128, 4, 128], F32, tag="smA")
            small_mm(P_, Qt, Qt)
            STT(M1[:], P_[:], -0.5, smc("sm_eye15"), op0=ALU.mult, op1=ALU.add)
            Qn = scr.tile([128, 4, 128], F32, tag="smB")
            small_mm(Qn, QtT, M1)
            small_mm(QtT, M1, QtT)
            CP(Qt[:], Qn[:])
        BQ = scr.tile([128, 4, 128], F32, tag="smA")
        small_mm(BQ, Bt, Qt)
        small_mm(Bt, Qt, BQ)
        small_mm(Wt, Qt, Wt)

    # ---------------- rank selection -> Ut ----------------
    dmat = scr.tile([128, 4, 128], F32, tag="smA")
    TT(dmat[:], Bt[:], smc("sm_eye"), op=ALU.mult)
    dcol = scr.tile([128, 4], F32, tag="dcol")
    nc.vector.reduce_sum(dcol[:], dmat[:], axis=AX.X)
    TS(dcol[:], dcol[:], cst["sm_tie"][:, 0:1], None, op0=ALU.add)
    dmat2 = scr.tile([128, 4, 128], F32, tag="smB")
    for g4 in range(NG4):
        TS(dmat2[:, g4], smc("sm_eye")[:, g4], dcol[:, g4:g4 + 1], None,
           op0=ALU.mult)
    Drow = epool.tile([128, 4, 128], F32, tag="GtT")
    small_mm(Drow, smc("sm_blk"), dmat2)
    Cc = scr.tile([128, 4, 128], F32, tag="smC")
    for g4 in range(NG4):
        STT(Cc[:, g4], Drow[:, g4], dcol[:, g4:g4 + 1],
            smc("sm_blk")[:, g4], op0=ALU.is_lt, op1=ALU.mult)
    rnk = scr.tile([128, 4], F32, tag="rnk")
    nc.vector.reduce_sum(rnk[:], Cc[:], axis=AX.X)
    Sel = scr.tile([128, 4, 128], F32, tag="smC")
    for g4 in range(NG4):
        TS(Sel[:, g4], smc("sm_iotasel")[:, g4], rnk[:, g4:g4 + 1], None,
           op0=ALU.is_equal)
    Pt = epool.tile([128, 4, 128], F32, tag="M1")
    small_mm(Pt, Wt, Sel)

    Utg = epool.tile([128, NG4, N], F32, tag="Utg")
    for g4 in range(NG4):
        psu = ps1.tile([128, 512], F32, tag="ps1")
        for r in range(4):
            MM(psu[32 * r:32 * r + N_EIGS, 0:N],
               Pt[32 * r:32 * r + K, g4, 32 * r:32 * r + N_EIGS],
               Xt[32 * r:32 * r + K, g4],
               start=True, stop=True, tile_position=(32 * r, 32 * r))
        CP(Utg[:, g4], psu[:, 0:N])

    un = scr.tile([128, NG4], F32, tag="un")
    for g4 in range(NG4):
        usq = scr.tile([128, N], F32, tag="ge")
        nc.scalar.activation(usq[:], Utg[:, g4], AF.Square)
        nc.vector.reduce_sum(un[:, g4:g4 + 1], usq[:], axis=AX.X)
    TS(un[:], un[:], 1e-30, None, op0=ALU.add)
    uns = scr.tile([128, NG4], F32, tag="uns")
    nc.scalar.activation(uns[:], un[:], AF.Sqrt)
    nc.vector.reciprocal(uns[:], uns[:])
    for g4 in range(NG4):
        TS(Utg[:, g4], Utg[:, g4], uns[:, g4:g4 + 1], None, op0=ALU.mult)

    if nc._dbg and li == 0 and g == 0:
        nc.sync.dma_start(nc._dbg["d_Ut"][:], Utg[:])
    # ---------------- group attention + out + proj ----------------
    aoutT = [spool.tile([CH, N], F32, tag="aoutT" + str(i), name="aoutT" + str(i)) for i in range(2)]
    for m in range(GM):
        g4, r = m // 4, m % 4
        bloc = m // H
        h = m % H
        psga = ps1.tile([128, 512], F32, tag="ps1")
        MM(psga[0:32, 0:N], cst["combs4"][32 * r:32 * r + N_EIGS, :],
           Utg[32 * r:32 * r + N_EIGS, g4],
           start=True, stop=True, tile_position=(32 * r, 0))
        gmx = scr.tile([32, 1], F32, tag="gmx")
        nc.vector.reduce_max(gmx[:], psga[0:32, 0:N], axis=AX.X)
        ngm = scr.tile([32, 1], F32, tag="ngm")
        nc.scalar.mul(ngm[:], gmx[:], -1.0)
        ga = scr.tile([32, N], F32, tag="ga")
        nc.scalar.activation(ga[:], psga[0:32, 0:N], AF.Exp, bias=ngm[:, 0:1])
        gs = scr.tile([32, 1], F32, tag="gs")
        nc.vector.reduce_sum(gs[:], ga[:], axis=AX.X)
        gr = scr.tile([32, 1], F32, tag="gr")
        nc.vector.reciprocal(gr[:], gs[:])
        TS(ga[:], ga[:], gr[:, 0:1], None, op0=ALU.mult)
        gaT = scr.tile([128, 2, 32], F32, tag="gaT")
        psgt = ps3.tile([128, 256], F32, tag="ps3")
        for kb in range(2):
            MM(psgt[:, 32 * kb:32 * (kb + 1)], ga[:, 128 * kb:128 * (kb + 1)],
               cst["i128"][0:32, 0:32], start=True, stop=True)
            CP(gaT[:, kb], psgt[:, 32 * kb:32 * (kb + 1)])
        psoh = ps3.tile([128, 256], F32, tag="ps3")
        for kb in range(2):
            MM(psoh[0:CH, 0:32], Vg[:, m, kb], gaT[:, kb],
               start=(kb == 0), stop=(kb == 1))
        CP(aoutT[bloc][:, 32 * h:32 * (h + 1)], psoh[0:CH, 0:32])

    for bloc in range(2):
        b = bidx[bloc]
        for ch in range(FC):
            psj = ps1.tile([128, 512], F32, tag="ps1")
            MM(psj[:, 0:N], r32(wproj[:, 128 * ch:128 * (ch + 1)]),
               r32(aoutT[bloc][:]), start=True, stop=True)
            TS(hT[:, ch, b], psj[:, 0:N], bproj[:, ch], None, op0=ALU.add)


def _mlp_block(nc, tc, cst, dram, hT, lns, lnb, b1t, b2t,
               wqrt, spool, scr, ps1, ps2, ps3, TT, TS, STT, CP, MM, r32, li):
    for pair in range(2):
        bb = [2 * pair, 2 * pair + 1]
        hnp = spool.tile([128, FC, 2, N], F32, tag="Sg")
        for bloc, b in enumerate(bb):
            pss = ps3.tile([128, 256], F32, tag="ps3")
            ps2s = ps3.tile([128, 256], F32, tag="ps3")
            for ch in range(FC):
                MM(pss[0:1, 0:128], cst["ones128"][:], hT[:, ch, b, 0:128],
                   start=(ch == 0), stop=(ch == FC - 1))
            # NOTE: sums are computed in two half-token blocks (psum free 128)
            for ch in range(FC):
                MM(ps2s[0:1, 0:128], cst["ones128"][:], hT[:, ch, b, 128:256],
                   start=(ch == 0), stop=(ch == FC - 1))
            sums = scr.tile([1, N], F32, tag="lsums")
            CP(sums[:, 0:128], pss[0:1, 0:128])
            CP(sums[:, 128:256], ps2s[0:1, 0:128])
            sq1 = scr.tile([128, N], F32, tag="lt1")
            psq1 = ps3.tile([128, 256], F32, tag="ps3")
            psq2 = ps3.tile([128, 256], F32, tag="ps3")
            for ch in range(FC):
                nc.scalar.activation(sq1[:], hT[:, ch, b], AF.Square)
                MM(psq1[0:1, 0:128], cst["ones128"][:], sq1[:, 0:128],
                   start=(ch == 0), stop=(ch == FC - 1))
                MM(psq2[0:1, 0:128], cst["ones128"][:], sq1[:, 128:256],
                   start=(ch == 0), stop=(ch == FC - 1))
            sqs = scr.tile([1, N], F32, tag="lsqs")
            CP(sqs[:, 0:128], psq1[0:1, 0:128])
            CP(sqs[:, 128:256], psq2[0:1, 0:128])
            mean = scr.tile([1, N], F32, tag="lmean")
            nc.scalar.mul(mean[:], sums[:], 1.0 / C)
            msq = scr.tile([1, N], F32, tag="lmsq")
            nc.scalar.mul(msq[:], sqs[:], 1.0 / C)
            m2 = scr.tile([1, N], F32, tag="lm2")
            nc.scalar.activation(m2[:], mean[:], AF.Square)
            var = scr.tile([1, N], F32, tag="lvar")
            TT(var[:], msq[:], m2[:], op=ALU.subtract)
            TS(var[:], var[:], 1e-5, None, op0=ALU.add)
            sdv = scr.tile([1, N], F32, tag="lsdv")
            nc.scalar.activation(sdv[:], var[:], AF.Sqrt)
            rst = scr.tile([1, N], F32, tag="lrst")
            nc.vector.reciprocal(rst[:], sdv[:])
            mrs = scr.tile([1, N], F32, tag="lmrs")
            TT(mrs[:], mean[:], rst[:], op=ALU.mult)
            psb1 = ps1.tile([128, 512], F32, tag="ps1")
            MM(psb1[:, 0:N], cst["onesrow"][:], rst[:], start=True, stop=True)
            MM(psb1[:, 256:512], cst["onesrow"][:], mrs[:], start=True, stop=True)
            rstB = scr.tile([128, N], F32, tag="lrstB")
            mrsB = scr.tile([128, N], F32, tag="lmrsB")
            CP(rstB[:], psb1[:, 0:N])
            CP(mrsB[:], psb1[:, 256:512])
            for ch in range(FC):
                t1 = scr.tile([128, N], F32, tag="lt1")
                TT(t1[:], hT[:, ch, b], rstB[:], op=ALU.mult)
                TT(t1[:], t1[:], mrsB[:], op=ALU.subtract)
                TS(hnp[:, ch, bloc], t1[:], lns[:, ch], lnb[:, ch],
                   op0=ALU.mult, op1=ALU.add)

        # MLP quarters: a1q = gelu(psum-accum) ; mlp2 accumulates into hT
        for q in range(4):
            w1q = wqrt.tile([128, FC, 768], F32, tag="wqrt")
            nc.sync.dma_start(
                w1q[:], dram["w1"][li, :, :, 768 * q:768 * (q + 1)]
                .rearrange("f p c -> p f c"))
            a1q = spool.tile([128, 6, 2, N], F32, tag="Vg")
            for jc in range(6):
                jg = 6 * q + jc
                psm = ps1.tile([128, 512], F32, tag="ps1")
                for ch in range(FC):
                    MM(psm[:],
                       r32(w1q[:, ch, 128 * jc:128 * (jc + 1)]),
                       r32(hnp[:, ch].rearrange("p a b -> p (a b)")),
                       start=(ch == 0), stop=(ch == FC - 1))
                nc.scalar.activation(
                    a1q[:, jc].rearrange("p a b -> p (a b)"), psm[:],
                    AF.Gelu, bias=b1t[:, jg])
            w2q = wqrt.tile([128, FC, 768], F32, tag="wqrt")
            nc.sync.dma_start(
                w2q[:], dram["w2"][li, 6 * q:6 * (q + 1)]
                .rearrange("j p c -> p j c"))
            for mc in range(FC):
                psm2 = ps1.tile([128, 512], F32, tag="ps1")
                for jc in range(6):
                    MM(psm2[:],
                       r32(w2q[:, jc, 128 * mc:128 * (mc + 1)]),
                       r32(a1q[:, jc].rearrange("p a b -> p (a b)")),
                       start=(jc == 0), stop=(jc == 5))
                hv = hT[:, mc, 2 * pair:2 * pair + 2, :].rearrange("p a b -> p (a b)")
                if q == 0:
                    STT(hv, psm2[:], b2t[:, mc], hv, op0=ALU.add, op1=ALU.add)
                else:
                    TT(hv, hv, psm2[:], op=ALU.add)
            # residual base already in hT (attn out); mlp adds on top


FC_A1 = 6


# ====================== host wrapper ======================

_NC_CACHE = {}


def _get_nc(n_layers=D, tap_layer=None):
    key = (n_layers, tap_layer)
    if key not in _NC_CACHE:
        _NC_CACHE[key] = build_nc(n_layers, tap_layer)
    return _NC_CACHE[key]


def kernel(**inputs):
    x = np.asarray(inputs["x"], dtype=np.float32)
    w = _prep_weights(inputs)
    xp = _prep_x(x)                      # [B, 12, 256]
    nc = _get_nc()
    in_maps = []
    for c in range(NCORES):
        im = {}
        im["xpt"] = np.ascontiguousarray(xp[BL * c:BL * (c + 1)])
        for kname, arr in w.items():
            im[kname] = arr
        for cn, arr in CONSTS.items():
            im[cn] = arr
        in_maps.append(im)
    res = run_bass_kernel_spmd(nc, in_maps, core_ids=list(range(NCORES)))
    outs = [res.results[c]["out"] for c in range(NCORES)]
    return np.concatenate(outs, axis=0).astype(np.float32)


if __name__ == "__main__":
    rng = np.random.RandomState(0)
    print("building 1-layer nc for smoke...")
    import time
    t0 = time.time()
    nc = build_nc(n_layers=1, tap_layer=0)
    print("build+compile took", time.time() - t0)



# revision 4
# speedup vs baseline: 1.4438x; 1.1813x over previous
"""Trainium2 Bass kernel for nn_CIFARViT (spectral group-attention ViT).

kernel(**inputs) takes the FULL unsharded inputs (keys as in setup_inputs),
shards the batch over 8 NeuronCores (pure data parallel, 4 images/core),
runs one Bass program per core via run_bass_kernel_spmd, and gathers the
full [32, 10] output.
"""
import sys
import os

if '/opt/trn_rl_repo' not in sys.path:
    sys.path.insert(0, '/opt/trn_rl_repo')
os.environ.setdefault("NEURON_RT_RESET_CORES", "1")

import numpy as np

import concourse.bass as bass
import concourse.mybir as mybir
from concourse import bacc
from concourse.tile import TileContext
from concourse.bass_utils import run_bass_kernel_spmd

F32 = mybir.dt.float32
F32R = mybir.dt.float32r
AF = mybir.ActivationFunctionType
ALU = mybir.AluOpType
AX = mybir.AxisListType

D = 8
C = 768
H = 8
B = 32
BL = 4
NCORES = 8
N = 256
CH = 96
N_EIGS = 5
NCLS = 10
FC = C // 128
SCALE_QK = CH ** -0.5

K = 16
ROUNDS = [5, 5, 2, 2, 2, 2, 2, 2]
if os.environ.get("KERNEL_R1"):
    ROUNDS = [int(os.environ["KERNEL_R1"])] * 8
D_CHEB = 2
A_INT, B_INT = 0.95, 1.35
C_CENT = (B_INT + A_INT) / 2.0
CHEB_S = 4.0 / (B_INT - A_INT)
ETA = 1e-2
NS_MID = (3e-3, 6)
NS_F1 = (1e-3, 8)
NS_F2 = (1e-5, 5)
DIAG_ITERS = 8
DIAG_CAP = 0.45
DIAG_DAMP = 0.4
TIE_D = 1e-5
GM = 16
NG4 = 4

_SEED = 1234


def _host_consts():
    rng = np.random.RandomState(_SEED)
    X0 = rng.randn(N, K).astype(np.float32)
    R0 = (rng.randn(N, K).astype(np.float32) / np.sqrt(N)) * ETA
    cs = {}
    cs["x0c"] = np.ascontiguousarray(X0.reshape(2, 128, K).transpose(1, 0, 2))
    cs["r0c"] = np.ascontiguousarray(R0.reshape(2, 128, K).transpose(1, 0, 2))
    cs["i128"] = np.eye(128, dtype=np.float32)
    cs["ni128"] = -np.eye(128, dtype=np.float32)
    cs["onesrow"] = np.ones((1, 128), dtype=np.float32)
    cs["ones128"] = np.ones((128, 1), dtype=np.float32)
    dm = np.zeros((2, 128, 256), dtype=np.float32)
    for r in range(2):
        for p in range(128):
            dm[r, p, 128 * r + p] = 1.0
    cs["dmask"] = np.ascontiguousarray(dm.transpose(1, 0, 2))
    cs["vmask"] = np.ascontiguousarray(1.0 - dm.transpose(1, 0, 2))
    import itertools
    combs = np.array(list(itertools.product([1.0, -1.0], repeat=N_EIGS)),
                     dtype=np.float32)
    c4 = np.zeros((128, 32), dtype=np.float32)
    for r in range(4):
        c4[32 * r:32 * r + N_EIGS, :] = combs.T
    cs["combs4"] = c4
    eye = np.zeros((128, 512), dtype=np.float32)
    blk = np.zeros((128, 512), dtype=np.float32)
    iot = np.full((128, 512), 1e9, dtype=np.float32)
    for g4 in range(4):
        for r in range(4):
            for i in range(K):
                eye[32 * r + i, 128 * g4 + 32 * r + i] = 1.0
                blk[32 * r + i, 128 * g4 + 32 * r:128 * g4 + 32 * r + K] = 1.0
                for j in range(N_EIGS):
                    iot[32 * r + i, 128 * g4 + 32 * r + j] = float(j)
    cs["sm_eye"] = eye
    cs["sm_eye15"] = 1.5 * eye
    cs["sm_blk"] = blk
    cs["sm_offblk"] = blk - eye
    cs["sm_iotasel"] = iot
    cs["sm_tie"] = (np.arange(128, dtype=np.float32) % 32 * TIE_D).reshape(128, 1)
    e32 = np.zeros((4, 128), dtype=np.float32)
    for r in range(4):
        e32[r, 32 * r:32 * (r + 1)] = 1.0
    cs["exp32"] = e32
    return cs


CONSTS = _host_consts()


def _prep_weights(inputs):
    w = {}
    qkv_w = np.asarray(inputs["qkv_w"], dtype=np.float32)
    w["wq"] = np.ascontiguousarray(qkv_w[:, :, 0:C]).reshape(D, FC, 128, C)
    w["wv"] = np.ascontiguousarray(qkv_w[:, :, 2 * C:3 * C]).reshape(D, FC, 128, C)
    w["wproj"] = np.ascontiguousarray(np.asarray(inputs["proj_w"], dtype=np.float32))
    w["bproj"] = np.asarray(inputs["proj_b"], dtype=np.float32).reshape(D, FC, 128, 1).copy()
    w["lns"] = np.asarray(inputs["g_ln_s"], dtype=np.float32).reshape(D, FC, 128, 1).copy()
    w["lnb"] = np.asarray(inputs["g_ln_b"], dtype=np.float32).reshape(D, FC, 128, 1).copy()
    w["w1"] = np.ascontiguousarray(np.asarray(inputs["mlp_w1"], dtype=np.float32)).reshape(D, FC, 128, 4 * C)
    w["b1"] = np.asarray(inputs["mlp_b1"], dtype=np.float32).reshape(D, 24, 128, 1).copy()
    w["w2"] = np.ascontiguousarray(np.asarray(inputs["mlp_w2"], dtype=np.float32)).reshape(D, 24, 128, C)
    w["b2"] = np.asarray(inputs["mlp_b2"], dtype=np.float32).reshape(D, FC, 128, 1).copy()
    w["lns2"] = np.asarray(inputs["ln_s"], dtype=np.float32).reshape(FC, 128, 1).copy()
    w["lnb2"] = np.asarray(inputs["ln_b"], dtype=np.float32).reshape(FC, 128, 1).copy()
    w["whead"] = np.asarray(inputs["head_w"], dtype=np.float32).reshape(FC, 128, NCLS).copy()
    w["bhead"] = np.asarray(inputs["head_b"], dtype=np.float32).reshape(1, NCLS).copy()
    pw = np.asarray(inputs["patch_w"], dtype=np.float32).reshape(C, 12)
    w["pwT"] = np.ascontiguousarray(pw.T)
    w["pbias"] = np.asarray(inputs["patch_b"], dtype=np.float32).reshape(FC, 128, 1).copy()
    pos = np.asarray(inputs["pos_emb"], dtype=np.float32).reshape(N, C)
    w["posT"] = np.ascontiguousarray(pos.T).reshape(FC, 128, N)
    return w


def _prep_x(x):
    Bb = x.shape[0]
    xp = np.asarray(x, dtype=np.float32).reshape(Bb, 3, 16, 2, 16, 2)
    xp = xp.transpose(0, 2, 4, 1, 3, 5).reshape(Bb, N, 12)
    return np.ascontiguousarray(np.swapaxes(xp, 1, 2))


# ====================== device program ======================

def build_nc(n_layers=D, tap_layer=None):
    nc = bacc.Bacc("TRN2", target_bir_lowering=False, debug=False)
    dram = {}

    def din(name, shape):
        dram[name] = nc.dram_tensor(name, list(shape), F32, kind="ExternalInput")

    din("xpt", (BL, 12, N))
    din("pwT", (12, C))
    din("pbias", (FC, 128, 1))
    din("posT", (FC, 128, N))
    din("wq", (D, FC, 128, C))
    din("wv", (D, FC, 128, C))
    din("wproj", (D, CH, C))
    din("bproj", (D, FC, 128, 1))
    din("lns", (D, FC, 128, 1))
    din("lnb", (D, FC, 128, 1))
    din("w1", (D, FC, 128, 4 * C))
    din("b1", (D, 24, 128, 1))
    din("w2", (D, 24, 128, C))
    din("b2", (D, FC, 128, 1))
    din("lns2", (FC, 128, 1))
    din("lnb2", (FC, 128, 1))
    din("whead", (FC, 128, NCLS))
    din("bhead", (1, NCLS))
    for cn, arr in CONSTS.items():
        din(cn, arr.shape)

    out = nc.dram_tensor("out", [BL, NCLS], F32, kind="ExternalOutput")
    tap = None
    if tap_layer is not None:
        tap = nc.dram_tensor("tap", [128, FC, BL, N], F32, kind="ExternalOutput")
    if os.environ.get("KERNEL_DEBUG_TAPS"):
        nc._dbg = {
            "d_emb": nc.dram_tensor("d_emb", [128, FC, BL, N], F32, kind="ExternalOutput"),
            "d_S": nc.dram_tensor("d_S", [128, GM, 2, N], F32, kind="ExternalOutput"),
            "d_V": nc.dram_tensor("d_V", [128, GM, 2, CH], F32, kind="ExternalOutput"),
            "d_u0": nc.dram_tensor("d_u0", [128, GM, 2, 1], F32, kind="ExternalOutput"),
            "d_X": nc.dram_tensor("d_X", [128, GM, 2, K], F32, kind="ExternalOutput"),
            "d_B": nc.dram_tensor("d_B", [128, 512], F32, kind="ExternalOutput"),
            "d_Ut": nc.dram_tensor("d_Ut", [128, NG4, N], F32, kind="ExternalOutput"),
            "d_att": nc.dram_tensor("d_att", [128, FC, BL, N], F32, kind="ExternalOutput"),
        }
    else:
        nc._dbg = {}

    with TileContext(nc) as tc:
        _emit(nc, tc, dram, out, tap, n_layers, tap_layer)
    nc.compile()
    return nc


def _emit(nc, tc, dram, out, tap, n_layers, tap_layer):
    import contextlib
    es = contextlib.ExitStack()
    with es:
        persist = es.enter_context(tc.tile_pool(name="persist", bufs=1))
        wpool = es.enter_context(tc.tile_pool(name="wpool", bufs=1))
        wstr = es.enter_context(tc.tile_pool(name="wstr", bufs=2))
        wqrt = es.enter_context(tc.tile_pool(name="wqrt", bufs=1))
        spool = es.enter_context(tc.tile_pool(name="spool", bufs=1))
        epool = es.enter_context(tc.tile_pool(name="epool", bufs=1))
        scr = es.enter_context(tc.tile_pool(name="scr", bufs=1))
        rowp = es.enter_context(tc.tile_pool(name="rowp", bufs=1))
        ps1 = es.enter_context(tc.tile_pool(name="ps1", bufs=2, space="PSUM"))
        ps2 = es.enter_context(tc.tile_pool(name="ps2", bufs=2, space="PSUM"))
        ps3 = es.enter_context(tc.tile_pool(name="ps3", bufs=2, space="PSUM"))

        cst = {}
        for cn, arr in CONSTS.items():
            t = persist.tile(list(arr.shape), F32, tag="c_" + cn)
            cst[cn] = t
            nc.sync.dma_start(t[:], dram[cn][:])

        def smc(name):
            return cst[name][:].rearrange("p (a b) -> p a b", a=4)

        hT = persist.tile([128, FC, BL, N], F32, tag="hT")

        def r32(ap):
            return ap

        TT = nc.vector.tensor_tensor
        TS = nc.vector.tensor_scalar
        STT = nc.vector.scalar_tensor_tensor
        CP = nc.vector.tensor_copy
        MM = nc.tensor.matmul

        # ============ patch embed ============
        xpt = persist.tile([12, BL, N], F32, tag="xpt")
        nc.sync.dma_start(xpt[:], dram["xpt"][:].rearrange("b k n -> k b n"))
        pwT = persist.tile([12, C], F32, tag="pwT")
        nc.sync.dma_start(pwT[:], dram["pwT"][:])
        pbias = persist.tile([128, FC, 1], F32, tag="pbias")
        nc.sync.dma_start(pbias[:], dram["pbias"][:].rearrange("f p o -> p f o"))
        posT = persist.tile([128, FC, N], F32, tag="posT")
        nc.sync.dma_start(posT[:], dram["posT"][:].rearrange("f p n -> p f n"))

        for b in range(BL):
            for ch in range(FC):
                ps = ps1.tile([128, 512], F32, tag="ps1")
                MM(ps[:, 0:N], r32(pwT[:, 128 * ch:128 * (ch + 1)]),
                   r32(xpt[:, b]), start=True, stop=True)
                tmp = scr.tile([128, N], F32, tag="ge")
                TS(tmp[:], ps[:, 0:N], pbias[:, ch], None, op0=ALU.add)
                TT(hT[:, ch, b], tmp[:], posT[:, ch], op=ALU.add)

        if nc._dbg:
            nc.sync.dma_start(nc._dbg["d_emb"][:], hT[:])
        # ===================== layers =====================
        for li in range(n_layers):
            wq = wpool.tile([128, FC, C], F32, tag="wq")
            nc.sync.dma_start(wq[:], dram["wq"][li].rearrange("f p c -> p f c"))
            wproj = wpool.tile([CH, C], F32, tag="wproj")
            nc.sync.dma_start(wproj[:], dram["wproj"][li])
            bproj = wpool.tile([128, FC, 1], F32, tag="bproj")
            nc.sync.dma_start(bproj[:], dram["bproj"][li].rearrange("f p o -> p f o"))
            lns = wpool.tile([128, FC, 1], F32, tag="lns")
            nc.sync.dma_start(lns[:], dram["lns"][li].rearrange("f p o -> p f o"))
            lnb = wpool.tile([128, FC, 1], F32, tag="lnb")
            nc.sync.dma_start(lnb[:], dram["lnb"][li].rearrange("f p o -> p f o"))
            b1t = wpool.tile([128, 24, 1], F32, tag="b1t")
            nc.sync.dma_start(b1t[:], dram["b1"][li].rearrange("j p o -> p j o"))
            b2t = wpool.tile([128, FC, 1], F32, tag="b2t")
            nc.sync.dma_start(b2t[:], dram["b2"][li].rearrange("f p o -> p f o"))

            for g in range(2):
                bidx = [2 * g, 2 * g + 1]
                Sg = spool.tile([128, GM, 2, N], F32, tag="Sg")
                Vg = spool.tile([128, GM, 2, CH], F32, tag="Vg")
                U0g = spool.tile([128, GM, 2, 1], F32, tag="U0g")
                U0r = spool.tile([128, NG4, N], F32, tag="U0r")

                # ---- v = hT^T Wv per matrix (wv streamed, SBUF-accumulate) --
                for ch in range(FC):
                    wvc = wstr.tile([128, C], F32, tag="wvc")
                    nc.sync.dma_start(wvc[:], dram["wv"][li, ch])
                    for m in range(GM):
                        b = bidx[m // H]
                        h = m % H
                        for rb in range(2):
                            psv = ps3.tile([128, 256], F32, tag="ps3")
                            MM(psv[:, 0:CH],
                               r32(hT[:, ch, b, 128 * rb:128 * (rb + 1)]),
                               r32(wvc[:, CH * h:CH * (h + 1)]),
                               start=True, stop=True)
                            if ch == 0:
                                CP(Vg[:, m, rb], psv[:, 0:CH])
                            else:
                                TT(Vg[:, m, rb], Vg[:, m, rb], psv[:, 0:CH],
                                   op=ALU.add)

                # ---- phase A (exp table): logits -> G into Sg ----
                for m in range(GM):
                    b = bidx[m // H]
                    h = m % H
                    qT = scr.tile([CH, N], F32, tag="qT")
                    psq = ps1.tile([128, 512], F32, tag="ps1")
                    for ch in range(FC):
                        MM(psq[0:CH, 0:N],
                           r32(wq[:, ch, CH * h:CH * (h + 1)]),
                           r32(hT[:, ch, b]),
                           start=(ch == 0), stop=(ch == FC - 1))
                    CP(qT[:], psq[0:CH, 0:N])
                    psl = ps2.tile([128, 512], F32, tag="ps2")
                    for rb in range(2):
                        MM(psl[:, 256 * rb:256 * (rb + 1)],
                           r32(qT[:, 128 * rb:128 * (rb + 1)]),
                           r32(qT[:]), start=True, stop=True)
                    rm = scr.tile([128, 1], F32, tag="rm")
                    nc.vector.reduce_max(rm[:], psl[:], axis=AX.X)
                    pst = ps3.tile([128, 256], F32, tag="ps3")
                    MM(pst[0:1, 0:128], rm[:], cst["i128"][:], start=True, stop=True)
                    mv = scr.tile([1, 1], F32, tag="mv")
                    nc.vector.reduce_max(mv[:], pst[0:1, 0:128], axis=AX.X)
                    pbc = ps3.tile([128, 256], F32, tag="ps3")
                    MM(pbc[:, 0:1], cst["onesrow"][:], mv[:], start=True, stop=True)
                    negm = scr.tile([128, 1], F32, tag="negm")
                    nc.scalar.mul(negm[:], pbc[:, 0:1], -SCALE_QK)
                    for rb in range(2):
                        ge = scr.tile([128, N], F32, tag="ge")
                        nc.scalar.activation(ge[:], psl[:, 256 * rb:256 * (rb + 1)],
                                             AF.Exp, bias=negm[:, 0:1],
                                             scale=SCALE_QK)
                        TT(Sg[:, m, rb], ge[:], cst["vmask"][:, rb], op=ALU.mult)

                # ---- phase B (sqrt table): G -> Ahat, u0 ----
                for m in range(GM):
                    g4, rr_ = m // 4, m % 4
                    rsum = scr.tile([128, 2], F32, tag="rsum")
                    for rb in range(2):
                        nc.vector.reduce_sum(rsum[:, rb:rb + 1], Sg[:, m, rb],
                                             axis=AX.X)
                    invr = scr.tile([128, 2], F32, tag="invr")
                    nc.vector.reciprocal(invr[:], rsum[:])
                    ptv = ps3.tile([128, 256], F32, tag="ps3")
                    for ob in range(2):
                        for kb in range(2):
                            MM(ptv[:, ob:ob + 1],
                               Sg[:, m, kb, 128 * ob:128 * (ob + 1)],
                               invr[:, kb:kb + 1],
                               start=(kb == 0), stop=(kb == 1))
                    deg = scr.tile([128, 2], F32, tag="deg")
                    TS(deg[:], ptv[:, 0:2], 0.5, 0.5, op0=ALU.mult, op1=ALU.add)
                    sd = scr.tile([128, 2], F32, tag="sd")
                    nc.scalar.activation(sd[:], deg[:], AF.Sqrt)
                    wv_ = scr.tile([128, 2], F32, tag="wv_")
                    nc.vector.reciprocal(wv_[:], sd[:])
                    alpha = scr.tile([128, 2], F32, tag="alpha")
                    TT(alpha[:], wv_[:], invr[:], op=ALU.mult)
                    nc.scalar.mul(alpha[:], alpha[:], 0.5)
                    psn = ps3.tile([128, 256], F32, tag="ps3")
                    for kb in range(2):
                        MM(psn[0:1, 0:1], deg[:, kb:kb + 1], cst["ones128"][:],
                           start=(kb == 0), stop=(kb == 1))
                    nrm = scr.tile([1, 1], F32, tag="nrm")
                    nc.scalar.activation(nrm[:], psn[0:1, 0:1], AF.Sqrt)
                    rn = scr.tile([1, 1], F32, tag="rn")
                    nc.vector.reciprocal(rn[:], nrm[:])
                    pbc = ps3.tile([128, 256], F32, tag="ps3")
                    MM(pbc[:, 0:1], cst["onesrow"][:], rn[:], start=True, stop=True)
                    rnb = scr.tile([128, 1], F32, tag="rnb")
                    CP(rnb[:], pbc[:, 0:1])
                    for kb in range(2):
                        TS(U0g[:, m, kb], sd[:, kb:kb + 1], rnb[:, 0:1], None,
                           op0=ALU.mult)
                    arow = rowp.tile([1, N], F32, tag="arow")
                    brow = rowp.tile([1, N], F32, tag="brow")
                    for (row_t, col_t) in ((arow, alpha), (brow, wv_)):
                        ptr2 = ps3.tile([128, 256], F32, tag="ps3")
                        for kb in range(2):
                            MM(ptr2[0:1, 128 * kb:128 * (kb + 1)],
                               col_t[:, kb:kb + 1], cst["i128"][:],
                               start=True, stop=True)
                        CP(row_t[:], ptr2[0:1, :])
                    ptr3 = ps3.tile([128, 256], F32, tag="ps3")
                    for kb in range(2):
                        MM(ptr3[32 * rr_:32 * rr_ + 1, 128 * kb:128 * (kb + 1)],
                           U0g[:, m, kb], cst["i128"][:],
                           start=True, stop=True, tile_position=(0, 32 * rr_))
                    CP(U0r[32 * rr_:32 * rr_ + 1, g4], ptr3[32 * rr_:32 * rr_ + 1, :])
                    for rb in range(2):
                        pso = ps1.tile([128, 512], F32, tag="ps1")
                        MM(pso[:, 0:N], arow[:, 128 * rb:128 * (rb + 1)],
                           brow[:], start=True, stop=False)
                        MM(pso[:, 0:N], brow[:, 128 * rb:128 * (rb + 1)],
                           arow[:], start=False, stop=True)
                        go = scr.tile([128, N], F32, tag="ge")
                        TT(go[:], Sg[:, m, rb], pso[:, 0:N], op=ALU.mult)
                        STT(Sg[:, m, rb], go[:], -CHEB_S, cst["dmask"][:, rb],
                            op0=ALU.mult, op1=ALU.bypass)
                        STT(Sg[:, m, rb], cst["dmask"][:, rb],
                            CHEB_S * (1.0 - C_CENT), Sg[:, m, rb],
                            op0=ALU.mult, op1=ALU.add)

                if nc._dbg and li == 0 and g == 0:
                    nc.sync.dma_start(nc._dbg["d_S"][:], Sg[:])
                    nc.sync.dma_start(nc._dbg["d_V"][:], Vg[:])
                    nc.sync.dma_start(nc._dbg["d_u0"][:], U0g[:])
                _eigensolve_and_attend(
                    nc, tc, cst, smc, dram, hT, Sg, Vg, U0g, U0r,
                    wproj, bproj, epool, spool, scr, ps1, ps2, ps3,
                    TT, TS, STT, CP, MM, r32, li, g, bidx)

            if nc._dbg and li == 0:
                nc.sync.dma_start(nc._dbg["d_att"][:], hT[:])
            _mlp_block(nc, tc, cst, dram, hT, lns, lnb, b1t, b2t,
                       wqrt, spool, scr, ps1, ps2, ps3,
                       TT, TS, STT, CP, MM, r32, li)

            if tap_layer is not None and li == tap_layer:
                nc.sync.dma_start(tap[:], hT[:])

        # ============ pool + final LN + head ============
        lns2 = wpool.tile([128, FC, 1], F32, tag="lns")
        nc.sync.dma_start(lns2[:], dram["lns2"][:].rearrange("f p o -> p f o"))
        lnb2 = wpool.tile([128, FC, 1], F32, tag="lnb")
        nc.sync.dma_start(lnb2[:], dram["lnb2"][:].rearrange("f p o -> p f o"))
        whead = wpool.tile([128, FC, NCLS], F32, tag="whead")
        nc.sync.dma_start(whead[:], dram["whead"][:].rearrange("f p c -> p f c"))
        bhead = wpool.tile([1, NCLS], F32, tag="bhead")
        nc.sync.dma_start(bhead[:], dram["bhead"][:])

        for b in range(BL):
            pooled = scr.tile([128, FC], F32, tag="pooled")
            for ch in range(FC):
                nc.vector.reduce_sum(pooled[:, ch:ch + 1], hT[:, ch, b],
                                     axis=AX.X)
            nc.scalar.mul(pooled[:], pooled[:], 1.0 / N)
            psa = ps3.tile([128, 256], F32, tag="ps3")
            for ch in range(FC):
                MM(psa[0:1, 0:1], pooled[:, ch:ch + 1], cst["ones128"][:],
                   start=(ch == 0), stop=(ch == FC - 1))
            sq = scr.tile([128, FC], F32, tag="poolsq")
            nc.scalar.activation(sq[:], pooled[:], AF.Square)
            psb = ps3.tile([128, 256], F32, tag="ps3")
            for ch in range(FC):
                MM(psb[0:1, 0:1], sq[:, ch:ch + 1], cst["ones128"][:],
                   start=(ch == 0), stop=(ch == FC - 1))
            mean = scr.tile([1, 1], F32, tag="fmean")
            nc.scalar.mul(mean[:], psa[0:1, 0:1], 1.0 / C)
            msq = scr.tile([1, 1], F32, tag="fmsq")
            nc.scalar.mul(msq[:], psb[0:1, 0:1], 1.0 / C)
            m2 = scr.tile([1, 1], F32, tag="fm2")
            nc.scalar.activation(m2[:], mean[:], AF.Square)
            var = scr.tile([1, 1], F32, tag="fvar")
            TT(var[:], msq[:], m2[:], op=ALU.subtract)
            TS(var[:], var[:], 1e-5, None, op0=ALU.add)
            sdv = scr.tile([1, 1], F32, tag="fsdv")
            nc.scalar.activation(sdv[:], var[:], AF.Sqrt)
            rstd = scr.tile([1, 1], F32, tag="frstd")
            nc.vector.reciprocal(rstd[:], sdv[:])
            two = scr.tile([1, 2], F32, tag="ftwo")
            CP(two[:, 0:1], mean[:])
            CP(two[:, 1:2], rstd[:])
            psc = ps3.tile([128, 256], F32, tag="ps3")
            MM(psc[:, 0:2], cst["onesrow"][:], two[:], start=True, stop=True)
            meanb = scr.tile([128, 1], F32, tag="fmeanb")
            rstdb = scr.tile([128, 1], F32, tag="frstdb")
            CP(meanb[:], psc[:, 0:1])
            CP(rstdb[:], psc[:, 1:2])
            pnorm = scr.tile([128, FC], F32, tag="pnorm")
            TS(pnorm[:], pooled[:], meanb[:, 0:1], None, op0=ALU.subtract)
            TS(pnorm[:], pnorm[:], rstdb[:, 0:1], None, op0=ALU.mult)
            for ch in range(FC):
                TS(pnorm[:, ch:ch + 1], pnorm[:, ch:ch + 1],
                   lns2[:, ch], lnb2[:, ch], op0=ALU.mult, op1=ALU.add)
            psh = ps3.tile([128, 256], F32, tag="ps3")
            for ch in range(FC):
                MM(psh[0:1, 0:NCLS], pnorm[:, ch:ch + 1], whead[:, ch],
                   start=(ch == 0), stop=(ch == FC - 1))
            ologit = scr.tile([1, NCLS], F32, tag="ologit")
            TT(ologit[:], psh[0:1, 0:NCLS], bhead[:], op=ALU.add)
            nc.sync.dma_start(out[b:b + 1, :], ologit[:])


def _eigensolve_and_attend(nc, tc, cst, smc, dram, hT, Sg, Vg, U0g, U0r,
                           wproj, bproj, epool, spool, scr, ps1, ps2, ps3,
                           TT, TS, STT, CP, MM, r32, li, g, bidx):
    Xg = epool.tile([128, GM, 2, K], F32, tag="Xg")
    for m in range(GM):
        for kb in range(2):
            CP(Xg[:, m, kb], cst["x0c"][:, kb])
    Xt = epool.tile([128, NG4, N], F32, tag="Xt")

    def cheb_round():
        for m in range(GM):
            Tp = epool.tile([128, 2, K], F32, tag="Tp")
            Tc = epool.tile([128, 2, K], F32, tag="Tc")
            for kb in range(2):
                CP(Tp[:, kb], Xg[:, m, kb])
            psx = ps2.tile([128, 512], F32, tag="ps2")
            for ob in range(2):
                for kb in range(2):
                    MM(psx[:, K * ob:K * (ob + 1)],
                       Sg[:, m, kb, 128 * ob:128 * (ob + 1)],
                       Tp[:, kb], start=(kb == 0), stop=(kb == 1))
            for kb in range(2):
                nc.scalar.mul(Tc[:, kb], psx[:, K * kb:K * (kb + 1)], 0.5)
            for _ in range(D_CHEB - 1):
                psy = ps2.tile([128, 512], F32, tag="ps2")
                for ob in range(2):
                    for kb in range(2):
                        MM(psy[:, K * ob:K * (ob + 1)],
                           Sg[:, m, kb, 128 * ob:128 * (ob + 1)],
                           Tc[:, kb], start=(kb == 0), stop=False)
                    MM(psy[:, K * ob:K * (ob + 1)], cst["ni128"][:], Tp[:, ob],
                       start=False, stop=True)
                for kb in range(2):
                    CP(Tp[:, kb], Tc[:, kb])
                    CP(Tc[:, kb], psy[:, K * kb:K * (kb + 1)])
            for kb in range(2):
                CP(Xg[:, m, kb], Tc[:, kb])

    def deflate():
        for m in range(GM):
            g4, r = m // 4, m % 4
            psp = ps3.tile([128, 256], F32, tag="ps3")
            for kb in range(2):
                MM(psp[32 * r:32 * r + 1, 0:K], U0g[:, m, kb], Xg[:, m, kb],
                   start=(kb == 0), stop=(kb == 1), tile_position=(0, 32 * r))
            pr = scr.tile([128, K], F32, tag="pr")
            nc.scalar.mul(pr[32 * r:32 * r + 1, :], psp[32 * r:32 * r + 1, 0:K], -1.0)
            psd = ps2.tile([128, 512], F32, tag="ps2")
            for kb in range(2):
                MM(psd[:, K * kb:K * (kb + 1)], cst["i128"][:], Xg[:, m, kb],
                   start=True, stop=False)
                MM(psd[:, K * kb:K * (kb + 1)],
                   U0r[32 * r:32 * r + 1, g4, 128 * kb:128 * (kb + 1)],
                   pr[32 * r:32 * r + 1, :],
                   start=False, stop=True, tile_position=(32 * r, 0))
            for kb in range(2):
                CP(Xg[:, m, kb], psd[:, K * kb:K * (kb + 1)])

    def small_mm(dst, lhsT, rhs):
        psb = ps2.tile([128, 512], F32, tag="ps2")
        pv = psb[:].rearrange("p (a b) -> p a b", a=4)
        for g4 in range(NG4):
            MM(pv[:, g4], lhsT[:, g4], rhs[:, g4], start=True, stop=True)
        CP(dst[:], pv[:])

    def build_xt():
        for m in range(GM):
            g4, r = m // 4, m % 4
            ptx = ps2.tile([128, 512], F32, tag="ps2")
            for kb in range(2):
                MM(ptx[32 * r:32 * r + K, 128 * kb:128 * (kb + 1)],
                   Xg[:, m, kb], cst["i128"][:],
                   start=True, stop=True, tile_position=(0, 32 * r))
            CP(Xt[32 * r:32 * r + K, g4], ptx[32 * r:32 * r + K, 0:256])

    def bcast_permat(v44):
        ptq = ps3.tile([128, 256], F32, tag="ps3")
        MM(ptq[0:4, 0:4], v44[:], cst["i128"][0:4, 0:4], start=True, stop=True)
        v44t = scr.tile([4, 4], F32, tag="v44t")
        CP(v44t[:], ptq[0:4, 0:4])
        ptw = ps3.tile([128, 256], F32, tag="ps3")
        MM(ptw[:, 0:4], cst["exp32"][:], v44t[:], start=True, stop=True)
        ob = scr.tile([128, 4], F32, tag="permat")
        CP(ob[:], ptw[:, 0:4])
        return ob

    def permat_max(src):
        ptm = ps3.tile([128, 256], F32, tag="ps3")
        MM(ptm[0:4, 0:128], src[:], cst["i128"][:], start=True, stop=True)
        tr = scr.tile([4, 128], F32, tag="tr44")
        CP(tr[:], ptm[0:4, 0:128])
        mx = scr.tile([4, 4], F32, tag="mx44")
        nc.vector.reduce_max(mx[:], tr[:].rearrange("p (a b) -> p a b", a=4),
                             axis=AX.X)
        return mx

    def whiten_pass(eps, steps):
        Gt = epool.tile([128, 4, 128], F32, tag="Gt")
        psg = ps2.tile([128, 512], F32, tag="ps2")
        pvg = psg[:].rearrange("p (a b) -> p a b", a=4)
        for m in range(GM):
            g4, r = m // 4, m % 4
            for kb in range(2):
                MM(psg[32 * r:32 * r + K,
                       128 * g4 + 32 * r:128 * g4 + 32 * r + K],
                   Xg[:, m, kb], Xg[:, m, kb],
                   start=(kb == 0), stop=(kb == 1), tile_position=(0, 32 * r))
        TT(Gt[:], pvg[:], smc("sm_blk"), op=ALU.mult)
        dg = scr.tile([128, 4, 128], F32, tag="smA")
        TT(dg[:], Gt[:], smc("sm_eye"), op=ALU.mult)
        dgt = scr.tile([128, 4], F32, tag="dgt")
        nc.vector.reduce_sum(dgt[:], dg[:], axis=AX.X)
        TS(dgt[:], dgt[:], 1e-12, None, op0=ALU.add)
        sq = scr.tile([128, 4], F32, tag="sq")
        nc.scalar.activation(sq[:], dgt[:], AF.Sqrt)
        srec = scr.tile([128, 4], F32, tag="srec")
        nc.vector.reciprocal(srec[:], sq[:])
        for g4 in range(NG4):
            TS(Gt[:, g4], Gt[:, g4], srec[:, g4:g4 + 1], None, op0=ALU.mult)
        GtT = epool.tile([128, 4, 128], F32, tag="GtT")
        small_mm(GtT, Gt, smc("sm_eye"))
        for g4 in range(NG4):
            TS(GtT[:, g4], GtT[:, g4], srec[:, g4:g4 + 1], None, op0=ALU.mult)
        STT(Gt[:], smc("sm_eye"), eps, GtT[:], op0=ALU.mult, op1=ALU.add)
        brs = scr.tile([128, 4], F32, tag="brs4")
        nc.vector.tensor_reduce(brs[:], Gt[:], axis=AX.X, op=ALU.add,
                                apply_absolute_value=True)
        mx = permat_max(brs)
        rec = scr.tile([4, 4], F32, tag="rec44")
        nc.vector.reciprocal(rec[:], mx[:])
        bre = bcast_permat(rec)
        for g4 in range(NG4):
            TS(Gt[:, g4], Gt[:, g4], bre[:, g4:g4 + 1], None, op0=ALU.mult)
        Yt = epool.tile([128, 4, 128], F32, tag="Yt")
        Zt = epool.tile([128, 4, 128], F32, tag="Zt")
        Tt = epool.tile([128, 4, 128], F32, tag="Tt")
        CP(Yt[:], Gt[:])
        CP(Zt[:], smc("sm_eye"))
        for _ in range(steps):
            psb = ps2.tile([128, 512], F32, tag="ps2")
            pv2 = psb[:].rearrange("p (a b) -> p a b", a=4)
            for g4 in range(NG4):
                MM(pv2[:, g4], Zt[:, g4], Yt[:, g4], start=True, stop=True)
            STT(Tt[:], pv2[:], -0.5, smc("sm_eye15"), op0=ALU.mult, op1=ALU.add)
            small_mm(Yt, Yt, Tt)
            small_mm(Zt, Tt, Zt)
        sqb = scr.tile([4, 4], F32, tag="sqb44")
        nc.scalar.activation(sqb[:], mx[:], AF.Sqrt)
        nc.vector.reciprocal(sqb[:], sqb[:])
        sbe = bcast_permat(sqb)
        for g4 in range(NG4):
            TS(Zt[:, g4], Zt[:, g4], srec[:, g4:g4 + 1], None, op0=ALU.mult)
            TS(Zt[:, g4], Zt[:, g4], sbe[:, g4:g4 + 1], None, op0=ALU.mult)
        build_xt()
        for m in range(GM):
            g4, r = m // 4, m % 4
            psx = ps2.tile([128, 512], F32, tag="ps2")
            for kb in range(2):
                MM(psx[:, K * kb:K * (kb + 1)],
                   Xt[32 * r:32 * r + K, g4, 128 * kb:128 * (kb + 1)],
                   Zt[32 * r:32 * r + K, g4, 32 * r:32 * r + K],
                   start=True, stop=True, tile_position=(32 * r, 0))
            for kb in range(2):
                CP(Xg[:, m, kb], psx[:, K * kb:K * (kb + 1)])

    # ---------------- rounds ----------------
    for rr in range(ROUNDS[li]):
        cheb_round()
        deflate()
        pass  # barrier removed
        whiten_pass(*NS_MID)
        pass  # barrier removed
        for m in range(GM):
            for kb in range(2):
                TT(Xg[:, m, kb], Xg[:, m, kb], cst["r0c"][:, kb], op=ALU.add)
    deflate()
    pass  # barrier removed
    whiten_pass(*NS_F1)
    pass  # barrier removed
    whiten_pass(*NS_F2)
    pass  # barrier removed
    if nc._dbg and li == 0 and g == 0:
        nc.sync.dma_start(nc._dbg["d_X"][:], Xg[:])

    # ---------------- B build ----------------
    Bt = epool.tile([128, 4, 128], F32, tag="Bt")
    Wt = epool.tile([128, 4, 128], F32, tag="Wt")
    psgB = ps2.tile([128, 512], F32, tag="ps2")
    pvB = psgB[:].rearrange("p (a b) -> p a b", a=4)
    for m in range(GM):
        g4, r = m // 4, m % 4
        psz = ps1.tile([128, 512], F32, tag="ps1")
        for ob in range(2):
            for kb in range(2):
                MM(psz[:, K * ob:K * (ob + 1)],
                   Sg[:, m, kb, 128 * ob:128 * (ob + 1)],
                   Xg[:, m, kb], start=(kb == 0), stop=(kb == 1))
        Zc = epool.tile([128, 2, K], F32, tag="Zc")
        for kb in range(2):
            CP(Zc[:, kb], psz[:, K * kb:K * (kb + 1)])
        for kb in range(2):
            MM(psgB[32 * r:32 * r + K,
                    128 * g4 + 32 * r:128 * g4 + 32 * r + K],
               Xg[:, m, kb], Zc[:, kb],
               start=(kb == 0), stop=(kb == 1), tile_position=(0, 32 * r))
    TT(Bt[:], pvB[:], smc("sm_blk"), op=ALU.mult)
    BtT = epool.tile([128, 4, 128], F32, tag="GtT")
    small_mm(BtT, Bt, smc("sm_eye"))
    TT(Bt[:], Bt[:], BtT[:], op=ALU.add)
    nc.scalar.mul(Bt[:], Bt[:], 0.5)
    CP(Wt[:], smc("sm_eye"))
    if nc._dbg and li == 0 and g == 0:
        nc.sync.dma_start(nc._dbg["d_B"][:], Bt[:].rearrange("p a b -> p (a b)"))
    build_xt()

    # ---------------- all-pairs diagonalizer ----------------
    Et = epool.tile([128, 4, 128], F32, tag="Et")
    Qt = epool.tile([128, 4, 128], F32, tag="Qt")
    QtT = epool.tile([128, 4, 128], F32, tag="QtT")
    M1 = epool.tile([128, 4, 128], F32, tag="M1")
    for it in range(DIAG_ITERS):
        dmat = scr.tile([128, 4, 128], F32, tag="smA")
        TT(dmat[:], Bt[:], smc("sm_eye"), op=ALU.mult)
        dcol = scr.tile([128, 4], F32, tag="dcol")
        nc.vector.reduce_sum(dcol[:], dmat[:], axis=AX.X)
        Drow = epool.tile([128, 4, 128], F32, tag="GtT")
        small_mm(Drow, smc("sm_blk"), dmat)
        dd = scr.tile([128, 4, 128], F32, tag="smB")
        for g4 in range(NG4):
            STT(dd[:, g4], Drow[:, g4], dcol[:, g4:g4 + 1],
                smc("sm_blk")[:, g4], op0=ALU.subtract, op1=ALU.mult)
        sgn = scr.tile([128, 4, 128], F32, tag="smC")
        TS(sgn[:], dd[:], 0.0, None, op0=ALU.is_ge)
        STT(dd[:], sgn[:], 2e-9, dd[:], op0=ALU.mult, op1=ALU.add)
        TS(dd[:], dd[:], -1e-9, None, op0=ALU.add)
        nc.vector.reciprocal(dd[:], dd[:])
        TT(Et[:], Bt[:], dd[:], op=ALU.mult)
        TT(Et[:], Et[:], smc("sm_offblk"), op=ALU.mult)
        TS(Et[:], Et[:], DIAG_CAP, None, op0=ALU.min)
        TS(Et[:], Et[:], -DIAG_CAP, None, op0=ALU.max)
        EtT = scr.tile([128, 4, 128], F32, tag="smA")
        small_mm(EtT, Et, smc("sm_eye"))
        TT(Et[:], Et[:], EtT[:], op=ALU.subtract)
        nc.scalar.mul(Et[:], Et[:], 0.5)
        ern = scr.tile([128, 4], F32, tag="ern4")
        nc.vector.tensor_reduce(ern[:], Et[:], axis=AX.X, op=ALU.add,
                                apply_absolute_value=True)
        emx = permat_max(ern)
        TS(emx[:], emx[:], 1e-9, None, op0=ALU.add)
        esc = scr.tile([4, 4], F32, tag="esc44")
        nc.vector.reciprocal(esc[:], emx[:])
        TS(esc[:], esc[:], DIAG_DAMP, 1.0, op0=ALU.mult, op1=ALU.min)
        ebe = bcast_permat(esc)
        for g4 in range(NG4):
            TS(Et[:, g4], Et[:, g4], ebe[:, g4:g4 + 1], None, op0=ALU.mult)
        TT(Qt[:], smc("sm_eye"), Et[:], op=ALU.add)
        TT(QtT[:], smc("sm_eye"), Et[:], op=ALU.subtract)
        for _ in range(2):
            P_ = scr.tile([128, 4, 128], F32, tag="smA")
            small_mm(P_, Qt, Qt)
            STT(M1[:], P_[:], -0.5, smc("sm_eye15"), op0=ALU.mult, op1=ALU.add)
            Qn = scr.tile([128, 4, 128], F32, tag="smB")
            small_mm(Qn, QtT, M1)
            small_mm(QtT, M1, QtT)
            CP(Qt[:], Qn[:])
        BQ = scr.tile([128, 4, 128], F32, tag="smA")
        small_mm(BQ, Bt, Qt)
        small_mm(Bt, Qt, BQ)
        small_mm(Wt, Qt, Wt)

    # ---------------- rank selection -> Ut ----------------
    dmat = scr.tile([128, 4, 128], F32, tag="smA")
    TT(dmat[:], Bt[:], smc("sm_eye"), op=ALU.mult)
    dcol = scr.tile([128, 4], F32, tag="dcol")
    nc.vector.reduce_sum(dcol[:], dmat[:], axis=AX.X)
    TS(dcol[:], dcol[:], cst["sm_tie"][:, 0:1], None, op0=ALU.add)
    dmat2 = scr.tile([128, 4, 128], F32, tag="smB")
    for g4 in range(NG4):
        TS(dmat2[:, g4], smc("sm_eye")[:, g4], dcol[:, g4:g4 + 1], None,
           op0=ALU.mult)
    Drow = epool.tile([128, 4, 128], F32, tag="GtT")
    small_mm(Drow, smc("sm_blk"), dmat2)
    Cc = scr.tile([128, 4, 128], F32, tag="smC")
    for g4 in range(NG4):
        STT(Cc[:, g4], Drow[:, g4], dcol[:, g4:g4 + 1],
            smc("sm_blk")[:, g4], op0=ALU.is_lt, op1=ALU.mult)
    rnk = scr.tile([128, 4], F32, tag="rnk")
    nc.vector.reduce_sum(rnk[:], Cc[:], axis=AX.X)
    Sel = scr.tile([128, 4, 128], F32, tag="smC")
    for g4 in range(NG4):
        TS(Sel[:, g4], smc("sm_iotasel")[:, g4], rnk[:, g4:g4 + 1], None,
           op0=ALU.is_equal)
    Pt = epool.tile([128, 4, 128], F32, tag="M1")
    small_mm(Pt, Wt, Sel)

    Utg = epool.tile([128, NG4, N], F32, tag="Utg")
    for g4 in range(NG4):
        psu = ps1.tile([128, 512], F32, tag="ps1")
        for r in range(4):
            MM(psu[32 * r:32 * r + N_EIGS, 0:N],
               Pt[32 * r:32 * r + K, g4, 32 * r:32 * r + N_EIGS],
               Xt[32 * r:32 * r + K, g4],
               start=True, stop=True, tile_position=(32 * r, 32 * r))
        CP(Utg[:, g4], psu[:, 0:N])

    un = scr.tile([128, NG4], F32, tag="un")
    for g4 in range(NG4):
        usq = scr.tile([128, N], F32, tag="ge")
        nc.scalar.activation(usq[:], Utg[:, g4], AF.Square)
        nc.vector.reduce_sum(un[:, g4:g4 + 1], usq[:], axis=AX.X)
    TS(un[:], un[:], 1e-30, None, op0=ALU.add)
    uns = scr.tile([128, NG4], F32, tag="uns")
    nc.scalar.activation(uns[:], un[:], AF.Sqrt)
    nc.vector.reciprocal(uns[:], uns[:])
    for g4 in range(NG4):
        TS(Utg[:, g4], Utg[:, g4], uns[:, g4:g4 + 1], None, op0=ALU.mult)

    if nc._dbg and li == 0 and g == 0:
        nc.sync.dma_start(nc._dbg["d_Ut"][:], Utg[:])
    # ---------------- group attention + out + proj ----------------
    aoutT = [spool.tile([CH, N], F32, tag="aoutT" + str(i), name="aoutT" + str(i)) for i in range(2)]
    for m in range(GM):
        g4, r = m // 4, m % 4
        bloc = m // H
        h = m % H
        psga = ps1.tile([128, 512], F32, tag="ps1")
        MM(psga[0:32, 0:N], cst["combs4"][32 * r:32 * r + N_EIGS, :],
           Utg[32 * r:32 * r + N_EIGS, g4],
           start=True, stop=True, tile_position=(32 * r, 0))
        gmx = scr.tile([32, 1], F32, tag="gmx")
        nc.vector.reduce_max(gmx[:], psga[0:32, 0:N], axis=AX.X)
        ngm = scr.tile([32, 1], F32, tag="ngm")
        nc.scalar.mul(ngm[:], gmx[:], -1.0)
        ga = scr.tile([32, N], F32, tag="ga")
        nc.scalar.activation(ga[:], psga[0:32, 0:N], AF.Exp, bias=ngm[:, 0:1])
        gs = scr.tile([32, 1], F32, tag="gs")
        nc.vector.reduce_sum(gs[:], ga[:], axis=AX.X)
        gr = scr.tile([32, 1], F32, tag="gr")
        nc.vector.reciprocal(gr[:], gs[:])
        TS(ga[:], ga[:], gr[:, 0:1], None, op0=ALU.mult)
        gaT = scr.tile([128, 2, 32], F32, tag="gaT")
        psgt = ps3.tile([128, 256], F32, tag="ps3")
        for kb in range(2):
            MM(psgt[:, 32 * kb:32 * (kb + 1)], ga[:, 128 * kb:128 * (kb + 1)],
               cst["i128"][0:32, 0:32], start=True, stop=True)
            CP(gaT[:, kb], psgt[:, 32 * kb:32 * (kb + 1)])
        psoh = ps3.tile([128, 256], F32, tag="ps3")
        for kb in range(2):
            MM(psoh[0:CH, 0:32], Vg[:, m, kb], gaT[:, kb],
               start=(kb == 0), stop=(kb == 1))
        CP(aoutT[bloc][:, 32 * h:32 * (h + 1)], psoh[0:CH, 0:32])

    for bloc in range(2):
        b = bidx[bloc]
        for ch in range(FC):
            psj = ps1.tile([128, 512], F32, tag="ps1")
            MM(psj[:, 0:N], r32(wproj[:, 128 * ch:128 * (ch + 1)]),
               r32(aoutT[bloc][:]), start=True, stop=True)
            TS(hT[:, ch, b], psj[:, 0:N], bproj[:, ch], None, op0=ALU.add)


def _mlp_block(nc, tc, cst, dram, hT, lns, lnb, b1t, b2t,
               wqrt, spool, scr, ps1, ps2, ps3, TT, TS, STT, CP, MM, r32, li):
    for pair in range(2):
        bb = [2 * pair, 2 * pair + 1]
        hnp = spool.tile([128, FC, 2, N], F32, tag="Sg")
        for bloc, b in enumerate(bb):
            pss = ps3.tile([128, 256], F32, tag="ps3")
            ps2s = ps3.tile([128, 256], F32, tag="ps3")
            for ch in range(FC):
                MM(pss[0:1, 0:128], cst["ones128"][:], hT[:, ch, b, 0:128],
                   start=(ch == 0), stop=(ch == FC - 1))
            # NOTE: sums are computed in two half-token blocks (psum free 128)
            for ch in range(FC):
                MM(ps2s[0:1, 0:128], cst["ones128"][:], hT[:, ch, b, 128:256],
                   start=(ch == 0), stop=(ch == FC - 1))
            sums = scr.tile([1, N], F32, tag="lsums")
            CP(sums[:, 0:128], pss[0:1, 0:128])
            CP(sums[:, 128:256], ps2s[0:1, 0:128])
            sq1 = scr.tile([128, N], F32, tag="lt1")
            psq1 = ps3.tile([128, 256], F32, tag="ps3")
            psq2 = ps3.tile([128, 256], F32, tag="ps3")
            for ch in range(FC):
                nc.scalar.activation(sq1[:], hT[:, ch, b], AF.Square)
                MM(psq1[0:1, 0:128], cst["ones128"][:], sq1[:, 0:128],
                   start=(ch == 0), stop=(ch == FC - 1))
                MM(psq2[0:1, 0:128], cst["ones128"][:], sq1[:, 128:256],
                   start=(ch == 0), stop=(ch == FC - 1))
            sqs = scr.tile([1, N], F32, tag="lsqs")
            CP(sqs[:, 0:128], psq1[0:1, 0:128])
            CP(sqs[:, 128:256], psq2[0:1, 0:128])
            mean = scr.tile([1, N], F32, tag="lmean")
            nc.scalar.mul(mean[:], sums[:], 1.0 / C)
            msq = scr.tile([1, N], F32, tag="lmsq")
            nc.scalar.mul(msq[:], sqs[:], 1.0 / C)
            m2 = scr.tile([1, N], F32, tag="lm2")
            nc.scalar.activation(m2[:], mean[:], AF.Square)
            var = scr.tile([1, N], F32, tag="lvar")
            TT(var[:], msq[:], m2[:], op=ALU.subtract)
            TS(var[:], var[:], 1e-5, None, op0=ALU.add)
            sdv = scr.tile([1, N], F32, tag="lsdv")
            nc.scalar.activation(sdv[:], var[:], AF.Sqrt)
            rst = scr.tile([1, N], F32, tag="lrst")
            nc.vector.reciprocal(rst[:], sdv[:])
            mrs = scr.tile([1, N], F32, tag="lmrs")
            TT(mrs[:], mean[:], rst[:], op=ALU.mult)
            psb1 = ps1.tile([128, 512], F32, tag="ps1")
            MM(psb1[:, 0:N], cst["onesrow"][:], rst[:], start=True, stop=True)
            MM(psb1[:, 256:512], cst["onesrow"][:], mrs[:], start=True, stop=True)
            rstB = scr.tile([128, N], F32, tag="lrstB")
            mrsB = scr.tile([128, N], F32, tag="lmrsB")
            CP(rstB[:], psb1[:, 0:N])
            CP(mrsB[:], psb1[:, 256:512])
            for ch in range(FC):
                t1 = scr.tile([128, N], F32, tag="lt1")
                TT(t1[:], hT[:, ch, b], rstB[:], op=ALU.mult)
                TT(t1[:], t1[:], mrsB[:], op=ALU.subtract)
                TS(hnp[:, ch, bloc], t1[:], lns[:, ch], lnb[:, ch],
                   op0=ALU.mult, op1=ALU.add)

        # MLP quarters: a1q = gelu(psum-accum) ; mlp2 accumulates into hT
        for q in range(4):
            w1q = wqrt.tile([128, FC, 768], F32, tag="wqrt")
            nc.sync.dma_start(
                w1q[:], dram["w1"][li, :, :, 768 * q:768 * (q + 1)]
                .rearrange("f p c -> p f c"))
            a1q = spool.tile([128, 6, 2, N], F32, tag="Vg")
            for jc in range(6):
                jg = 6 * q + jc
                psm = ps1.tile([128, 512], F32, tag="ps1")
                for ch in range(FC):
                    MM(psm[:],
                       r32(w1q[:, ch, 128 * jc:128 * (jc + 1)]),
                       r32(hnp[:, ch].rearrange("p a b -> p (a b)")),
                       start=(ch == 0), stop=(ch == FC - 1))
                nc.scalar.activation(
                    a1q[:, jc].rearrange("p a b -> p (a b)"), psm[:],
                    AF.Gelu, bias=b1t[:, jg])
            w2q = wqrt.tile([128, FC, 768], F32, tag="wqrt")
            nc.sync.dma_start(
                w2q[:], dram["w2"][li, 6 * q:6 * (q + 1)]
                .rearrange("j p c -> p j c"))
            for mc in range(FC):
                psm2 = ps1.tile([128, 512], F32, tag="ps1")
                for jc in range(6):
                    MM(psm2[:],
                       r32(w2q[:, jc, 128 * mc:128 * (mc + 1)]),
                       r32(a1q[:, jc].rearrange("p a b -> p (a b)")),
                       start=(jc == 0), stop=(jc == 5))
                hv = hT[:, mc, 2 * pair:2 * pair + 2, :].rearrange("p a b -> p (a b)")
                if q == 0:
                    STT(hv, psm2[:], b2t[:, mc], hv, op0=ALU.add, op1=ALU.add)
                else:
                    TT(hv, hv, psm2[:], op=ALU.add)
            # residual base already in hT (attn out); mlp adds on top


FC_A1 = 6


# ====================== host wrapper ======================

_NC_CACHE = {}


def _get_nc(n_layers=D, tap_layer=None):
    key = (n_layers, tap_layer)
    if key not in _NC_CACHE:
        _NC_CACHE[key] = build_nc(n_layers, tap_layer)
    return _NC_CACHE[key]


def kernel(**inputs):
    x = np.asarray(inputs["x"], dtype=np.float32)
    w = _prep_weights(inputs)
    xp = _prep_x(x)                      # [B, 12, 256]
    nc = _get_nc()
    in_maps = []
    for c in range(NCORES):
        im = {}
        im["xpt"] = np.ascontiguousarray(xp[BL * c:BL * (c + 1)])
        for kname, arr in w.items():
            im[kname] = arr
        for cn, arr in CONSTS.items():
            im[cn] = arr
        in_maps.append(im)
    res = run_bass_kernel_spmd(nc, in_maps, core_ids=list(range(NCORES)))
    outs = [res.results[c]["out"] for c in range(NCORES)]
    return np.concatenate(outs, axis=0).astype(np.float32)


if __name__ == "__main__":
    rng = np.random.RandomState(0)
    print("building 1-layer nc for smoke...")
    import time
    t0 = time.time()
    nc = build_nc(n_layers=1, tap_layer=0)
    print("build+compile took", time.time() - t0)



# revision 5
# speedup vs baseline: 1.6884x; 1.1695x over previous
"""Trainium2 Bass kernel for nn_CIFARViT (spectral group-attention ViT).

kernel(**inputs) takes the FULL unsharded inputs (keys as in setup_inputs),
shards the batch over 8 NeuronCores (pure data parallel, 4 images/core),
runs one Bass program per core via run_bass_kernel_spmd, and gathers the
full [32, 10] output.
"""
import sys
import os

if '/opt/trn_rl_repo' not in sys.path:
    sys.path.insert(0, '/opt/trn_rl_repo')
os.environ.setdefault("NEURON_RT_RESET_CORES", "1")

import numpy as np

import concourse.bass as bass
import concourse.mybir as mybir
from concourse import bacc
from concourse.tile import TileContext
from concourse.bass_utils import run_bass_kernel_spmd

F32 = mybir.dt.float32
F32R = mybir.dt.float32r
AF = mybir.ActivationFunctionType
ALU = mybir.AluOpType
AX = mybir.AxisListType

D = 8
C = 768
H = 8
B = 32
BL = 4
NCORES = 8
N = 256
CH = 96
N_EIGS = 5
NCLS = 10
FC = C // 128
SCALE_QK = CH ** -0.5

K = 16
ROUNDS = [4, 4, 1, 1, 1, 1, 1, 1]
if os.environ.get("KERNEL_R1"):
    ROUNDS = [int(os.environ["KERNEL_R1"])] * 8
D_CHEB = 2
A_INT, B_INT = 0.95, 1.35
C_CENT = (B_INT + A_INT) / 2.0
CHEB_S = 4.0 / (B_INT - A_INT)
ETA = 1e-2
NS_MID = (3e-3, 5)
NS_F1 = (1e-3, 7)
NS_F2 = (1e-5, 4)
DIAG_ITERS = 6
DIAG_CAP = 0.45
DIAG_DAMP = 0.4
TIE_D = 1e-5
GM = 16
NG4 = 4

_SEED = 1234


def _host_consts():
    rng = np.random.RandomState(_SEED)
    X0 = rng.randn(N, K).astype(np.float32)
    R0 = (rng.randn(N, K).astype(np.float32) / np.sqrt(N)) * ETA
    cs = {}
    cs["x0c"] = np.ascontiguousarray(X0.reshape(2, 128, K).transpose(1, 0, 2))
    cs["r0c"] = np.ascontiguousarray(R0.reshape(2, 128, K).transpose(1, 0, 2))
    cs["i128"] = np.eye(128, dtype=np.float32)
    cs["ni128"] = -np.eye(128, dtype=np.float32)
    cs["onesrow"] = np.ones((1, 128), dtype=np.float32)
    cs["ones128"] = np.ones((128, 1), dtype=np.float32)
    dm = np.zeros((2, 128, 256), dtype=np.float32)
    for r in range(2):
        for p in range(128):
            dm[r, p, 128 * r + p] = 1.0
    cs["dmask"] = np.ascontiguousarray(dm.transpose(1, 0, 2))
    cs["vmask"] = np.ascontiguousarray(1.0 - dm.transpose(1, 0, 2))
    import itertools
    combs = np.array(list(itertools.product([1.0, -1.0], repeat=N_EIGS)),
                     dtype=np.float32)
    c4 = np.zeros((128, 32), dtype=np.float32)
    for r in range(4):
        c4[32 * r:32 * r + N_EIGS, :] = combs.T
    cs["combs4"] = c4
    eye = np.zeros((128, 512), dtype=np.float32)
    blk = np.zeros((128, 512), dtype=np.float32)
    iot = np.full((128, 512), 1e9, dtype=np.float32)
    for g4 in range(4):
        for r in range(4):
            for i in range(K):
                eye[32 * r + i, 128 * g4 + 32 * r + i] = 1.0
                blk[32 * r + i, 128 * g4 + 32 * r:128 * g4 + 32 * r + K] = 1.0
                for j in range(N_EIGS):
                    iot[32 * r + i, 128 * g4 + 32 * r + j] = float(j)
    cs["sm_eye"] = eye
    cs["sm_eye15"] = 1.5 * eye
    cs["sm_blk"] = blk
    cs["sm_offblk"] = blk - eye
    cs["sm_iotasel"] = iot
    cs["sm_tie"] = (np.arange(128, dtype=np.float32) % 32 * TIE_D).reshape(128, 1)
    e32 = np.zeros((4, 128), dtype=np.float32)
    for r in range(4):
        e32[r, 32 * r:32 * (r + 1)] = 1.0
    cs["exp32"] = e32
    return cs


CONSTS = _host_consts()


def _prep_weights(inputs):
    w = {}
    qkv_w = np.asarray(inputs["qkv_w"], dtype=np.float32)
    w["wq"] = np.ascontiguousarray(qkv_w[:, :, 0:C]).reshape(D, FC, 128, C)
    w["wv"] = np.ascontiguousarray(qkv_w[:, :, 2 * C:3 * C]).reshape(D, FC, 128, C)
    w["wproj"] = np.ascontiguousarray(np.asarray(inputs["proj_w"], dtype=np.float32))
    w["bproj"] = np.asarray(inputs["proj_b"], dtype=np.float32).reshape(D, FC, 128, 1).copy()
    w["lns"] = np.asarray(inputs["g_ln_s"], dtype=np.float32).reshape(D, FC, 128, 1).copy()
    w["lnb"] = np.asarray(inputs["g_ln_b"], dtype=np.float32).reshape(D, FC, 128, 1).copy()
    w["w1"] = np.ascontiguousarray(np.asarray(inputs["mlp_w1"], dtype=np.float32)).reshape(D, FC, 128, 4 * C)
    w["b1"] = np.asarray(inputs["mlp_b1"], dtype=np.float32).reshape(D, 24, 128, 1).copy()
    w["w2"] = np.ascontiguousarray(np.asarray(inputs["mlp_w2"], dtype=np.float32)).reshape(D, 24, 128, C)
    w["b2"] = np.asarray(inputs["mlp_b2"], dtype=np.float32).reshape(D, FC, 128, 1).copy()
    w["lns2"] = np.asarray(inputs["ln_s"], dtype=np.float32).reshape(FC, 128, 1).copy()
    w["lnb2"] = np.asarray(inputs["ln_b"], dtype=np.float32).reshape(FC, 128, 1).copy()
    w["whead"] = np.asarray(inputs["head_w"], dtype=np.float32).reshape(FC, 128, NCLS).copy()
    w["bhead"] = np.asarray(inputs["head_b"], dtype=np.float32).reshape(1, NCLS).copy()
    pw = np.asarray(inputs["patch_w"], dtype=np.float32).reshape(C, 12)
    w["pwT"] = np.ascontiguousarray(pw.T)
    w["pbias"] = np.asarray(inputs["patch_b"], dtype=np.float32).reshape(FC, 128, 1).copy()
    pos = np.asarray(inputs["pos_emb"], dtype=np.float32).reshape(N, C)
    w["posT"] = np.ascontiguousarray(pos.T).reshape(FC, 128, N)
    return w


def _prep_x(x):
    Bb = x.shape[0]
    xp = np.asarray(x, dtype=np.float32).reshape(Bb, 3, 16, 2, 16, 2)
    xp = xp.transpose(0, 2, 4, 1, 3, 5).reshape(Bb, N, 12)
    return np.ascontiguousarray(np.swapaxes(xp, 1, 2))


# ====================== device program ======================

def build_nc(n_layers=D, tap_layer=None):
    nc = bacc.Bacc("TRN2", target_bir_lowering=False, debug=False)
    dram = {}

    def din(name, shape):
        dram[name] = nc.dram_tensor(name, list(shape), F32, kind="ExternalInput")

    din("xpt", (BL, 12, N))
    din("pwT", (12, C))
    din("pbias", (FC, 128, 1))
    din("posT", (FC, 128, N))
    din("wq", (D, FC, 128, C))
    din("wv", (D, FC, 128, C))
    din("wproj", (D, CH, C))
    din("bproj", (D, FC, 128, 1))
    din("lns", (D, FC, 128, 1))
    din("lnb", (D, FC, 128, 1))
    din("w1", (D, FC, 128, 4 * C))
    din("b1", (D, 24, 128, 1))
    din("w2", (D, 24, 128, C))
    din("b2", (D, FC, 128, 1))
    din("lns2", (FC, 128, 1))
    din("lnb2", (FC, 128, 1))
    din("whead", (FC, 128, NCLS))
    din("bhead", (1, NCLS))
    for cn, arr in CONSTS.items():
        din(cn, arr.shape)

    out = nc.dram_tensor("out", [BL, NCLS], F32, kind="ExternalOutput")
    tap = None
    if tap_layer is not None:
        tap = nc.dram_tensor("tap", [128, FC, BL, N], F32, kind="ExternalOutput")
    if os.environ.get("KERNEL_DEBUG_TAPS"):
        nc._dbg = {
            "d_emb": nc.dram_tensor("d_emb", [128, FC, BL, N], F32, kind="ExternalOutput"),
            "d_S": nc.dram_tensor("d_S", [128, GM, 2, N], F32, kind="ExternalOutput"),
            "d_V": nc.dram_tensor("d_V", [128, GM, 2, CH], F32, kind="ExternalOutput"),
            "d_u0": nc.dram_tensor("d_u0", [128, GM, 2, 1], F32, kind="ExternalOutput"),
            "d_X": nc.dram_tensor("d_X", [128, GM, 2, K], F32, kind="ExternalOutput"),
            "d_B": nc.dram_tensor("d_B", [128, 512], F32, kind="ExternalOutput"),
            "d_Ut": nc.dram_tensor("d_Ut", [128, NG4, N], F32, kind="ExternalOutput"),
            "d_att": nc.dram_tensor("d_att", [128, FC, BL, N], F32, kind="ExternalOutput"),
        }
    else:
        nc._dbg = {}

    with TileContext(nc) as tc:
        _emit(nc, tc, dram, out, tap, n_layers, tap_layer)
    nc.compile()
    return nc


def _emit(nc, tc, dram, out, tap, n_layers, tap_layer):
    import contextlib
    es = contextlib.ExitStack()
    with es:
        persist = es.enter_context(tc.tile_pool(name="persist", bufs=1))
        wpool = es.enter_context(tc.tile_pool(name="wpool", bufs=1))
        wstr = es.enter_context(tc.tile_pool(name="wstr", bufs=2))
        wqrt = es.enter_context(tc.tile_pool(name="wqrt", bufs=1))
        spool = es.enter_context(tc.tile_pool(name="spool", bufs=1))
        epool = es.enter_context(tc.tile_pool(name="epool", bufs=1))
        scr = es.enter_context(tc.tile_pool(name="scr", bufs=1))
        rowp = es.enter_context(tc.tile_pool(name="rowp", bufs=1))
        ps1 = es.enter_context(tc.tile_pool(name="ps1", bufs=2, space="PSUM"))
        ps2 = es.enter_context(tc.tile_pool(name="ps2", bufs=2, space="PSUM"))
        ps3 = es.enter_context(tc.tile_pool(name="ps3", bufs=2, space="PSUM"))

        cst = {}
        for cn, arr in CONSTS.items():
            t = persist.tile(list(arr.shape), F32, tag="c_" + cn)
            cst[cn] = t
            nc.sync.dma_start(t[:], dram[cn][:])

        def smc(name):
            return cst[name][:].rearrange("p (a b) -> p a b", a=4)

        hT = persist.tile([128, FC, BL, N], F32, tag="hT")

        def r32(ap):
            return ap

        TT = nc.vector.tensor_tensor
        TS = nc.vector.tensor_scalar
        STT = nc.vector.scalar_tensor_tensor
        CP = nc.vector.tensor_copy
        MM = nc.tensor.matmul

        # ============ patch embed ============
        xpt = persist.tile([12, BL, N], F32, tag="xpt")
        nc.sync.dma_start(xpt[:], dram["xpt"][:].rearrange("b k n -> k b n"))
        pwT = persist.tile([12, C], F32, tag="pwT")
        nc.sync.dma_start(pwT[:], dram["pwT"][:])
        pbias = persist.tile([128, FC, 1], F32, tag="pbias")
        nc.sync.dma_start(pbias[:], dram["pbias"][:].rearrange("f p o -> p f o"))
        posT = persist.tile([128, FC, N], F32, tag="posT")
        nc.sync.dma_start(posT[:], dram["posT"][:].rearrange("f p n -> p f n"))

        for b in range(BL):
            for ch in range(FC):
                ps = ps1.tile([128, 512], F32, tag="ps1")
                MM(ps[:, 0:N], r32(pwT[:, 128 * ch:128 * (ch + 1)]),
                   r32(xpt[:, b]), start=True, stop=True)
                tmp = scr.tile([128, N], F32, tag="ge")
                TS(tmp[:], ps[:, 0:N], pbias[:, ch], None, op0=ALU.add)
                TT(hT[:, ch, b], tmp[:], posT[:, ch], op=ALU.add)

        if nc._dbg:
            nc.sync.dma_start(nc._dbg["d_emb"][:], hT[:])
        # ===================== layers =====================
        for li in range(n_layers):
            wq = wpool.tile([128, FC, C], F32, tag="wq")
            nc.sync.dma_start(wq[:], dram["wq"][li].rearrange("f p c -> p f c"))
            wproj = wpool.tile([CH, C], F32, tag="wproj")
            nc.sync.dma_start(wproj[:], dram["wproj"][li])
            bproj = wpool.tile([128, FC, 1], F32, tag="bproj")
            nc.sync.dma_start(bproj[:], dram["bproj"][li].rearrange("f p o -> p f o"))
            lns = wpool.tile([128, FC, 1], F32, tag="lns")
            nc.sync.dma_start(lns[:], dram["lns"][li].rearrange("f p o -> p f o"))
            lnb = wpool.tile([128, FC, 1], F32, tag="lnb")
            nc.sync.dma_start(lnb[:], dram["lnb"][li].rearrange("f p o -> p f o"))
            b1t = wpool.tile([128, 24, 1], F32, tag="b1t")
            nc.sync.dma_start(b1t[:], dram["b1"][li].rearrange("j p o -> p j o"))
            b2t = wpool.tile([128, FC, 1], F32, tag="b2t")
            nc.sync.dma_start(b2t[:], dram["b2"][li].rearrange("f p o -> p f o"))

            for g in range(2):
                bidx = [2 * g, 2 * g + 1]
                Sg = spool.tile([128, GM, 2, N], F32, tag="Sg")
                Vg = spool.tile([128, GM, 2, CH], F32, tag="Vg")
                U0g = spool.tile([128, GM, 2, 1], F32, tag="U0g")
                U0r = spool.tile([128, NG4, N], F32, tag="U0r")

                # ---- v = hT^T Wv per matrix (wv streamed, SBUF-accumulate) --
                for ch in range(FC):
                    wvc = wstr.tile([128, C], F32, tag="wvc")
                    nc.sync.dma_start(wvc[:], dram["wv"][li, ch])
                    for m in range(GM):
                        b = bidx[m // H]
                        h = m % H
                        for rb in range(2):
                            psv = ps3.tile([128, 256], F32, tag="ps3")
                            MM(psv[:, 0:CH],
                               r32(hT[:, ch, b, 128 * rb:128 * (rb + 1)]),
                               r32(wvc[:, CH * h:CH * (h + 1)]),
                               start=True, stop=True)
                            if ch == 0:
                                CP(Vg[:, m, rb], psv[:, 0:CH])
                            else:
                                TT(Vg[:, m, rb], Vg[:, m, rb], psv[:, 0:CH],
                                   op=ALU.add)

                # ---- phase A (exp table): logits -> G into Sg ----
                for m in range(GM):
                    b = bidx[m // H]
                    h = m % H
                    qT = scr.tile([CH, N], F32, tag="qT")
                    psq = ps1.tile([128, 512], F32, tag="ps1")
                    for ch in range(FC):
                        MM(psq[0:CH, 0:N],
                           r32(wq[:, ch, CH * h:CH * (h + 1)]),
                           r32(hT[:, ch, b]),
                           start=(ch == 0), stop=(ch == FC - 1))
                    CP(qT[:], psq[0:CH, 0:N])
                    psl = ps2.tile([128, 512], F32, tag="ps2")
                    for rb in range(2):
                        MM(psl[:, 256 * rb:256 * (rb + 1)],
                           r32(qT[:, 128 * rb:128 * (rb + 1)]),
                           r32(qT[:]), start=True, stop=True)
                    rm = scr.tile([128, 1], F32, tag="rm")
                    nc.vector.reduce_max(rm[:], psl[:], axis=AX.X)
                    pst = ps3.tile([128, 256], F32, tag="ps3")
                    MM(pst[0:1, 0:128], rm[:], cst["i128"][:], start=True, stop=True)
                    mv = scr.tile([1, 1], F32, tag="mv")
                    nc.vector.reduce_max(mv[:], pst[0:1, 0:128], axis=AX.X)
                    pbc = ps3.tile([128, 256], F32, tag="ps3")
                    MM(pbc[:, 0:1], cst["onesrow"][:], mv[:], start=True, stop=True)
                    negm = scr.tile([128, 1], F32, tag="negm")
                    nc.scalar.mul(negm[:], pbc[:, 0:1], -SCALE_QK)
                    for rb in range(2):
                        ge = scr.tile([128, N], F32, tag="ge")
                        nc.scalar.activation(ge[:], psl[:, 256 * rb:256 * (rb + 1)],
                                             AF.Exp, bias=negm[:, 0:1],
                                             scale=SCALE_QK)
                        TT(Sg[:, m, rb], ge[:], cst["vmask"][:, rb], op=ALU.mult)

                # ---- phase B (sqrt table): G -> Ahat, u0 ----
                for m in range(GM):
                    g4, rr_ = m // 4, m % 4
                    rsum = scr.tile([128, 2], F32, tag="rsum")
                    for rb in range(2):
                        nc.vector.reduce_sum(rsum[:, rb:rb + 1], Sg[:, m, rb],
                                             axis=AX.X)
                    invr = scr.tile([128, 2], F32, tag="invr")
                    nc.vector.reciprocal(invr[:], rsum[:])
                    ptv = ps3.tile([128, 256], F32, tag="ps3")
                    for ob in range(2):
                        for kb in range(2):
                            MM(ptv[:, ob:ob + 1],
                               Sg[:, m, kb, 128 * ob:128 * (ob + 1)],
                               invr[:, kb:kb + 1],
                               start=(kb == 0), stop=(kb == 1))
                    deg = scr.tile([128, 2], F32, tag="deg")
                    TS(deg[:], ptv[:, 0:2], 0.5, 0.5, op0=ALU.mult, op1=ALU.add)
                    sd = scr.tile([128, 2], F32, tag="sd")
                    nc.scalar.activation(sd[:], deg[:], AF.Sqrt)
                    wv_ = scr.tile([128, 2], F32, tag="wv_")
                    nc.vector.reciprocal(wv_[:], sd[:])
                    alpha = scr.tile([128, 2], F32, tag="alpha")
                    TT(alpha[:], wv_[:], invr[:], op=ALU.mult)
                    nc.scalar.mul(alpha[:], alpha[:], 0.5)
                    psn = ps3.tile([128, 256], F32, tag="ps3")
                    for kb in range(2):
                        MM(psn[0:1, 0:1], deg[:, kb:kb + 1], cst["ones128"][:],
                           start=(kb == 0), stop=(kb == 1))
                    nrm = scr.tile([1, 1], F32, tag="nrm")
                    nc.scalar.activation(nrm[:], psn[0:1, 0:1], AF.Sqrt)
                    rn = scr.tile([1, 1], F32, tag="rn")
                    nc.vector.reciprocal(rn[:], nrm[:])
                    pbc = ps3.tile([128, 256], F32, tag="ps3")
                    MM(pbc[:, 0:1], cst["onesrow"][:], rn[:], start=True, stop=True)
                    rnb = scr.tile([128, 1], F32, tag="rnb")
                    CP(rnb[:], pbc[:, 0:1])
                    for kb in range(2):
                        TS(U0g[:, m, kb], sd[:, kb:kb + 1], rnb[:, 0:1], None,
                           op0=ALU.mult)
                    arow = rowp.tile([1, N], F32, tag="arow")
                    brow = rowp.tile([1, N], F32, tag="brow")
                    for (row_t, col_t) in ((arow, alpha), (brow, wv_)):
                        ptr2 = ps3.tile([128, 256], F32, tag="ps3")
                        for kb in range(2):
                            MM(ptr2[0:1, 128 * kb:128 * (kb + 1)],
                               col_t[:, kb:kb + 1], cst["i128"][:],
                               start=True, stop=True)
                        CP(row_t[:], ptr2[0:1, :])
                    ptr3 = ps3.tile([128, 256], F32, tag="ps3")
                    for kb in range(2):
                        MM(ptr3[32 * rr_:32 * rr_ + 1, 128 * kb:128 * (kb + 1)],
                           U0g[:, m, kb], cst["i128"][:],
                           start=True, stop=True, tile_position=(0, 32 * rr_))
                    CP(U0r[32 * rr_:32 * rr_ + 1, g4], ptr3[32 * rr_:32 * rr_ + 1, :])
                    for rb in range(2):
                        pso = ps1.tile([128, 512], F32, tag="ps1")
                        MM(pso[:, 0:N], arow[:, 128 * rb:128 * (rb + 1)],
                           brow[:], start=True, stop=False)
                        MM(pso[:, 0:N], brow[:, 128 * rb:128 * (rb + 1)],
                           arow[:], start=False, stop=True)
                        go = scr.tile([128, N], F32, tag="ge")
                        TT(go[:], Sg[:, m, rb], pso[:, 0:N], op=ALU.mult)
                        STT(Sg[:, m, rb], go[:], -CHEB_S, cst["dmask"][:, rb],
                            op0=ALU.mult, op1=ALU.bypass)
                        STT(Sg[:, m, rb], cst["dmask"][:, rb],
                            CHEB_S * (1.0 - C_CENT), Sg[:, m, rb],
                            op0=ALU.mult, op1=ALU.add)

                if nc._dbg and li == 0 and g == 0:
                    nc.sync.dma_start(nc._dbg["d_S"][:], Sg[:])
                    nc.sync.dma_start(nc._dbg["d_V"][:], Vg[:])
                    nc.sync.dma_start(nc._dbg["d_u0"][:], U0g[:])
                _eigensolve_and_attend(
                    nc, tc, cst, smc, dram, hT, Sg, Vg, U0g, U0r,
                    wproj, bproj, epool, spool, scr, ps1, ps2, ps3,
                    TT, TS, STT, CP, MM, r32, li, g, bidx)

            if nc._dbg and li == 0:
                nc.sync.dma_start(nc._dbg["d_att"][:], hT[:])
            _mlp_block(nc, tc, cst, dram, hT, lns, lnb, b1t, b2t,
                       wqrt, spool, scr, ps1, ps2, ps3,
                       TT, TS, STT, CP, MM, r32, li)

            if tap_layer is not None and li == tap_layer:
                nc.sync.dma_start(tap[:], hT[:])

        # ============ pool + final LN + head ============
        lns2 = wpool.tile([128, FC, 1], F32, tag="lns")
        nc.sync.dma_start(lns2[:], dram["lns2"][:].rearrange("f p o -> p f o"))
        lnb2 = wpool.tile([128, FC, 1], F32, tag="lnb")
        nc.sync.dma_start(lnb2[:], dram["lnb2"][:].rearrange("f p o -> p f o"))
        whead = wpool.tile([128, FC, NCLS], F32, tag="whead")
        nc.sync.dma_start(whead[:], dram["whead"][:].rearrange("f p c -> p f c"))
        bhead = wpool.tile([1, NCLS], F32, tag="bhead")
        nc.sync.dma_start(bhead[:], dram["bhead"][:])

        for b in range(BL):
            pooled = scr.tile([128, FC], F32, tag="pooled")
            for ch in range(FC):
                nc.vector.reduce_sum(pooled[:, ch:ch + 1], hT[:, ch, b],
                                     axis=AX.X)
            nc.scalar.mul(pooled[:], pooled[:], 1.0 / N)
            psa = ps3.tile([128, 256], F32, tag="ps3")
            for ch in range(FC):
                MM(psa[0:1, 0:1], pooled[:, ch:ch + 1], cst["ones128"][:],
                   start=(ch == 0), stop=(ch == FC - 1))
            sq = scr.tile([128, FC], F32, tag="poolsq")
            nc.scalar.activation(sq[:], pooled[:], AF.Square)
            psb = ps3.tile([128, 256], F32, tag="ps3")
            for ch in range(FC):
                MM(psb[0:1, 0:1], sq[:, ch:ch + 1], cst["ones128"][:],
                   start=(ch == 0), stop=(ch == FC - 1))
            mean = scr.tile([1, 1], F32, tag="fmean")
            nc.scalar.mul(mean[:], psa[0:1, 0:1], 1.0 / C)
            msq = scr.tile([1, 1], F32, tag="fmsq")
            nc.scalar.mul(msq[:], psb[0:1, 0:1], 1.0 / C)
            m2 = scr.tile([1, 1], F32, tag="fm2")
            nc.scalar.activation(m2[:], mean[:], AF.Square)
            var = scr.tile([1, 1], F32, tag="fvar")
            TT(var[:], msq[:], m2[:], op=ALU.subtract)
            TS(var[:], var[:], 1e-5, None, op0=ALU.add)
            sdv = scr.tile([1, 1], F32, tag="fsdv")
            nc.scalar.activation(sdv[:], var[:], AF.Sqrt)
            rstd = scr.tile([1, 1], F32, tag="frstd")
            nc.vector.reciprocal(rstd[:], sdv[:])
            two = scr.tile([1, 2], F32, tag="ftwo")
            CP(two[:, 0:1], mean[:])
            CP(two[:, 1:2], rstd[:])
            psc = ps3.tile([128, 256], F32, tag="ps3")
            MM(psc[:, 0:2], cst["onesrow"][:], two[:], start=True, stop=True)
            meanb = scr.tile([128, 1], F32, tag="fmeanb")
            rstdb = scr.tile([128, 1], F32, tag="frstdb")
            CP(meanb[:], psc[:, 0:1])
            CP(rstdb[:], psc[:, 1:2])
            pnorm = scr.tile([128, FC], F32, tag="pnorm")
            TS(pnorm[:], pooled[:], meanb[:, 0:1], None, op0=ALU.subtract)
            TS(pnorm[:], pnorm[:], rstdb[:, 0:1], None, op0=ALU.mult)
            for ch in range(FC):
                TS(pnorm[:, ch:ch + 1], pnorm[:, ch:ch + 1],
                   lns2[:, ch], lnb2[:, ch], op0=ALU.mult, op1=ALU.add)
            psh = ps3.tile([128, 256], F32, tag="ps3")
            for ch in range(FC):
                MM(psh[0:1, 0:NCLS], pnorm[:, ch:ch + 1], whead[:, ch],
                   start=(ch == 0), stop=(ch == FC - 1))
            ologit = scr.tile([1, NCLS], F32, tag="ologit")
            TT(ologit[:], psh[0:1, 0:NCLS], bhead[:], op=ALU.add)
            nc.sync.dma_start(out[b:b + 1, :], ologit[:])


def _eigensolve_and_attend(nc, tc, cst, smc, dram, hT, Sg, Vg, U0g, U0r,
                           wproj, bproj, epool, spool, scr, ps1, ps2, ps3,
                           TT, TS, STT, CP, MM, r32, li, g, bidx):
    Xg = epool.tile([128, GM, 2, K], F32, tag="Xg")
    for m in range(GM):
        for kb in range(2):
            CP(Xg[:, m, kb], cst["x0c"][:, kb])
    Xt = epool.tile([128, NG4, N], F32, tag="Xt")

    def cheb_round():
        for m in range(GM):
            Tp = epool.tile([128, 2, K], F32, tag="Tp")
            Tc = epool.tile([128, 2, K], F32, tag="Tc")
            for kb in range(2):
                CP(Tp[:, kb], Xg[:, m, kb])
            psx = ps2.tile([128, 512], F32, tag="ps2")
            for ob in range(2):
                for kb in range(2):
                    MM(psx[:, K * ob:K * (ob + 1)],
                       Sg[:, m, kb, 128 * ob:128 * (ob + 1)],
                       Tp[:, kb], start=(kb == 0), stop=(kb == 1))
            for kb in range(2):
                nc.scalar.mul(Tc[:, kb], psx[:, K * kb:K * (kb + 1)], 0.5)
            for _ in range(D_CHEB - 1):
                psy = ps2.tile([128, 512], F32, tag="ps2")
                for ob in range(2):
                    for kb in range(2):
                        MM(psy[:, K * ob:K * (ob + 1)],
                           Sg[:, m, kb, 128 * ob:128 * (ob + 1)],
                           Tc[:, kb], start=(kb == 0), stop=False)
                    MM(psy[:, K * ob:K * (ob + 1)], cst["ni128"][:], Tp[:, ob],
                       start=False, stop=True)
                for kb in range(2):
                    CP(Tp[:, kb], Tc[:, kb])
                    CP(Tc[:, kb], psy[:, K * kb:K * (kb + 1)])
            for kb in range(2):
                CP(Xg[:, m, kb], Tc[:, kb])

    def deflate():
        for m in range(GM):
            g4, r = m // 4, m % 4
            psp = ps3.tile([128, 256], F32, tag="ps3")
            for kb in range(2):
                MM(psp[32 * r:32 * r + 1, 0:K], U0g[:, m, kb], Xg[:, m, kb],
                   start=(kb == 0), stop=(kb == 1), tile_position=(0, 32 * r))
            pr = scr.tile([128, K], F32, tag="pr")
            nc.scalar.mul(pr[32 * r:32 * r + 1, :], psp[32 * r:32 * r + 1, 0:K], -1.0)
            psd = ps2.tile([128, 512], F32, tag="ps2")
            for kb in range(2):
                MM(psd[:, K * kb:K * (kb + 1)], cst["i128"][:], Xg[:, m, kb],
                   start=True, stop=False)
                MM(psd[:, K * kb:K * (kb + 1)],
                   U0r[32 * r:32 * r + 1, g4, 128 * kb:128 * (kb + 1)],
                   pr[32 * r:32 * r + 1, :],
                   start=False, stop=True, tile_position=(32 * r, 0))
            for kb in range(2):
                CP(Xg[:, m, kb], psd[:, K * kb:K * (kb + 1)])

    def small_mm(dst, lhsT, rhs):
        psb = ps2.tile([128, 512], F32, tag="ps2")
        pv = psb[:].rearrange("p (a b) -> p a b", a=4)
        for g4 in range(NG4):
            MM(pv[:, g4], lhsT[:, g4], rhs[:, g4], start=True, stop=True)
        CP(dst[:], pv[:])

    def build_xt():
        for m in range(GM):
            g4, r = m // 4, m % 4
            ptx = ps2.tile([128, 512], F32, tag="ps2")
            for kb in range(2):
                MM(ptx[32 * r:32 * r + K, 128 * kb:128 * (kb + 1)],
                   Xg[:, m, kb], cst["i128"][:],
                   start=True, stop=True, tile_position=(0, 32 * r))
            CP(Xt[32 * r:32 * r + K, g4], ptx[32 * r:32 * r + K, 0:256])

    def bcast_permat(v44):
        ptq = ps3.tile([128, 256], F32, tag="ps3")
        MM(ptq[0:4, 0:4], v44[:], cst["i128"][0:4, 0:4], start=True, stop=True)
        v44t = scr.tile([4, 4], F32, tag="v44t")
        CP(v44t[:], ptq[0:4, 0:4])
        ptw = ps3.tile([128, 256], F32, tag="ps3")
        MM(ptw[:, 0:4], cst["exp32"][:], v44t[:], start=True, stop=True)
        ob = scr.tile([128, 4], F32, tag="permat")
        CP(ob[:], ptw[:, 0:4])
        return ob

    def permat_max(src):
        ptm = ps3.tile([128, 256], F32, tag="ps3")
        MM(ptm[0:4, 0:128], src[:], cst["i128"][:], start=True, stop=True)
        tr = scr.tile([4, 128], F32, tag="tr44")
        CP(tr[:], ptm[0:4, 0:128])
        mx = scr.tile([4, 4], F32, tag="mx44")
        nc.vector.reduce_max(mx[:], tr[:].rearrange("p (a b) -> p a b", a=4),
                             axis=AX.X)
        return mx

    def whiten_pass(eps, steps):
        Gt = epool.tile([128, 4, 128], F32, tag="Gt")
        psg = ps2.tile([128, 512], F32, tag="ps2")
        pvg = psg[:].rearrange("p (a b) -> p a b", a=4)
        for m in range(GM):
            g4, r = m // 4, m % 4
            for kb in range(2):
                MM(psg[32 * r:32 * r + K,
                       128 * g4 + 32 * r:128 * g4 + 32 * r + K],
                   Xg[:, m, kb], Xg[:, m, kb],
                   start=(kb == 0), stop=(kb == 1), tile_position=(0, 32 * r))
        TT(Gt[:], pvg[:], smc("sm_blk"), op=ALU.mult)
        dg = scr.tile([128, 4, 128], F32, tag="smA")
        TT(dg[:], Gt[:], smc("sm_eye"), op=ALU.mult)
        dgt = scr.tile([128, 4], F32, tag="dgt")
        nc.vector.reduce_sum(dgt[:], dg[:], axis=AX.X)
        TS(dgt[:], dgt[:], 1e-12, None, op0=ALU.add)
        sq = scr.tile([128, 4], F32, tag="sq")
        nc.scalar.activation(sq[:], dgt[:], AF.Sqrt)
        srec = scr.tile([128, 4], F32, tag="srec")
        nc.vector.reciprocal(srec[:], sq[:])
        for g4 in range(NG4):
            TS(Gt[:, g4], Gt[:, g4], srec[:, g4:g4 + 1], None, op0=ALU.mult)
        GtT = epool.tile([128, 4, 128], F32, tag="GtT")
        small_mm(GtT, Gt, smc("sm_eye"))
        for g4 in range(NG4):
            TS(GtT[:, g4], GtT[:, g4], srec[:, g4:g4 + 1], None, op0=ALU.mult)
        STT(Gt[:], smc("sm_eye"), eps, GtT[:], op0=ALU.mult, op1=ALU.add)
        brs = scr.tile([128, 4], F32, tag="brs4")
        nc.vector.tensor_reduce(brs[:], Gt[:], axis=AX.X, op=ALU.add,
                                apply_absolute_value=True)
        mx = permat_max(brs)
        rec = scr.tile([4, 4], F32, tag="rec44")
        nc.vector.reciprocal(rec[:], mx[:])
        bre = bcast_permat(rec)
        for g4 in range(NG4):
            TS(Gt[:, g4], Gt[:, g4], bre[:, g4:g4 + 1], None, op0=ALU.mult)
        Yt = epool.tile([128, 4, 128], F32, tag="Yt")
        Zt = epool.tile([128, 4, 128], F32, tag="Zt")
        Tt = epool.tile([128, 4, 128], F32, tag="Tt")
        CP(Yt[:], Gt[:])
        CP(Zt[:], smc("sm_eye"))
        for _ in range(steps):
            psb = ps2.tile([128, 512], F32, tag="ps2")
            pv2 = psb[:].rearrange("p (a b) -> p a b", a=4)
            for g4 in range(NG4):
                MM(pv2[:, g4], Zt[:, g4], Yt[:, g4], start=True, stop=True)
            STT(Tt[:], pv2[:], -0.5, smc("sm_eye15"), op0=ALU.mult, op1=ALU.add)
            small_mm(Yt, Yt, Tt)
            small_mm(Zt, Tt, Zt)
        sqb = scr.tile([4, 4], F32, tag="sqb44")
        nc.scalar.activation(sqb[:], mx[:], AF.Sqrt)
        nc.vector.reciprocal(sqb[:], sqb[:])
        sbe = bcast_permat(sqb)
        for g4 in range(NG4):
            TS(Zt[:, g4], Zt[:, g4], srec[:, g4:g4 + 1], None, op0=ALU.mult)
            TS(Zt[:, g4], Zt[:, g4], sbe[:, g4:g4 + 1], None, op0=ALU.mult)
        build_xt()
        for m in range(GM):
            g4, r = m // 4, m % 4
            psx = ps2.tile([128, 512], F32, tag="ps2")
            for kb in range(2):
                MM(psx[:, K * kb:K * (kb + 1)],
                   Xt[32 * r:32 * r + K, g4, 128 * kb:128 * (kb + 1)],
                   Zt[32 * r:32 * r + K, g4, 32 * r:32 * r + K],
                   start=True, stop=True, tile_position=(32 * r, 0))
            for kb in range(2):
                CP(Xg[:, m, kb], psx[:, K * kb:K * (kb + 1)])

    # ---------------- rounds ----------------
    for rr in range(ROUNDS[li]):
        cheb_round()
        deflate()
        pass  # barrier removed
        whiten_pass(*NS_MID)
        pass  # barrier removed
        for m in range(GM):
            for kb in range(2):
                TT(Xg[:, m, kb], Xg[:, m, kb], cst["r0c"][:, kb], op=ALU.add)
    deflate()
    pass  # barrier removed
    whiten_pass(*NS_F1)
    pass  # barrier removed
    whiten_pass(*NS_F2)
    pass  # barrier removed
    if nc._dbg and li == 0 and g == 0:
        nc.sync.dma_start(nc._dbg["d_X"][:], Xg[:])

    # ---------------- B build ----------------
    Bt = epool.tile([128, 4, 128], F32, tag="Bt")
    Wt = epool.tile([128, 4, 128], F32, tag="Wt")
    psgB = ps2.tile([128, 512], F32, tag="ps2")
    pvB = psgB[:].rearrange("p (a b) -> p a b", a=4)
    for m in range(GM):
        g4, r = m // 4, m % 4
        psz = ps1.tile([128, 512], F32, tag="ps1")
        for ob in range(2):
            for kb in range(2):
                MM(psz[:, K * ob:K * (ob + 1)],
                   Sg[:, m, kb, 128 * ob:128 * (ob + 1)],
                   Xg[:, m, kb], start=(kb == 0), stop=(kb == 1))
        Zc = epool.tile([128, 2, K], F32, tag="Zc")
        for kb in range(2):
            CP(Zc[:, kb], psz[:, K * kb:K * (kb + 1)])
        for kb in range(2):
            MM(psgB[32 * r:32 * r + K,
                    128 * g4 + 32 * r:128 * g4 + 32 * r + K],
               Xg[:, m, kb], Zc[:, kb],
               start=(kb == 0), stop=(kb == 1), tile_position=(0, 32 * r))
    TT(Bt[:], pvB[:], smc("sm_blk"), op=ALU.mult)
    BtT = epool.tile([128, 4, 128], F32, tag="GtT")
    small_mm(BtT, Bt, smc("sm_eye"))
    TT(Bt[:], Bt[:], BtT[:], op=ALU.add)
    nc.scalar.mul(Bt[:], Bt[:], 0.5)
    CP(Wt[:], smc("sm_eye"))
    if nc._dbg and li == 0 and g == 0:
        nc.sync.dma_start(nc._dbg["d_B"][:], Bt[:].rearrange("p a b -> p (a b)"))
    build_xt()

    # ---------------- all-pairs diagonalizer ----------------
    Et = epool.tile([128, 4, 128], F32, tag="Et")
    Qt = epool.tile([128, 4, 128], F32, tag="Qt")
    QtT = epool.tile([128, 4, 128], F32, tag="QtT")
    M1 = epool.tile([128, 4, 128], F32, tag="M1")
    for it in range(DIAG_ITERS):
        dmat = scr.tile([128, 4, 128], F32, tag="smA")
        TT(dmat[:], Bt[:], smc("sm_eye"), op=ALU.mult)
        dcol = scr.tile([128, 4], F32, tag="dcol")
        nc.vector.reduce_sum(dcol[:], dmat[:], axis=AX.X)
        Drow = epool.tile([128, 4, 128], F32, tag="GtT")
        small_mm(Drow, smc("sm_blk"), dmat)
        dd = scr.tile([128, 4, 128], F32, tag="smB")
        for g4 in range(NG4):
            STT(dd[:, g4], Drow[:, g4], dcol[:, g4:g4 + 1],
                smc("sm_blk")[:, g4], op0=ALU.subtract, op1=ALU.mult)
        sgn = scr.tile([128, 4, 128], F32, tag="smC")
        TS(sgn[:], dd[:], 0.0, None, op0=ALU.is_ge)
        STT(dd[:], sgn[:], 2e-9, dd[:], op0=ALU.mult, op1=ALU.add)
        TS(dd[:], dd[:], -1e-9, None, op0=ALU.add)
        nc.vector.reciprocal(dd[:], dd[:])
        TT(Et[:], Bt[:], dd[:], op=ALU.mult)
        TT(Et[:], Et[:], smc("sm_offblk"), op=ALU.mult)
        TS(Et[:], Et[:], DIAG_CAP, None, op0=ALU.min)
        TS(Et[:], Et[:], -DIAG_CAP, None, op0=ALU.max)
        EtT = scr.tile([128, 4, 128], F32, tag="smA")
        small_mm(EtT, Et, smc("sm_eye"))
        TT(Et[:], Et[:], EtT[:], op=ALU.subtract)
        nc.scalar.mul(Et[:], Et[:], 0.5)
        ern = scr.tile([128, 4], F32, tag="ern4")
        nc.vector.tensor_reduce(ern[:], Et[:], axis=AX.X, op=ALU.add,
                                apply_absolute_value=True)
        emx = permat_max(ern)
        TS(emx[:], emx[:], 1e-9, None, op0=ALU.add)
        esc = scr.tile([4, 4], F32, tag="esc44")
        nc.vector.reciprocal(esc[:], emx[:])
        TS(esc[:], esc[:], DIAG_DAMP, 1.0, op0=ALU.mult, op1=ALU.min)
        ebe = bcast_permat(esc)
        for g4 in range(NG4):
            TS(Et[:, g4], Et[:, g4], ebe[:, g4:g4 + 1], None, op0=ALU.mult)
        TT(Qt[:], smc("sm_eye"), Et[:], op=ALU.add)
        TT(QtT[:], smc("sm_eye"), Et[:], op=ALU.subtract)
        for _ in range(2):
            P_ = scr.tile([128, 4, 128], F32, tag="smA")
            small_mm(P_, Qt, Qt)
            STT(M1[:], P_[:], -0.5, smc("sm_eye15"), op0=ALU.mult, op1=ALU.add)
            Qn = scr.tile([128, 4, 128], F32, tag="smB")
            small_mm(Qn, QtT, M1)
            small_mm(QtT, M1, QtT)
            CP(Qt[:], Qn[:])
        BQ = scr.tile([128, 4, 128], F32, tag="smA")
        small_mm(BQ, Bt, Qt)
        small_mm(Bt, Qt, BQ)
        small_mm(Wt, Qt, Wt)

    # ---------------- rank selection -> Ut ----------------
    dmat = scr.tile([128, 4, 128], F32, tag="smA")
    TT(dmat[:], Bt[:], smc("sm_eye"), op=ALU.mult)
    dcol = scr.tile([128, 4], F32, tag="dcol")
    nc.vector.reduce_sum(dcol[:], dmat[:], axis=AX.X)
    TS(dcol[:], dcol[:], cst["sm_tie"][:, 0:1], None, op0=ALU.add)
    dmat2 = scr.tile([128, 4, 128], F32, tag="smB")
    for g4 in range(NG4):
        TS(dmat2[:, g4], smc("sm_eye")[:, g4], dcol[:, g4:g4 + 1], None,
           op0=ALU.mult)
    Drow = epool.tile([128, 4, 128], F32, tag="GtT")
    small_mm(Drow, smc("sm_blk"), dmat2)
    Cc = scr.tile([128, 4, 128], F32, tag="smC")
    for g4 in range(NG4):
        STT(Cc[:, g4], Drow[:, g4], dcol[:, g4:g4 + 1],
            smc("sm_blk")[:, g4], op0=ALU.is_lt, op1=ALU.mult)
    rnk = scr.tile([128, 4], F32, tag="rnk")
    nc.vector.reduce_sum(rnk[:], Cc[:], axis=AX.X)
    Sel = scr.tile([128, 4, 128], F32, tag="smC")
    for g4 in range(NG4):
        TS(Sel[:, g4], smc("sm_iotasel")[:, g4], rnk[:, g4:g4 + 1], None,
           op0=ALU.is_equal)
    Pt = epool.tile([128, 4, 128], F32, tag="M1")
    small_mm(Pt, Wt, Sel)

    Utg = epool.tile([128, NG4, N], F32, tag="Utg")
    for g4 in range(NG4):
        psu = ps1.tile([128, 512], F32, tag="ps1")
        for r in range(4):
            MM(psu[32 * r:32 * r + N_EIGS, 0:N],
               Pt[32 * r:32 * r + K, g4, 32 * r:32 * r + N_EIGS],
               Xt[32 * r:32 * r + K, g4],
               start=True, stop=True, tile_position=(32 * r, 32 * r))
        CP(Utg[:, g4], psu[:, 0:N])

    un = scr.tile([128, NG4], F32, tag="un")
    for g4 in range(NG4):
        usq = scr.tile([128, N], F32, tag="ge")
        nc.scalar.activation(usq[:], Utg[:, g4], AF.Square)
        nc.vector.reduce_sum(un[:, g4:g4 + 1], usq[:], axis=AX.X)
    TS(un[:], un[:], 1e-30, None, op0=ALU.add)
    uns = scr.tile([128, NG4], F32, tag="uns")
    nc.scalar.activation(uns[:], un[:], AF.Sqrt)
    nc.vector.reciprocal(uns[:], uns[:])
    for g4 in range(NG4):
        TS(Utg[:, g4], Utg[:, g4], uns[:, g4:g4 + 1], None, op0=ALU.mult)

    if nc._dbg and li == 0 and g == 0:
        nc.sync.dma_start(nc._dbg["d_Ut"][:], Utg[:])
    # ---------------- group attention + out + proj ----------------
    aoutT = [spool.tile([CH, N], F32, tag="aoutT" + str(i), name="aoutT" + str(i)) for i in range(2)]
    for m in range(GM):
        g4, r = m // 4, m % 4
        bloc = m // H
        h = m % H
        psga = ps1.tile([128, 512], F32, tag="ps1")
        MM(psga[0:32, 0:N], cst["combs4"][32 * r:32 * r + N_EIGS, :],
           Utg[32 * r:32 * r + N_EIGS, g4],
           start=True, stop=True, tile_position=(32 * r, 0))
        gmx = scr.tile([32, 1], F32, tag="gmx")
        nc.vector.reduce_max(gmx[:], psga[0:32, 0:N], axis=AX.X)
        ngm = scr.tile([32, 1], F32, tag="ngm")
        nc.scalar.mul(ngm[:], gmx[:], -1.0)
        ga = scr.tile([32, N], F32, tag="ga")
        nc.scalar.activation(ga[:], psga[0:32, 0:N], AF.Exp, bias=ngm[:, 0:1])
        gs = scr.tile([32, 1], F32, tag="gs")
        nc.vector.reduce_sum(gs[:], ga[:], axis=AX.X)
        gr = scr.tile([32, 1], F32, tag="gr")
        nc.vector.reciprocal(gr[:], gs[:])
        TS(ga[:], ga[:], gr[:, 0:1], None, op0=ALU.mult)
        gaT = scr.tile([128, 2, 32], F32, tag="gaT")
        psgt = ps3.tile([128, 256], F32, tag="ps3")
        for kb in range(2):
            MM(psgt[:, 32 * kb:32 * (kb + 1)], ga[:, 128 * kb:128 * (kb + 1)],
               cst["i128"][0:32, 0:32], start=True, stop=True)
            CP(gaT[:, kb], psgt[:, 32 * kb:32 * (kb + 1)])
        psoh = ps3.tile([128, 256], F32, tag="ps3")
        for kb in range(2):
            MM(psoh[0:CH, 0:32], Vg[:, m, kb], gaT[:, kb],
               start=(kb == 0), stop=(kb == 1))
        CP(aoutT[bloc][:, 32 * h:32 * (h + 1)], psoh[0:CH, 0:32])

    for bloc in range(2):
        b = bidx[bloc]
        for ch in range(FC):
            psj = ps1.tile([128, 512], F32, tag="ps1")
            MM(psj[:, 0:N], r32(wproj[:, 128 * ch:128 * (ch + 1)]),
               r32(aoutT[bloc][:]), start=True, stop=True)
            TS(hT[:, ch, b], psj[:, 0:N], bproj[:, ch], None, op0=ALU.add)


def _mlp_block(nc, tc, cst, dram, hT, lns, lnb, b1t, b2t,
               wqrt, spool, scr, ps1, ps2, ps3, TT, TS, STT, CP, MM, r32, li):
    for pair in range(2):
        bb = [2 * pair, 2 * pair + 1]
        hnp = spool.tile([128, FC, 2, N], F32, tag="Sg")
        for bloc, b in enumerate(bb):
            pss = ps3.tile([128, 256], F32, tag="ps3")
            ps2s = ps3.tile([128, 256], F32, tag="ps3")
            for ch in range(FC):
                MM(pss[0:1, 0:128], cst["ones128"][:], hT[:, ch, b, 0:128],
                   start=(ch == 0), stop=(ch == FC - 1))
            # NOTE: sums are computed in two half-token blocks (psum free 128)
            for ch in range(FC):
                MM(ps2s[0:1, 0:128], cst["ones128"][:], hT[:, ch, b, 128:256],
                   start=(ch == 0), stop=(ch == FC - 1))
            sums = scr.tile([1, N], F32, tag="lsums")
            CP(sums[:, 0:128], pss[0:1, 0:128])
            CP(sums[:, 128:256], ps2s[0:1, 0:128])
            sq1 = scr.tile([128, N], F32, tag="lt1")
            psq1 = ps3.tile([128, 256], F32, tag="ps3")
            psq2 = ps3.tile([128, 256], F32, tag="ps3")
            for ch in range(FC):
                nc.scalar.activation(sq1[:], hT[:, ch, b], AF.Square)
                MM(psq1[0:1, 0:128], cst["ones128"][:], sq1[:, 0:128],
                   start=(ch == 0), stop=(ch == FC - 1))
                MM(psq2[0:1, 0:128], cst["ones128"][:], sq1[:, 128:256],
                   start=(ch == 0), stop=(ch == FC - 1))
            sqs = scr.tile([1, N], F32, tag="lsqs")
            CP(sqs[:, 0:128], psq1[0:1, 0:128])
            CP(sqs[:, 128:256], psq2[0:1, 0:128])
            mean = scr.tile([1, N], F32, tag="lmean")
            nc.scalar.mul(mean[:], sums[:], 1.0 / C)
            msq = scr.tile([1, N], F32, tag="lmsq")
            nc.scalar.mul(msq[:], sqs[:], 1.0 / C)
            m2 = scr.tile([1, N], F32, tag="lm2")
            nc.scalar.activation(m2[:], mean[:], AF.Square)
            var = scr.tile([1, N], F32, tag="lvar")
            TT(var[:], msq[:], m2[:], op=ALU.subtract)
            TS(var[:], var[:], 1e-5, None, op0=ALU.add)
            sdv = scr.tile([1, N], F32, tag="lsdv")
            nc.scalar.activation(sdv[:], var[:], AF.Sqrt)
            rst = scr.tile([1, N], F32, tag="lrst")
            nc.vector.reciprocal(rst[:], sdv[:])
            mrs = scr.tile([1, N], F32, tag="lmrs")
            TT(mrs[:], mean[:], rst[:], op=ALU.mult)
            psb1 = ps1.tile([128, 512], F32, tag="ps1")
            MM(psb1[:, 0:N], cst["onesrow"][:], rst[:], start=True, stop=True)
            MM(psb1[:, 256:512], cst["onesrow"][:], mrs[:], start=True, stop=True)
            rstB = scr.tile([128, N], F32, tag="lrstB")
            mrsB = scr.tile([128, N], F32, tag="lmrsB")
            CP(rstB[:], psb1[:, 0:N])
            CP(mrsB[:], psb1[:, 256:512])
            for ch in range(FC):
                t1 = scr.tile([128, N], F32, tag="lt1")
                TT(t1[:], hT[:, ch, b], rstB[:], op=ALU.mult)
                TT(t1[:], t1[:], mrsB[:], op=ALU.subtract)
                TS(hnp[:, ch, bloc], t1[:], lns[:, ch], lnb[:, ch],
                   op0=ALU.mult, op1=ALU.add)

        # MLP quarters: a1q = gelu(psum-accum) ; mlp2 accumulates into hT
        for q in range(4):
            w1q = wqrt.tile([128, FC, 768], F32, tag="wqrt")
            nc.sync.dma_start(
                w1q[:], dram["w1"][li, :, :, 768 * q:768 * (q + 1)]
                .rearrange("f p c -> p f c"))
            a1q = spool.tile([128, 6, 2, N], F32, tag="Vg")
            for jc in range(6):
                jg = 6 * q + jc
                psm = ps1.tile([128, 512], F32, tag="ps1")
                for ch in range(FC):
                    MM(psm[:],
                       r32(w1q[:, ch, 128 * jc:128 * (jc + 1)]),
                       r32(hnp[:, ch].rearrange("p a b -> p (a b)")),
                       start=(ch == 0), stop=(ch == FC - 1))
                nc.scalar.activation(
                    a1q[:, jc].rearrange("p a b -> p (a b)"), psm[:],
                    AF.Gelu, bias=b1t[:, jg])
            w2q = wqrt.tile([128, FC, 768], F32, tag="wqrt")
            nc.sync.dma_start(
                w2q[:], dram["w2"][li, 6 * q:6 * (q + 1)]
                .rearrange("j p c -> p j c"))
            for mc in range(FC):
                psm2 = ps1.tile([128, 512], F32, tag="ps1")
                for jc in range(6):
                    MM(psm2[:],
                       r32(w2q[:, jc, 128 * mc:128 * (mc + 1)]),
                       r32(a1q[:, jc].rearrange("p a b -> p (a b)")),
                       start=(jc == 0), stop=(jc == 5))
                hv = hT[:, mc, 2 * pair:2 * pair + 2, :].rearrange("p a b -> p (a b)")
                if q == 0:
                    STT(hv, psm2[:], b2t[:, mc], hv, op0=ALU.add, op1=ALU.add)
                else:
                    TT(hv, hv, psm2[:], op=ALU.add)
            # residual base already in hT (attn out); mlp adds on top


FC_A1 = 6


# ====================== host wrapper ======================

_NC_CACHE = {}


def _get_nc(n_layers=D, tap_layer=None):
    key = (n_layers, tap_layer)
    if key not in _NC_CACHE:
        _NC_CACHE[key] = build_nc(n_layers, tap_layer)
    return _NC_CACHE[key]


def kernel(**inputs):
    x = np.asarray(inputs["x"], dtype=np.float32)
    w = _prep_weights(inputs)
    xp = _prep_x(x)                      # [B, 12, 256]
    nc = _get_nc()
    in_maps = []
    for c in range(NCORES):
        im = {}
        im["xpt"] = np.ascontiguousarray(xp[BL * c:BL * (c + 1)])
        for kname, arr in w.items():
            im[kname] = arr
        for cn, arr in CONSTS.items():
            im[cn] = arr
        in_maps.append(im)
    res = run_bass_kernel_spmd(nc, in_maps, core_ids=list(range(NCORES)))
    outs = [res.results[c]["out"] for c in range(NCORES)]
    return np.concatenate(outs, axis=0).astype(np.float32)


if __name__ == "__main__":
    rng = np.random.RandomState(0)
    print("building 1-layer nc for smoke...")
    import time
    t0 = time.time()
    nc = build_nc(n_layers=1, tap_layer=0)
    print("build+compile took", time.time() - t0)



# revision 6
# speedup vs baseline: 1.8169x; 1.0761x over previous
"""Trainium2 Bass kernel for nn_CIFARViT (spectral group-attention ViT).

kernel(**inputs) takes the FULL unsharded inputs (keys as in setup_inputs),
shards the batch over 8 NeuronCores (pure data parallel, 4 images/core),
runs one Bass program per core via run_bass_kernel_spmd, and gathers the
full [32, 10] output.
"""
import sys
import os

if '/opt/trn_rl_repo' not in sys.path:
    sys.path.insert(0, '/opt/trn_rl_repo')
os.environ.setdefault("NEURON_RT_RESET_CORES", "1")

import numpy as np

import concourse.bass as bass
import concourse.mybir as mybir
from concourse import bacc
from concourse.tile import TileContext
from concourse.bass_utils import run_bass_kernel_spmd

F32 = mybir.dt.float32
F32R = mybir.dt.float32r
AF = mybir.ActivationFunctionType
ALU = mybir.AluOpType
AX = mybir.AxisListType

D = 8
C = 768
H = 8
B = 32
BL = 4
NCORES = 8
N = 256
CH = 96
N_EIGS = 5
NCLS = 10
FC = C // 128
SCALE_QK = CH ** -0.5

K = 16
ROUNDS = [3, 3, 1, 1, 1, 1, 1, 1]
if os.environ.get("KERNEL_R1"):
    ROUNDS = [int(os.environ["KERNEL_R1"])] * 8
D_CHEB = 2
A_INT, B_INT = 0.95, 1.35
C_CENT = (B_INT + A_INT) / 2.0
CHEB_S = 4.0 / (B_INT - A_INT)
ETA = 1e-2
NS_MID = (3e-3, 4)
NS_F1 = (1e-3, 6)
NS_F2 = (1e-5, 3)
DIAG_ITERS = 5
DIAG_CAP = 0.45
DIAG_DAMP = 0.4
TIE_D = 1e-5
GM = 16
NG4 = 4

_SEED = 1234


def _host_consts():
    rng = np.random.RandomState(_SEED)
    X0 = rng.randn(N, K).astype(np.float32)
    R0 = (rng.randn(N, K).astype(np.float32) / np.sqrt(N)) * ETA
    cs = {}
    cs["x0c"] = np.ascontiguousarray(X0.reshape(2, 128, K).transpose(1, 0, 2))
    cs["r0c"] = np.ascontiguousarray(R0.reshape(2, 128, K).transpose(1, 0, 2))
    cs["i128"] = np.eye(128, dtype=np.float32)
    cs["ni128"] = -np.eye(128, dtype=np.float32)
    cs["onesrow"] = np.ones((1, 128), dtype=np.float32)
    cs["ones128"] = np.ones((128, 1), dtype=np.float32)
    dm = np.zeros((2, 128, 256), dtype=np.float32)
    for r in range(2):
        for p in range(128):
            dm[r, p, 128 * r + p] = 1.0
    cs["dmask"] = np.ascontiguousarray(dm.transpose(1, 0, 2))
    cs["vmask"] = np.ascontiguousarray(1.0 - dm.transpose(1, 0, 2))
    import itertools
    combs = np.array(list(itertools.product([1.0, -1.0], repeat=N_EIGS)),
                     dtype=np.float32)
    c4 = np.zeros((128, 32), dtype=np.float32)
    for r in range(4):
        c4[32 * r:32 * r + N_EIGS, :] = combs.T
    cs["combs4"] = c4
    eye = np.zeros((128, 512), dtype=np.float32)
    blk = np.zeros((128, 512), dtype=np.float32)
    iot = np.full((128, 512), 1e9, dtype=np.float32)
    for g4 in range(4):
        for r in range(4):
            for i in range(K):
                eye[32 * r + i, 128 * g4 + 32 * r + i] = 1.0
                blk[32 * r + i, 128 * g4 + 32 * r:128 * g4 + 32 * r + K] = 1.0
                for j in range(N_EIGS):
                    iot[32 * r + i, 128 * g4 + 32 * r + j] = float(j)
    cs["sm_eye"] = eye
    cs["sm_eye15"] = 1.5 * eye
    cs["sm_blk"] = blk
    cs["sm_offblk"] = blk - eye
    cs["sm_iotasel"] = iot
    cs["sm_tie"] = (np.arange(128, dtype=np.float32) % 32 * TIE_D).reshape(128, 1)
    e32 = np.zeros((4, 128), dtype=np.float32)
    for r in range(4):
        e32[r, 32 * r:32 * (r + 1)] = 1.0
    cs["exp32"] = e32
    return cs


CONSTS = _host_consts()


def _prep_weights(inputs):
    w = {}
    qkv_w = np.asarray(inputs["qkv_w"], dtype=np.float32)
    w["wq"] = np.ascontiguousarray(qkv_w[:, :, 0:C]).reshape(D, FC, 128, C)
    w["wv"] = np.ascontiguousarray(qkv_w[:, :, 2 * C:3 * C]).reshape(D, FC, 128, C)
    w["wproj"] = np.ascontiguousarray(np.asarray(inputs["proj_w"], dtype=np.float32))
    w["bproj"] = np.asarray(inputs["proj_b"], dtype=np.float32).reshape(D, FC, 128, 1).copy()
    w["lns"] = np.asarray(inputs["g_ln_s"], dtype=np.float32).reshape(D, FC, 128, 1).copy()
    w["lnb"] = np.asarray(inputs["g_ln_b"], dtype=np.float32).reshape(D, FC, 128, 1).copy()
    w["w1"] = np.ascontiguousarray(np.asarray(inputs["mlp_w1"], dtype=np.float32)).reshape(D, FC, 128, 4 * C)
    w["b1"] = np.asarray(inputs["mlp_b1"], dtype=np.float32).reshape(D, 24, 128, 1).copy()
    w["w2"] = np.ascontiguousarray(np.asarray(inputs["mlp_w2"], dtype=np.float32)).reshape(D, 24, 128, C)
    w["b2"] = np.asarray(inputs["mlp_b2"], dtype=np.float32).reshape(D, FC, 128, 1).copy()
    w["lns2"] = np.asarray(inputs["ln_s"], dtype=np.float32).reshape(FC, 128, 1).copy()
    w["lnb2"] = np.asarray(inputs["ln_b"], dtype=np.float32).reshape(FC, 128, 1).copy()
    w["whead"] = np.asarray(inputs["head_w"], dtype=np.float32).reshape(FC, 128, NCLS).copy()
    w["bhead"] = np.asarray(inputs["head_b"], dtype=np.float32).reshape(1, NCLS).copy()
    pw = np.asarray(inputs["patch_w"], dtype=np.float32).reshape(C, 12)
    w["pwT"] = np.ascontiguousarray(pw.T)
    w["pbias"] = np.asarray(inputs["patch_b"], dtype=np.float32).reshape(FC, 128, 1).copy()
    pos = np.asarray(inputs["pos_emb"], dtype=np.float32).reshape(N, C)
    w["posT"] = np.ascontiguousarray(pos.T).reshape(FC, 128, N)
    return w


def _prep_x(x):
    Bb = x.shape[0]
    xp = np.asarray(x, dtype=np.float32).reshape(Bb, 3, 16, 2, 16, 2)
    xp = xp.transpose(0, 2, 4, 1, 3, 5).reshape(Bb, N, 12)
    return np.ascontiguousarray(np.swapaxes(xp, 1, 2))


# ====================== device program ======================

def build_nc(n_layers=D, tap_layer=None):
    nc = bacc.Bacc("TRN2", target_bir_lowering=False, debug=False)
    dram = {}

    def din(name, shape):
        dram[name] = nc.dram_tensor(name, list(shape), F32, kind="ExternalInput")

    din("xpt", (BL, 12, N))
    din("pwT", (12, C))
    din("pbias", (FC, 128, 1))
    din("posT", (FC, 128, N))
    din("wq", (D, FC, 128, C))
    din("wv", (D, FC, 128, C))
    din("wproj", (D, CH, C))
    din("bproj", (D, FC, 128, 1))
    din("lns", (D, FC, 128, 1))
    din("lnb", (D, FC, 128, 1))
    din("w1", (D, FC, 128, 4 * C))
    din("b1", (D, 24, 128, 1))
    din("w2", (D, 24, 128, C))
    din("b2", (D, FC, 128, 1))
    din("lns2", (FC, 128, 1))
    din("lnb2", (FC, 128, 1))
    din("whead", (FC, 128, NCLS))
    din("bhead", (1, NCLS))
    for cn, arr in CONSTS.items():
        din(cn, arr.shape)

    out = nc.dram_tensor("out", [BL, NCLS], F32, kind="ExternalOutput")
    tap = None
    if tap_layer is not None:
        tap = nc.dram_tensor("tap", [128, FC, BL, N], F32, kind="ExternalOutput")
    if os.environ.get("KERNEL_DEBUG_TAPS"):
        nc._dbg = {
            "d_emb": nc.dram_tensor("d_emb", [128, FC, BL, N], F32, kind="ExternalOutput"),
            "d_S": nc.dram_tensor("d_S", [128, GM, 2, N], F32, kind="ExternalOutput"),
            "d_V": nc.dram_tensor("d_V", [128, GM, 2, CH], F32, kind="ExternalOutput"),
            "d_u0": nc.dram_tensor("d_u0", [128, GM, 2, 1], F32, kind="ExternalOutput"),
            "d_X": nc.dram_tensor("d_X", [128, GM, 2, K], F32, kind="ExternalOutput"),
            "d_B": nc.dram_tensor("d_B", [128, 512], F32, kind="ExternalOutput"),
            "d_Ut": nc.dram_tensor("d_Ut", [128, NG4, N], F32, kind="ExternalOutput"),
            "d_att": nc.dram_tensor("d_att", [128, FC, BL, N], F32, kind="ExternalOutput"),
        }
    else:
        nc._dbg = {}

    with TileContext(nc) as tc:
        _emit(nc, tc, dram, out, tap, n_layers, tap_layer)
    nc.compile()
    return nc


def _emit(nc, tc, dram, out, tap, n_layers, tap_layer):
    import contextlib
    es = contextlib.ExitStack()
    with es:
        persist = es.enter_context(tc.tile_pool(name="persist", bufs=1))
        wpool = es.enter_context(tc.tile_pool(name="wpool", bufs=1))
        wstr = es.enter_context(tc.tile_pool(name="wstr", bufs=2))
        wqrt = es.enter_context(tc.tile_pool(name="wqrt", bufs=1))
        spool = es.enter_context(tc.tile_pool(name="spool", bufs=1))
        epool = es.enter_context(tc.tile_pool(name="epool", bufs=1))
        scr = es.enter_context(tc.tile_pool(name="scr", bufs=1))
        rowp = es.enter_context(tc.tile_pool(name="rowp", bufs=1))
        ps1 = es.enter_context(tc.tile_pool(name="ps1", bufs=2, space="PSUM"))
        ps2 = es.enter_context(tc.tile_pool(name="ps2", bufs=2, space="PSUM"))
        ps3 = es.enter_context(tc.tile_pool(name="ps3", bufs=2, space="PSUM"))

        cst = {}
        for cn, arr in CONSTS.items():
            t = persist.tile(list(arr.shape), F32, tag="c_" + cn)
            cst[cn] = t
            nc.sync.dma_start(t[:], dram[cn][:])

        def smc(name):
            return cst[name][:].rearrange("p (a b) -> p a b", a=4)

        hT = persist.tile([128, FC, BL, N], F32, tag="hT")

        def r32(ap):
            return ap

        TT = nc.vector.tensor_tensor
        TS = nc.vector.tensor_scalar
        STT = nc.vector.scalar_tensor_tensor
        CP = nc.vector.tensor_copy
        MM = nc.tensor.matmul

        # ============ patch embed ============
        xpt = persist.tile([12, BL, N], F32, tag="xpt")
        nc.sync.dma_start(xpt[:], dram["xpt"][:].rearrange("b k n -> k b n"))
        pwT = persist.tile([12, C], F32, tag="pwT")
        nc.sync.dma_start(pwT[:], dram["pwT"][:])
        pbias = persist.tile([128, FC, 1], F32, tag="pbias")
        nc.sync.dma_start(pbias[:], dram["pbias"][:].rearrange("f p o -> p f o"))
        posT = persist.tile([128, FC, N], F32, tag="posT")
        nc.sync.dma_start(posT[:], dram["posT"][:].rearrange("f p n -> p f n"))

        for b in range(BL):
            for ch in range(FC):
                ps = ps1.tile([128, 512], F32, tag="ps1")
                MM(ps[:, 0:N], r32(pwT[:, 128 * ch:128 * (ch + 1)]),
                   r32(xpt[:, b]), start=True, stop=True)
                tmp = scr.tile([128, N], F32, tag="ge")
                TS(tmp[:], ps[:, 0:N], pbias[:, ch], None, op0=ALU.add)
                TT(hT[:, ch, b], tmp[:], posT[:, ch], op=ALU.add)

        if nc._dbg:
            nc.sync.dma_start(nc._dbg["d_emb"][:], hT[:])
        # ===================== layers =====================
        for li in range(n_layers):
            wq = wpool.tile([128, FC, C], F32, tag="wq")
            nc.sync.dma_start(wq[:], dram["wq"][li].rearrange("f p c -> p f c"))
            wproj = wpool.tile([CH, C], F32, tag="wproj")
            nc.sync.dma_start(wproj[:], dram["wproj"][li])
            bproj = wpool.tile([128, FC, 1], F32, tag="bproj")
            nc.sync.dma_start(bproj[:], dram["bproj"][li].rearrange("f p o -> p f o"))
            lns = wpool.tile([128, FC, 1], F32, tag="lns")
            nc.sync.dma_start(lns[:], dram["lns"][li].rearrange("f p o -> p f o"))
            lnb = wpool.tile([128, FC, 1], F32, tag="lnb")
            nc.sync.dma_start(lnb[:], dram["lnb"][li].rearrange("f p o -> p f o"))
            b1t = wpool.tile([128, 24, 1], F32, tag="b1t")
            nc.sync.dma_start(b1t[:], dram["b1"][li].rearrange("j p o -> p j o"))
            b2t = wpool.tile([128, FC, 1], F32, tag="b2t")
            nc.sync.dma_start(b2t[:], dram["b2"][li].rearrange("f p o -> p f o"))

            for g in range(2):
                bidx = [2 * g, 2 * g + 1]
                Sg = spool.tile([128, GM, 2, N], F32, tag="Sg")
                Vg = spool.tile([128, GM, 2, CH], F32, tag="Vg")
                U0g = spool.tile([128, GM, 2, 1], F32, tag="U0g")
                U0r = spool.tile([128, NG4, N], F32, tag="U0r")

                # ---- v = hT^T Wv per matrix (wv streamed, SBUF-accumulate) --
                for ch in range(FC):
                    wvc = wstr.tile([128, C], F32, tag="wvc")
                    nc.sync.dma_start(wvc[:], dram["wv"][li, ch])
                    for m in range(GM):
                        b = bidx[m // H]
                        h = m % H
                        for rb in range(2):
                            psv = ps3.tile([128, 256], F32, tag="ps3")
                            MM(psv[:, 0:CH],
                               r32(hT[:, ch, b, 128 * rb:128 * (rb + 1)]),
                               r32(wvc[:, CH * h:CH * (h + 1)]),
                               start=True, stop=True)
                            if ch == 0:
                                CP(Vg[:, m, rb], psv[:, 0:CH])
                            else:
                                TT(Vg[:, m, rb], Vg[:, m, rb], psv[:, 0:CH],
                                   op=ALU.add)

                # ---- phase A (exp table): logits -> G into Sg ----
                for m in range(GM):
                    b = bidx[m // H]
                    h = m % H
                    qT = scr.tile([CH, N], F32, tag="qT")
                    psq = ps1.tile([128, 512], F32, tag="ps1")
                    for ch in range(FC):
                        MM(psq[0:CH, 0:N],
                           r32(wq[:, ch, CH * h:CH * (h + 1)]),
                           r32(hT[:, ch, b]),
                           start=(ch == 0), stop=(ch == FC - 1))
                    CP(qT[:], psq[0:CH, 0:N])
                    psl = ps2.tile([128, 512], F32, tag="ps2")
                    for rb in range(2):
                        MM(psl[:, 256 * rb:256 * (rb + 1)],
                           r32(qT[:, 128 * rb:128 * (rb + 1)]),
                           r32(qT[:]), start=True, stop=True)
                    rm = scr.tile([128, 1], F32, tag="rm")
                    nc.vector.reduce_max(rm[:], psl[:], axis=AX.X)
                    pst = ps3.tile([128, 256], F32, tag="ps3")
                    MM(pst[0:1, 0:128], rm[:], cst["i128"][:], start=True, stop=True)
                    mv = scr.tile([1, 1], F32, tag="mv")
                    nc.vector.reduce_max(mv[:], pst[0:1, 0:128], axis=AX.X)
                    pbc = ps3.tile([128, 256], F32, tag="ps3")
                    MM(pbc[:, 0:1], cst["onesrow"][:], mv[:], start=True, stop=True)
                    negm = scr.tile([128, 1], F32, tag="negm")
                    nc.scalar.mul(negm[:], pbc[:, 0:1], -SCALE_QK)
                    for rb in range(2):
                        ge = scr.tile([128, N], F32, tag="ge")
                        nc.scalar.activation(ge[:], psl[:, 256 * rb:256 * (rb + 1)],
                                             AF.Exp, bias=negm[:, 0:1],
                                             scale=SCALE_QK)
                        TT(Sg[:, m, rb], ge[:], cst["vmask"][:, rb], op=ALU.mult)

                # ---- phase B (sqrt table): G -> Ahat, u0 ----
                for m in range(GM):
                    g4, rr_ = m // 4, m % 4
                    rsum = scr.tile([128, 2], F32, tag="rsum")
                    for rb in range(2):
                        nc.vector.reduce_sum(rsum[:, rb:rb + 1], Sg[:, m, rb],
                                             axis=AX.X)
                    invr = scr.tile([128, 2], F32, tag="invr")
                    nc.vector.reciprocal(invr[:], rsum[:])
                    ptv = ps3.tile([128, 256], F32, tag="ps3")
                    for ob in range(2):
                        for kb in range(2):
                            MM(ptv[:, ob:ob + 1],
                               Sg[:, m, kb, 128 * ob:128 * (ob + 1)],
                               invr[:, kb:kb + 1],
                               start=(kb == 0), stop=(kb == 1))
                    deg = scr.tile([128, 2], F32, tag="deg")
                    TS(deg[:], ptv[:, 0:2], 0.5, 0.5, op0=ALU.mult, op1=ALU.add)
                    sd = scr.tile([128, 2], F32, tag="sd")
                    nc.scalar.activation(sd[:], deg[:], AF.Sqrt)
                    wv_ = scr.tile([128, 2], F32, tag="wv_")
                    nc.vector.reciprocal(wv_[:], sd[:])
                    alpha = scr.tile([128, 2], F32, tag="alpha")
                    TT(alpha[:], wv_[:], invr[:], op=ALU.mult)
                    nc.scalar.mul(alpha[:], alpha[:], 0.5)
                    psn = ps3.tile([128, 256], F32, tag="ps3")
                    for kb in range(2):
                        MM(psn[0:1, 0:1], deg[:, kb:kb + 1], cst["ones128"][:],
                           start=(kb == 0), stop=(kb == 1))
                    nrm = scr.tile([1, 1], F32, tag="nrm")
                    nc.scalar.activation(nrm[:], psn[0:1, 0:1], AF.Sqrt)
                    rn = scr.tile([1, 1], F32, tag="rn")
                    nc.vector.reciprocal(rn[:], nrm[:])
                    pbc = ps3.tile([128, 256], F32, tag="ps3")
                    MM(pbc[:, 0:1], cst["onesrow"][:], rn[:], start=True, stop=True)
                    rnb = scr.tile([128, 1], F32, tag="rnb")
                    CP(rnb[:], pbc[:, 0:1])
                    for kb in range(2):
                        TS(U0g[:, m, kb], sd[:, kb:kb + 1], rnb[:, 0:1], None,
                           op0=ALU.mult)
                    arow = rowp.tile([1, N], F32, tag="arow")
                    brow = rowp.tile([1, N], F32, tag="brow")
                    for (row_t, col_t) in ((arow, alpha), (brow, wv_)):
                        ptr2 = ps3.tile([128, 256], F32, tag="ps3")
                        for kb in range(2):
                            MM(ptr2[0:1, 128 * kb:128 * (kb + 1)],
                               col_t[:, kb:kb + 1], cst["i128"][:],
                               start=True, stop=True)
                        CP(row_t[:], ptr2[0:1, :])
                    ptr3 = ps3.tile([128, 256], F32, tag="ps3")
                    for kb in range(2):
                        MM(ptr3[32 * rr_:32 * rr_ + 1, 128 * kb:128 * (kb + 1)],
                           U0g[:, m, kb], cst["i128"][:],
                           start=True, stop=True, tile_position=(0, 32 * rr_))
                    CP(U0r[32 * rr_:32 * rr_ + 1, g4], ptr3[32 * rr_:32 * rr_ + 1, :])
                    for rb in range(2):
                        pso = ps1.tile([128, 512], F32, tag="ps1")
                        MM(pso[:, 0:N], arow[:, 128 * rb:128 * (rb + 1)],
                           brow[:], start=True, stop=False)
                        MM(pso[:, 0:N], brow[:, 128 * rb:128 * (rb + 1)],
                           arow[:], start=False, stop=True)
                        go = scr.tile([128, N], F32, tag="ge")
                        TT(go[:], Sg[:, m, rb], pso[:, 0:N], op=ALU.mult)
                        STT(Sg[:, m, rb], go[:], -CHEB_S, cst["dmask"][:, rb],
                            op0=ALU.mult, op1=ALU.bypass)
                        STT(Sg[:, m, rb], cst["dmask"][:, rb],
                            CHEB_S * (1.0 - C_CENT), Sg[:, m, rb],
                            op0=ALU.mult, op1=ALU.add)

                if nc._dbg and li == 0 and g == 0:
                    nc.sync.dma_start(nc._dbg["d_S"][:], Sg[:])
                    nc.sync.dma_start(nc._dbg["d_V"][:], Vg[:])
                    nc.sync.dma_start(nc._dbg["d_u0"][:], U0g[:])
                _eigensolve_and_attend(
                    nc, tc, cst, smc, dram, hT, Sg, Vg, U0g, U0r,
                    wproj, bproj, epool, spool, scr, ps1, ps2, ps3,
                    TT, TS, STT, CP, MM, r32, li, g, bidx)

            if nc._dbg and li == 0:
                nc.sync.dma_start(nc._dbg["d_att"][:], hT[:])
            _mlp_block(nc, tc, cst, dram, hT, lns, lnb, b1t, b2t,
                       wqrt, spool, scr, ps1, ps2, ps3,
                       TT, TS, STT, CP, MM, r32, li)

            if tap_layer is not None and li == tap_layer:
                nc.sync.dma_start(tap[:], hT[:])

        # ============ pool + final LN + head ============
        lns2 = wpool.tile([128, FC, 1], F32, tag="lns")
        nc.sync.dma_start(lns2[:], dram["lns2"][:].rearrange("f p o -> p f o"))
        lnb2 = wpool.tile([128, FC, 1], F32, tag="lnb")
        nc.sync.dma_start(lnb2[:], dram["lnb2"][:].rearrange("f p o -> p f o"))
        whead = wpool.tile([128, FC, NCLS], F32, tag="whead")
        nc.sync.dma_start(whead[:], dram["whead"][:].rearrange("f p c -> p f c"))
        bhead = wpool.tile([1, NCLS], F32, tag="bhead")
        nc.sync.dma_start(bhead[:], dram["bhead"][:])

        for b in range(BL):
            pooled = scr.tile([128, FC], F32, tag="pooled")
            for ch in range(FC):
                nc.vector.reduce_sum(pooled[:, ch:ch + 1], hT[:, ch, b],
                                     axis=AX.X)
            nc.scalar.mul(pooled[:], pooled[:], 1.0 / N)
            psa = ps3.tile([128, 256], F32, tag="ps3")
            for ch in range(FC):
                MM(psa[0:1, 0:1], pooled[:, ch:ch + 1], cst["ones128"][:],
                   start=(ch == 0), stop=(ch == FC - 1))
            sq = scr.tile([128, FC], F32, tag="poolsq")
            nc.scalar.activation(sq[:], pooled[:], AF.Square)
            psb = ps3.tile([128, 256], F32, tag="ps3")
            for ch in range(FC):
                MM(psb[0:1, 0:1], sq[:, ch:ch + 1], cst["ones128"][:],
                   start=(ch == 0), stop=(ch == FC - 1))
            mean = scr.tile([1, 1], F32, tag="fmean")
            nc.scalar.mul(mean[:], psa[0:1, 0:1], 1.0 / C)
            msq = scr.tile([1, 1], F32, tag="fmsq")
            nc.scalar.mul(msq[:], psb[0:1, 0:1], 1.0 / C)
            m2 = scr.tile([1, 1], F32, tag="fm2")
            nc.scalar.activation(m2[:], mean[:], AF.Square)
            var = scr.tile([1, 1], F32, tag="fvar")
            TT(var[:], msq[:], m2[:], op=ALU.subtract)
            TS(var[:], var[:], 1e-5, None, op0=ALU.add)
            sdv = scr.tile([1, 1], F32, tag="fsdv")
            nc.scalar.activation(sdv[:], var[:], AF.Sqrt)
            rstd = scr.tile([1, 1], F32, tag="frstd")
            nc.vector.reciprocal(rstd[:], sdv[:])
            two = scr.tile([1, 2], F32, tag="ftwo")
            CP(two[:, 0:1], mean[:])
            CP(two[:, 1:2], rstd[:])
            psc = ps3.tile([128, 256], F32, tag="ps3")
            MM(psc[:, 0:2], cst["onesrow"][:], two[:], start=True, stop=True)
            meanb = scr.tile([128, 1], F32, tag="fmeanb")
            rstdb = scr.tile([128, 1], F32, tag="frstdb")
            CP(meanb[:], psc[:, 0:1])
            CP(rstdb[:], psc[:, 1:2])
            pnorm = scr.tile([128, FC], F32, tag="pnorm")
            TS(pnorm[:], pooled[:], meanb[:, 0:1], None, op0=ALU.subtract)
            TS(pnorm[:], pnorm[:], rstdb[:, 0:1], None, op0=ALU.mult)
            for ch in range(FC):
                TS(pnorm[:, ch:ch + 1], pnorm[:, ch:ch + 1],
                   lns2[:, ch], lnb2[:, ch], op0=ALU.mult, op1=ALU.add)
            psh = ps3.tile([128, 256], F32, tag="ps3")
            for ch in range(FC):
                MM(psh[0:1, 0:NCLS], pnorm[:, ch:ch + 1], whead[:, ch],
                   start=(ch == 0), stop=(ch == FC - 1))
            ologit = scr.tile([1, NCLS], F32, tag="ologit")
            TT(ologit[:], psh[0:1, 0:NCLS], bhead[:], op=ALU.add)
            nc.sync.dma_start(out[b:b + 1, :], ologit[:])


def _eigensolve_and_attend(nc, tc, cst, smc, dram, hT, Sg, Vg, U0g, U0r,
                           wproj, bproj, epool, spool, scr, ps1, ps2, ps3,
                           TT, TS, STT, CP, MM, r32, li, g, bidx):
    Xg = epool.tile([128, GM, 2, K], F32, tag="Xg")
    for m in range(GM):
        for kb in range(2):
            CP(Xg[:, m, kb], cst["x0c"][:, kb])
    Xt = epool.tile([128, NG4, N], F32, tag="Xt")

    def cheb_round():
        for m in range(GM):
            Tp = epool.tile([128, 2, K], F32, tag="Tp")
            Tc = epool.tile([128, 2, K], F32, tag="Tc")
            for kb in range(2):
                CP(Tp[:, kb], Xg[:, m, kb])
            psx = ps2.tile([128, 512], F32, tag="ps2")
            for ob in range(2):
                for kb in range(2):
                    MM(psx[:, K * ob:K * (ob + 1)],
                       Sg[:, m, kb, 128 * ob:128 * (ob + 1)],
                       Tp[:, kb], start=(kb == 0), stop=(kb == 1))
            for kb in range(2):
                nc.scalar.mul(Tc[:, kb], psx[:, K * kb:K * (kb + 1)], 0.5)
            for _ in range(D_CHEB - 1):
                psy = ps2.tile([128, 512], F32, tag="ps2")
                for ob in range(2):
                    for kb in range(2):
                        MM(psy[:, K * ob:K * (ob + 1)],
                           Sg[:, m, kb, 128 * ob:128 * (ob + 1)],
                           Tc[:, kb], start=(kb == 0), stop=False)
                    MM(psy[:, K * ob:K * (ob + 1)], cst["ni128"][:], Tp[:, ob],
                       start=False, stop=True)
                for kb in range(2):
                    CP(Tp[:, kb], Tc[:, kb])
                    CP(Tc[:, kb], psy[:, K * kb:K * (kb + 1)])
            for kb in range(2):
                CP(Xg[:, m, kb], Tc[:, kb])

    def deflate():
        for m in range(GM):
            g4, r = m // 4, m % 4
            psp = ps3.tile([128, 256], F32, tag="ps3")
            for kb in range(2):
                MM(psp[32 * r:32 * r + 1, 0:K], U0g[:, m, kb], Xg[:, m, kb],
                   start=(kb == 0), stop=(kb == 1), tile_position=(0, 32 * r))
            pr = scr.tile([128, K], F32, tag="pr")
            nc.scalar.mul(pr[32 * r:32 * r + 1, :], psp[32 * r:32 * r + 1, 0:K], -1.0)
            psd = ps2.tile([128, 512], F32, tag="ps2")
            for kb in range(2):
                MM(psd[:, K * kb:K * (kb + 1)], cst["i128"][:], Xg[:, m, kb],
                   start=True, stop=False)
                MM(psd[:, K * kb:K * (kb + 1)],
                   U0r[32 * r:32 * r + 1, g4, 128 * kb:128 * (kb + 1)],
                   pr[32 * r:32 * r + 1, :],
                   start=False, stop=True, tile_position=(32 * r, 0))
            for kb in range(2):
                CP(Xg[:, m, kb], psd[:, K * kb:K * (kb + 1)])

    def small_mm(dst, lhsT, rhs):
        psb = ps2.tile([128, 512], F32, tag="ps2")
        pv = psb[:].rearrange("p (a b) -> p a b", a=4)
        for g4 in range(NG4):
            MM(pv[:, g4], lhsT[:, g4], rhs[:, g4], start=True, stop=True)
        CP(dst[:], pv[:])

    def build_xt():
        for m in range(GM):
            g4, r = m // 4, m % 4
            ptx = ps2.tile([128, 512], F32, tag="ps2")
            for kb in range(2):
                MM(ptx[32 * r:32 * r + K, 128 * kb:128 * (kb + 1)],
                   Xg[:, m, kb], cst["i128"][:],
                   start=True, stop=True, tile_position=(0, 32 * r))
            CP(Xt[32 * r:32 * r + K, g4], ptx[32 * r:32 * r + K, 0:256])

    def bcast_permat(v44):
        ptq = ps3.tile([128, 256], F32, tag="ps3")
        MM(ptq[0:4, 0:4], v44[:], cst["i128"][0:4, 0:4], start=True, stop=True)
        v44t = scr.tile([4, 4], F32, tag="v44t")
        CP(v44t[:], ptq[0:4, 0:4])
        ptw = ps3.tile([128, 256], F32, tag="ps3")
        MM(ptw[:, 0:4], cst["exp32"][:], v44t[:], start=True, stop=True)
        ob = scr.tile([128, 4], F32, tag="permat")
        CP(ob[:], ptw[:, 0:4])
        return ob

    def permat_max(src):
        ptm = ps3.tile([128, 256], F32, tag="ps3")
        MM(ptm[0:4, 0:128], src[:], cst["i128"][:], start=True, stop=True)
        tr = scr.tile([4, 128], F32, tag="tr44")
        CP(tr[:], ptm[0:4, 0:128])
        mx = scr.tile([4, 4], F32, tag="mx44")
        nc.vector.reduce_max(mx[:], tr[:].rearrange("p (a b) -> p a b", a=4),
                             axis=AX.X)
        return mx

    def whiten_pass(eps, steps):
        Gt = epool.tile([128, 4, 128], F32, tag="Gt")
        psg = ps2.tile([128, 512], F32, tag="ps2")
        pvg = psg[:].rearrange("p (a b) -> p a b", a=4)
        for m in range(GM):
            g4, r = m // 4, m % 4
            for kb in range(2):
                MM(psg[32 * r:32 * r + K,
                       128 * g4 + 32 * r:128 * g4 + 32 * r + K],
                   Xg[:, m, kb], Xg[:, m, kb],
                   start=(kb == 0), stop=(kb == 1), tile_position=(0, 32 * r))
        TT(Gt[:], pvg[:], smc("sm_blk"), op=ALU.mult)
        dg = scr.tile([128, 4, 128], F32, tag="smA")
        TT(dg[:], Gt[:], smc("sm_eye"), op=ALU.mult)
        dgt = scr.tile([128, 4], F32, tag="dgt")
        nc.vector.reduce_sum(dgt[:], dg[:], axis=AX.X)
        TS(dgt[:], dgt[:], 1e-12, None, op0=ALU.add)
        sq = scr.tile([128, 4], F32, tag="sq")
        nc.scalar.activation(sq[:], dgt[:], AF.Sqrt)
        srec = scr.tile([128, 4], F32, tag="srec")
        nc.vector.reciprocal(srec[:], sq[:])
        for g4 in range(NG4):
            TS(Gt[:, g4], Gt[:, g4], srec[:, g4:g4 + 1], None, op0=ALU.mult)
        GtT = epool.tile([128, 4, 128], F32, tag="GtT")
        small_mm(GtT, Gt, smc("sm_eye"))
        for g4 in range(NG4):
            TS(GtT[:, g4], GtT[:, g4], srec[:, g4:g4 + 1], None, op0=ALU.mult)
        STT(Gt[:], smc("sm_eye"), eps, GtT[:], op0=ALU.mult, op1=ALU.add)
        brs = scr.tile([128, 4], F32, tag="brs4")
        nc.vector.tensor_reduce(brs[:], Gt[:], axis=AX.X, op=ALU.add,
                                apply_absolute_value=True)
        mx = permat_max(brs)
        rec = scr.tile([4, 4], F32, tag="rec44")
        nc.vector.reciprocal(rec[:], mx[:])
        bre = bcast_permat(rec)
        for g4 in range(NG4):
            TS(Gt[:, g4], Gt[:, g4], bre[:, g4:g4 + 1], None, op0=ALU.mult)
        Yt = epool.tile([128, 4, 128], F32, tag="Yt")
        Zt = epool.tile([128, 4, 128], F32, tag="Zt")
        Tt = epool.tile([128, 4, 128], F32, tag="Tt")
        CP(Yt[:], Gt[:])
        CP(Zt[:], smc("sm_eye"))
        for _ in range(steps):
            psb = ps2.tile([128, 512], F32, tag="ps2")
            pv2 = psb[:].rearrange("p (a b) -> p a b", a=4)
            for g4 in range(NG4):
                MM(pv2[:, g4], Zt[:, g4], Yt[:, g4], start=True, stop=True)
            STT(Tt[:], pv2[:], -0.5, smc("sm_eye15"), op0=ALU.mult, op1=ALU.add)
            small_mm(Yt, Yt, Tt)
            small_mm(Zt, Tt, Zt)
        sqb = scr.tile([4, 4], F32, tag="sqb44")
        nc.scalar.activation(sqb[:], mx[:], AF.Sqrt)
        nc.vector.reciprocal(sqb[:], sqb[:])
        sbe = bcast_permat(sqb)
        for g4 in range(NG4):
            TS(Zt[:, g4], Zt[:, g4], srec[:, g4:g4 + 1], None, op0=ALU.mult)
            TS(Zt[:, g4], Zt[:, g4], sbe[:, g4:g4 + 1], None, op0=ALU.mult)
        build_xt()
        for m in range(GM):
            g4, r = m // 4, m % 4
            psx = ps2.tile([128, 512], F32, tag="ps2")
            for kb in range(2):
                MM(psx[:, K * kb:K * (kb + 1)],
                   Xt[32 * r:32 * r + K, g4, 128 * kb:128 * (kb + 1)],
                   Zt[32 * r:32 * r + K, g4, 32 * r:32 * r + K],
                   start=True, stop=True, tile_position=(32 * r, 0))
            for kb in range(2):
                CP(Xg[:, m, kb], psx[:, K * kb:K * (kb + 1)])

    # ---------------- rounds ----------------
    for rr in range(ROUNDS[li]):
        cheb_round()
        deflate()
        pass  # barrier removed
        whiten_pass(*NS_MID)
        pass  # barrier removed
        for m in range(GM):
            for kb in range(2):
                TT(Xg[:, m, kb], Xg[:, m, kb], cst["r0c"][:, kb], op=ALU.add)
    deflate()
    pass  # barrier removed
    whiten_pass(*NS_F1)
    pass  # barrier removed
    whiten_pass(*NS_F2)
    pass  # barrier removed
    if nc._dbg and li == 0 and g == 0:
        nc.sync.dma_start(nc._dbg["d_X"][:], Xg[:])

    # ---------------- B build ----------------
    Bt = epool.tile([128, 4, 128], F32, tag="Bt")
    Wt = epool.tile([128, 4, 128], F32, tag="Wt")
    psgB = ps2.tile([128, 512], F32, tag="ps2")
    pvB = psgB[:].rearrange("p (a b) -> p a b", a=4)
    for m in range(GM):
        g4, r = m // 4, m % 4
        psz = ps1.tile([128, 512], F32, tag="ps1")
        for ob in range(2):
            for kb in range(2):
                MM(psz[:, K * ob:K * (ob + 1)],
                   Sg[:, m, kb, 128 * ob:128 * (ob + 1)],
                   Xg[:, m, kb], start=(kb == 0), stop=(kb == 1))
        Zc = epool.tile([128, 2, K], F32, tag="Zc")
        for kb in range(2):
            CP(Zc[:, kb], psz[:, K * kb:K * (kb + 1)])
        for kb in range(2):
            MM(psgB[32 * r:32 * r + K,
                    128 * g4 + 32 * r:128 * g4 + 32 * r + K],
               Xg[:, m, kb], Zc[:, kb],
               start=(kb == 0), stop=(kb == 1), tile_position=(0, 32 * r))
    TT(Bt[:], pvB[:], smc("sm_blk"), op=ALU.mult)
    BtT = epool.tile([128, 4, 128], F32, tag="GtT")
    small_mm(BtT, Bt, smc("sm_eye"))
    TT(Bt[:], Bt[:], BtT[:], op=ALU.add)
    nc.scalar.mul(Bt[:], Bt[:], 0.5)
    CP(Wt[:], smc("sm_eye"))
    if nc._dbg and li == 0 and g == 0:
        nc.sync.dma_start(nc._dbg["d_B"][:], Bt[:].rearrange("p a b -> p (a b)"))
    build_xt()

    # ---------------- all-pairs diagonalizer ----------------
    Et = epool.tile([128, 4, 128], F32, tag="Et")
    Qt = epool.tile([128, 4, 128], F32, tag="Qt")
    QtT = epool.tile([128, 4, 128], F32, tag="QtT")
    M1 = epool.tile([128, 4, 128], F32, tag="M1")
    for it in range(DIAG_ITERS):
        dmat = scr.tile([128, 4, 128], F32, tag="smA")
        TT(dmat[:], Bt[:], smc("sm_eye"), op=ALU.mult)
        dcol = scr.tile([128, 4], F32, tag="dcol")
        nc.vector.reduce_sum(dcol[:], dmat[:], axis=AX.X)
        Drow = epool.tile([128, 4, 128], F32, tag="GtT")
        small_mm(Drow, smc("sm_blk"), dmat)
        dd = scr.tile([128, 4, 128], F32, tag="smB")
        for g4 in range(NG4):
            STT(dd[:, g4], Drow[:, g4], dcol[:, g4:g4 + 1],
                smc("sm_blk")[:, g4], op0=ALU.subtract, op1=ALU.mult)
        sgn = scr.tile([128, 4, 128], F32, tag="smC")
        TS(sgn[:], dd[:], 0.0, None, op0=ALU.is_ge)
        STT(dd[:], sgn[:], 2e-9, dd[:], op0=ALU.mult, op1=ALU.add)
        TS(dd[:], dd[:], -1e-9, None, op0=ALU.add)
        nc.vector.reciprocal(dd[:], dd[:])
        TT(Et[:], Bt[:], dd[:], op=ALU.mult)
        TT(Et[:], Et[:], smc("sm_offblk"), op=ALU.mult)
        TS(Et[:], Et[:], DIAG_CAP, None, op0=ALU.min)
        TS(Et[:], Et[:], -DIAG_CAP, None, op0=ALU.max)
        EtT = scr.tile([128, 4, 128], F32, tag="smA")
        small_mm(EtT, Et, smc("sm_eye"))
        TT(Et[:], Et[:], EtT[:], op=ALU.subtract)
        nc.scalar.mul(Et[:], Et[:], 0.5)
        ern = scr.tile([128, 4], F32, tag="ern4")
        nc.vector.tensor_reduce(ern[:], Et[:], axis=AX.X, op=ALU.add,
                                apply_absolute_value=True)
        emx = permat_max(ern)
        TS(emx[:], emx[:], 1e-9, None, op0=ALU.add)
        esc = scr.tile([4, 4], F32, tag="esc44")
        nc.vector.reciprocal(esc[:], emx[:])
        TS(esc[:], esc[:], DIAG_DAMP, 1.0, op0=ALU.mult, op1=ALU.min)
        ebe = bcast_permat(esc)
        for g4 in range(NG4):
            TS(Et[:, g4], Et[:, g4], ebe[:, g4:g4 + 1], None, op0=ALU.mult)
        TT(Qt[:], smc("sm_eye"), Et[:], op=ALU.add)
        TT(QtT[:], smc("sm_eye"), Et[:], op=ALU.subtract)
        for _ in range(2):
            P_ = scr.tile([128, 4, 128], F32, tag="smA")
            small_mm(P_, Qt, Qt)
            STT(M1[:], P_[:], -0.5, smc("sm_eye15"), op0=ALU.mult, op1=ALU.add)
            Qn = scr.tile([128, 4, 128], F32, tag="smB")
            small_mm(Qn, QtT, M1)
            small_mm(QtT, M1, QtT)
            CP(Qt[:], Qn[:])
        BQ = scr.tile([128, 4, 128], F32, tag="smA")
        small_mm(BQ, Bt, Qt)
        small_mm(Bt, Qt, BQ)
        small_mm(Wt, Qt, Wt)

    # ---------------- rank selection -> Ut ----------------
    dmat = scr.tile([128, 4, 128], F32, tag="smA")
    TT(dmat[:], Bt[:], smc("sm_eye"), op=ALU.mult)
    dcol = scr.tile([128, 4], F32, tag="dcol")
    nc.vector.reduce_sum(dcol[:], dmat[:], axis=AX.X)
    TS(dcol[:], dcol[:], cst["sm_tie"][:, 0:1], None, op0=ALU.add)
    dmat2 = scr.tile([128, 4, 128], F32, tag="smB")
    for g4 in range(NG4):
        TS(dmat2[:, g4], smc("sm_eye")[:, g4], dcol[:, g4:g4 + 1], None,
           op0=ALU.mult)
    Drow = epool.tile([128, 4, 128], F32, tag="GtT")
    small_mm(Drow, smc("sm_blk"), dmat2)
    Cc = scr.tile([128, 4, 128], F32, tag="smC")
    for g4 in range(NG4):
        STT(Cc[:, g4], Drow[:, g4], dcol[:, g4:g4 + 1],
            smc("sm_blk")[:, g4], op0=ALU.is_lt, op1=ALU.mult)
    rnk = scr.tile([128, 4], F32, tag="rnk")
    nc.vector.reduce_sum(rnk[:], Cc[:], axis=AX.X)
    Sel = scr.tile([128, 4, 128], F32, tag="smC")
    for g4 in range(NG4):
        TS(Sel[:, g4], smc("sm_iotasel")[:, g4], rnk[:, g4:g4 + 1], None,
           op0=ALU.is_equal)
    Pt = epool.tile([128, 4, 128], F32, tag="M1")
    small_mm(Pt, Wt, Sel)

    Utg = epool.tile([128, NG4, N], F32, tag="Utg")
    for g4 in range(NG4):
        psu = ps1.tile([128, 512], F32, tag="ps1")
        for r in range(4):
            MM(psu[32 * r:32 * r + N_EIGS, 0:N],
               Pt[32 * r:32 * r + K, g4, 32 * r:32 * r + N_EIGS],
               Xt[32 * r:32 * r + K, g4],
               start=True, stop=True, tile_position=(32 * r, 32 * r))
        CP(Utg[:, g4], psu[:, 0:N])

    un = scr.tile([128, NG4], F32, tag="un")
    for g4 in range(NG4):
        usq = scr.tile([128, N], F32, tag="ge")
        nc.scalar.activation(usq[:], Utg[:, g4], AF.Square)
        nc.vector.reduce_sum(un[:, g4:g4 + 1], usq[:], axis=AX.X)
    TS(un[:], un[:], 1e-30, None, op0=ALU.add)
    uns = scr.tile([128, NG4], F32, tag="uns")
    nc.scalar.activation(uns[:], un[:], AF.Sqrt)
    nc.vector.reciprocal(uns[:], uns[:])
    for g4 in range(NG4):
        TS(Utg[:, g4], Utg[:, g4], uns[:, g4:g4 + 1], None, op0=ALU.mult)

    if nc._dbg and li == 0 and g == 0:
        nc.sync.dma_start(nc._dbg["d_Ut"][:], Utg[:])
    # ---------------- group attention + out + proj ----------------
    aoutT = [spool.tile([CH, N], F32, tag="aoutT" + str(i), name="aoutT" + str(i)) for i in range(2)]
    for m in range(GM):
        g4, r = m // 4, m % 4
        bloc = m // H
        h = m % H
        psga = ps1.tile([128, 512], F32, tag="ps1")
        MM(psga[0:32, 0:N], cst["combs4"][32 * r:32 * r + N_EIGS, :],
           Utg[32 * r:32 * r + N_EIGS, g4],
           start=True, stop=True, tile_position=(32 * r, 0))
        gmx = scr.tile([32, 1], F32, tag="gmx")
        nc.vector.reduce_max(gmx[:], psga[0:32, 0:N], axis=AX.X)
        ngm = scr.tile([32, 1], F32, tag="ngm")
        nc.scalar.mul(ngm[:], gmx[:], -1.0)
        ga = scr.tile([32, N], F32, tag="ga")
        nc.scalar.activation(ga[:], psga[0:32, 0:N], AF.Exp, bias=ngm[:, 0:1])
        gs = scr.tile([32, 1], F32, tag="gs")
        nc.vector.reduce_sum(gs[:], ga[:], axis=AX.X)
        gr = scr.tile([32, 1], F32, tag="gr")
        nc.vector.reciprocal(gr[:], gs[:])
        TS(ga[:], ga[:], gr[:, 0:1], None, op0=ALU.mult)
        gaT = scr.tile([128, 2, 32], F32, tag="gaT")
        psgt = ps3.tile([128, 256], F32, tag="ps3")
        for kb in range(2):
            MM(psgt[:, 32 * kb:32 * (kb + 1)], ga[:, 128 * kb:128 * (kb + 1)],
               cst["i128"][0:32, 0:32], start=True, stop=True)
            CP(gaT[:, kb], psgt[:, 32 * kb:32 * (kb + 1)])
        psoh = ps3.tile([128, 256], F32, tag="ps3")
        for kb in range(2):
            MM(psoh[0:CH, 0:32], Vg[:, m, kb], gaT[:, kb],
               start=(kb == 0), stop=(kb == 1))
        CP(aoutT[bloc][:, 32 * h:32 * (h + 1)], psoh[0:CH, 0:32])

    for bloc in range(2):
        b = bidx[bloc]
        for ch in range(FC):
            psj = ps1.tile([128, 512], F32, tag="ps1")
            MM(psj[:, 0:N], r32(wproj[:, 128 * ch:128 * (ch + 1)]),
               r32(aoutT[bloc][:]), start=True, stop=True)
            TS(hT[:, ch, b], psj[:, 0:N], bproj[:, ch], None, op0=ALU.add)


def _mlp_block(nc, tc, cst, dram, hT, lns, lnb, b1t, b2t,
               wqrt, spool, scr, ps1, ps2, ps3, TT, TS, STT, CP, MM, r32, li):
    for pair in range(2):
        bb = [2 * pair, 2 * pair + 1]
        hnp = spool.tile([128, FC, 2, N], F32, tag="Sg")
        for bloc, b in enumerate(bb):
            pss = ps3.tile([128, 256], F32, tag="ps3")
            ps2s = ps3.tile([128, 256], F32, tag="ps3")
            for ch in range(FC):
                MM(pss[0:1, 0:128], cst["ones128"][:], hT[:, ch, b, 0:128],
                   start=(ch == 0), stop=(ch == FC - 1))
            # NOTE: sums are computed in two half-token blocks (psum free 128)
            for ch in range(FC):
                MM(ps2s[0:1, 0:128], cst["ones128"][:], hT[:, ch, b, 128:256],
                   start=(ch == 0), stop=(ch == FC - 1))
            sums = scr.tile([1, N], F32, tag="lsums")
            CP(sums[:, 0:128], pss[0:1, 0:128])
            CP(sums[:, 128:256], ps2s[0:1, 0:128])
            sq1 = scr.tile([128, N], F32, tag="lt1")
            psq1 = ps3.tile([128, 256], F32, tag="ps3")
            psq2 = ps3.tile([128, 256], F32, tag="ps3")
            for ch in range(FC):
                nc.scalar.activation(sq1[:], hT[:, ch, b], AF.Square)
                MM(psq1[0:1, 0:128], cst["ones128"][:], sq1[:, 0:128],
                   start=(ch == 0), stop=(ch == FC - 1))
                MM(psq2[0:1, 0:128], cst["ones128"][:], sq1[:, 128:256],
                   start=(ch == 0), stop=(ch == FC - 1))
            sqs = scr.tile([1, N], F32, tag="lsqs")
            CP(sqs[:, 0:128], psq1[0:1, 0:128])
            CP(sqs[:, 128:256], psq2[0:1, 0:128])
            mean = scr.tile([1, N], F32, tag="lmean")
            nc.scalar.mul(mean[:], sums[:], 1.0 / C)
            msq = scr.tile([1, N], F32, tag="lmsq")
            nc.scalar.mul(msq[:], sqs[:], 1.0 / C)
            m2 = scr.tile([1, N], F32, tag="lm2")
            nc.scalar.activation(m2[:], mean[:], AF.Square)
            var = scr.tile([1, N], F32, tag="lvar")
            TT(var[:], msq[:], m2[:], op=ALU.subtract)
            TS(var[:], var[:], 1e-5, None, op0=ALU.add)
            sdv = scr.tile([1, N], F32, tag="lsdv")
            nc.scalar.activation(sdv[:], var[:], AF.Sqrt)
            rst = scr.tile([1, N], F32, tag="lrst")
            nc.vector.reciprocal(rst[:], sdv[:])
            mrs = scr.tile([1, N], F32, tag="lmrs")
            TT(mrs[:], mean[:], rst[:], op=ALU.mult)
            psb1 = ps1.tile([128, 512], F32, tag="ps1")
            MM(psb1[:, 0:N], cst["onesrow"][:], rst[:], start=True, stop=True)
            MM(psb1[:, 256:512], cst["onesrow"][:], mrs[:], start=True, stop=True)
            rstB = scr.tile([128, N], F32, tag="lrstB")
            mrsB = scr.tile([128, N], F32, tag="lmrsB")
            CP(rstB[:], psb1[:, 0:N])
            CP(mrsB[:], psb1[:, 256:512])
            for ch in range(FC):
                t1 = scr.tile([128, N], F32, tag="lt1")
                TT(t1[:], hT[:, ch, b], rstB[:], op=ALU.mult)
                TT(t1[:], t1[:], mrsB[:], op=ALU.subtract)
                TS(hnp[:, ch, bloc], t1[:], lns[:, ch], lnb[:, ch],
                   op0=ALU.mult, op1=ALU.add)

        # MLP quarters: a1q = gelu(psum-accum) ; mlp2 accumulates into hT
        for q in range(4):
            w1q = wqrt.tile([128, FC, 768], F32, tag="wqrt")
            nc.sync.dma_start(
                w1q[:], dram["w1"][li, :, :, 768 * q:768 * (q + 1)]
                .rearrange("f p c -> p f c"))
            a1q = spool.tile([128, 6, 2, N], F32, tag="Vg")
            for jc in range(6):
                jg = 6 * q + jc
                psm = ps1.tile([128, 512], F32, tag="ps1")
                for ch in range(FC):
                    MM(psm[:],
                       r32(w1q[:, ch, 128 * jc:128 * (jc + 1)]),
                       r32(hnp[:, ch].rearrange("p a b -> p (a b)")),
                       start=(ch == 0), stop=(ch == FC - 1))
                nc.scalar.activation(
                    a1q[:, jc].rearrange("p a b -> p (a b)"), psm[:],
                    AF.Gelu, bias=b1t[:, jg])
            w2q = wqrt.tile([128, FC, 768], F32, tag="wqrt")
            nc.sync.dma_start(
                w2q[:], dram["w2"][li, 6 * q:6 * (q + 1)]
                .rearrange("j p c -> p j c"))
            for mc in range(FC):
                psm2 = ps1.tile([128, 512], F32, tag="ps1")
                for jc in range(6):
                    MM(psm2[:],
                       r32(w2q[:, jc, 128 * mc:128 * (mc + 1)]),
                       r32(a1q[:, jc].rearrange("p a b -> p (a b)")),
                       start=(jc == 0), stop=(jc == 5))
                hv = hT[:, mc, 2 * pair:2 * pair + 2, :].rearrange("p a b -> p (a b)")
                if q == 0:
                    STT(hv, psm2[:], b2t[:, mc], hv, op0=ALU.add, op1=ALU.add)
                else:
                    TT(hv, hv, psm2[:], op=ALU.add)
            # residual base already in hT (attn out); mlp adds on top


FC_A1 = 6


# ====================== host wrapper ======================

_NC_CACHE = {}


def _get_nc(n_layers=D, tap_layer=None):
    key = (n_layers, tap_layer)
    if key not in _NC_CACHE:
        _NC_CACHE[key] = build_nc(n_layers, tap_layer)
    return _NC_CACHE[key]


def kernel(**inputs):
    x = np.asarray(inputs["x"], dtype=np.float32)
    w = _prep_weights(inputs)
    xp = _prep_x(x)                      # [B, 12, 256]
    nc = _get_nc()
    in_maps = []
    for c in range(NCORES):
        im = {}
        im["xpt"] = np.ascontiguousarray(xp[BL * c:BL * (c + 1)])
        for kname, arr in w.items():
            im[kname] = arr
        for cn, arr in CONSTS.items():
            im[cn] = arr
        in_maps.append(im)
    res = run_bass_kernel_spmd(nc, in_maps, core_ids=list(range(NCORES)))
    outs = [res.results[c]["out"] for c in range(NCORES)]
    return np.concatenate(outs, axis=0).astype(np.float32)


if __name__ == "__main__":
    rng = np.random.RandomState(0)
    print("building 1-layer nc for smoke...")
    import time
    t0 = time.time()
    nc = build_nc(n_layers=1, tap_layer=0)
    print("build+compile took", time.time() - t0)



# revision 19
# speedup vs baseline: 2.2343x; 1.2297x over previous
"""Trainium2 Bass kernel for nn_CIFARViT (spectral group-attention ViT).

kernel(**inputs) takes the FULL unsharded inputs (keys as in setup_inputs),
shards the batch over 8 NeuronCores (pure data parallel, 4 images/core),
runs one Bass program per core via run_bass_kernel_spmd, and gathers the
full [32, 10] output.
"""
import sys
import os

if '/opt/trn_rl_repo' not in sys.path:
    sys.path.insert(0, '/opt/trn_rl_repo')
os.environ.setdefault("NEURON_RT_RESET_CORES", "1")

import numpy as np

import concourse.bass as bass
import concourse.mybir as mybir
from concourse import bacc
from concourse.tile import TileContext
from concourse.bass_utils import run_bass_kernel_spmd

F32 = mybir.dt.float32
F32R = mybir.dt.float32r
BF16 = mybir.dt.bfloat16
AF = mybir.ActivationFunctionType
ALU = mybir.AluOpType
AX = mybir.AxisListType

D = 8
C = 768
H = 8
B = 32
BL = 4
NCORES = 8
N = 256
CH = 96
N_EIGS = 5
NCLS = 10
FC = C // 128
SCALE_QK = CH ** -0.5

K = 16
ROUNDS = [3, 3, 1, 1, 1, 1, 1, 1]
if os.environ.get("KERNEL_R1"):
    ROUNDS = [int(os.environ["KERNEL_R1"])] * 8
D_CHEB = 2
A_INT, B_INT = 0.95, 1.35
C_CENT = (B_INT + A_INT) / 2.0
CHEB_S = 4.0 / (B_INT - A_INT)
ETA = 1e-2
NS_MID = (3e-3, 4)
NS_F1 = (1e-3, 6)
NS_F2 = (1e-5, 3)
DIAG_ITERS = 5
DIAG_CAP = 0.45
DIAG_DAMP = 0.4
TIE_D = 1e-5
GM = 16
NG4 = 4

_SEED = 1234


def _host_consts():
    rng = np.random.RandomState(_SEED)
    X0 = rng.randn(N, K).astype(np.float32)
    R0 = (rng.randn(N, K).astype(np.float32) / np.sqrt(N)) * ETA
    cs = {}
    cs["x0c"] = np.ascontiguousarray(X0.reshape(2, 128, K).transpose(1, 0, 2))
    cs["r0c"] = np.ascontiguousarray(R0.reshape(2, 128, K).transpose(1, 0, 2))
    cs["i128"] = np.eye(128, dtype=np.float32)
    cs["ni128"] = -np.eye(128, dtype=np.float32)
    cs["onesrow"] = np.ones((1, 128), dtype=np.float32)
    cs["ones128"] = np.ones((128, 1), dtype=np.float32)
    dm = np.zeros((2, 128, 256), dtype=np.float32)
    for r in range(2):
        for p in range(128):
            dm[r, p, 128 * r + p] = 1.0
    cs["dmask"] = np.ascontiguousarray(dm.transpose(1, 0, 2))
    cs["vmask"] = np.ascontiguousarray(1.0 - dm.transpose(1, 0, 2))
    import itertools
    combs = np.array(list(itertools.product([1.0, -1.0], repeat=N_EIGS)),
                     dtype=np.float32)
    c4 = np.zeros((128, 32), dtype=np.float32)
    for r in range(4):
        c4[32 * r:32 * r + N_EIGS, :] = combs.T
    cs["combs4"] = c4
    eye = np.zeros((128, 512), dtype=np.float32)
    blk = np.zeros((128, 512), dtype=np.float32)
    iot = np.full((128, 512), 1e9, dtype=np.float32)
    for g4 in range(4):
        for r in range(4):
            for i in range(K):
                eye[32 * r + i, 128 * g4 + 32 * r + i] = 1.0
                blk[32 * r + i, 128 * g4 + 32 * r:128 * g4 + 32 * r + K] = 1.0
                for j in range(N_EIGS):
                    iot[32 * r + i, 128 * g4 + 32 * r + j] = float(j)
    cs["sm_eye"] = eye
    cs["sm_eye15"] = 1.5 * eye
    cs["sm_blk"] = blk
    cs["sm_offblk"] = blk - eye
    cs["sm_iotasel"] = iot
    cs["sm_tie"] = (np.arange(128, dtype=np.float32) % 32 * TIE_D).reshape(128, 1)
    e32 = np.zeros((4, 128), dtype=np.float32)
    for r in range(4):
        e32[r, 32 * r:32 * (r + 1)] = 1.0
    cs["exp32"] = e32
    return cs


CONSTS = _host_consts()


def _prep_weights(inputs):
    import ml_dtypes
    bf16 = ml_dtypes.bfloat16
    w = {}
    qkv_w = np.asarray(inputs["qkv_w"], dtype=np.float32)
    w["wq"] = np.ascontiguousarray(qkv_w[:, :, 0:C]).reshape(D, FC, 128, C).astype(bf16)
    w["wv"] = np.ascontiguousarray(qkv_w[:, :, 2 * C:3 * C]).reshape(D, FC, 128, C).astype(bf16)
    w["wproj"] = np.ascontiguousarray(np.asarray(inputs["proj_w"], dtype=np.float32)).astype(bf16)
    w["bproj"] = np.asarray(inputs["proj_b"], dtype=np.float32).reshape(D, FC, 128, 1).copy()
    w["lns"] = np.asarray(inputs["g_ln_s"], dtype=np.float32).reshape(D, FC, 128, 1).copy()
    w["lnb"] = np.asarray(inputs["g_ln_b"], dtype=np.float32).reshape(D, FC, 128, 1).copy()
    w["w1"] = np.ascontiguousarray(np.asarray(inputs["mlp_w1"], dtype=np.float32)).reshape(D, FC, 128, 4 * C).astype(bf16)
    w["b1"] = np.asarray(inputs["mlp_b1"], dtype=np.float32).reshape(D, 24, 128, 1).copy()
    w["w2"] = np.ascontiguousarray(np.asarray(inputs["mlp_w2"], dtype=np.float32)).reshape(D, 24, 128, C).astype(bf16)
    w["b2"] = np.asarray(inputs["mlp_b2"], dtype=np.float32).reshape(D, FC, 128, 1).copy()
    w["lns2"] = np.asarray(inputs["ln_s"], dtype=np.float32).reshape(FC, 128, 1).copy()
    w["lnb2"] = np.asarray(inputs["ln_b"], dtype=np.float32).reshape(FC, 128, 1).copy()
    w["whead"] = np.asarray(inputs["head_w"], dtype=np.float32).reshape(FC, 128, NCLS).copy()
    w["bhead"] = np.asarray(inputs["head_b"], dtype=np.float32).reshape(1, NCLS).copy()
    pw = np.asarray(inputs["patch_w"], dtype=np.float32).reshape(C, 12)
    w["pwT"] = np.ascontiguousarray(pw.T)
    w["pbias"] = np.asarray(inputs["patch_b"], dtype=np.float32).reshape(FC, 128, 1).copy()
    pos = np.asarray(inputs["pos_emb"], dtype=np.float32).reshape(N, C)
    w["posT"] = np.ascontiguousarray(pos.T).reshape(FC, 128, N)
    return w


def _prep_x(x):
    Bb = x.shape[0]
    xp = np.asarray(x, dtype=np.float32).reshape(Bb, 3, 16, 2, 16, 2)
    xp = xp.transpose(0, 2, 4, 1, 3, 5).reshape(Bb, N, 12)
    return np.ascontiguousarray(np.swapaxes(xp, 1, 2))


# ====================== device program ======================

def build_nc(n_layers=D, tap_layer=None):
    nc = bacc.Bacc("TRN2", target_bir_lowering=False, debug=False)
    dram = {}

    def din(name, shape, dt=F32):
        dram[name] = nc.dram_tensor(name, list(shape), dt, kind="ExternalInput")

    din("xpt", (BL, 12, N))
    din("pwT", (12, C))
    din("pbias", (FC, 128, 1))
    din("posT", (FC, 128, N))
    din("wq", (D, FC, 128, C), BF16)
    din("wv", (D, FC, 128, C), BF16)
    din("wproj", (D, CH, C), BF16)
    din("bproj", (D, FC, 128, 1))
    din("lns", (D, FC, 128, 1))
    din("lnb", (D, FC, 128, 1))
    din("w1", (D, FC, 128, 4 * C), BF16)
    din("b1", (D, 24, 128, 1))
    din("w2", (D, 24, 128, C), BF16)
    din("b2", (D, FC, 128, 1))
    din("lns2", (FC, 128, 1))
    din("lnb2", (FC, 128, 1))
    din("whead", (FC, 128, NCLS))
    din("bhead", (1, NCLS))
    for cn, arr in CONSTS.items():
        din(cn, arr.shape)

    out = nc.dram_tensor("out", [BL, NCLS], F32, kind="ExternalOutput")
    tap = None
    if tap_layer is not None:
        tap = nc.dram_tensor("tap", [128, FC, BL, N], F32, kind="ExternalOutput")
    if os.environ.get("KERNEL_DEBUG_TAPS"):
        nc._dbg = {
            "d_emb": nc.dram_tensor("d_emb", [128, FC, BL, N], F32, kind="ExternalOutput"),
            "d_S": nc.dram_tensor("d_S", [128, GM, 2, N], F32, kind="ExternalOutput"),
            "d_V": nc.dram_tensor("d_V", [128, GM, 2, CH], F32, kind="ExternalOutput"),
            "d_u0": nc.dram_tensor("d_u0", [128, GM, 2, 1], F32, kind="ExternalOutput"),
            "d_X": nc.dram_tensor("d_X", [128, GM, 2, K], F32, kind="ExternalOutput"),
            "d_B": nc.dram_tensor("d_B", [128, 512], F32, kind="ExternalOutput"),
            "d_Ut": nc.dram_tensor("d_Ut", [128, NG4, N], F32, kind="ExternalOutput"),
            "d_att": nc.dram_tensor("d_att", [128, FC, BL, N], F32, kind="ExternalOutput"),
        }
    else:
        nc._dbg = {}

    with TileContext(nc) as tc:
        _emit(nc, tc, dram, out, tap, n_layers, tap_layer)
    nc.compile()
    return nc


def _emit(nc, tc, dram, out, tap, n_layers, tap_layer):
    import contextlib
    es = contextlib.ExitStack()
    with es:
        persist = es.enter_context(tc.tile_pool(name="persist", bufs=1))
        wpool = es.enter_context(tc.tile_pool(name="wpool", bufs=1))
        wstr = es.enter_context(tc.tile_pool(name="wstr", bufs=2))
        wqrt = es.enter_context(tc.tile_pool(name="wqrt", bufs=1))
        spool = es.enter_context(tc.tile_pool(name="spool", bufs=1))
        epool = es.enter_context(tc.tile_pool(name="epool", bufs=1))
        scr = es.enter_context(tc.tile_pool(name="scr", bufs=1))
        rowp = es.enter_context(tc.tile_pool(name="rowp", bufs=1))
        ps1 = es.enter_context(tc.tile_pool(name="ps1", bufs=2, space="PSUM"))
        ps2 = es.enter_context(tc.tile_pool(name="ps2", bufs=2, space="PSUM"))
        ps3 = es.enter_context(tc.tile_pool(name="ps3", bufs=2, space="PSUM"))

        cst = {}
        for cn, arr in CONSTS.items():
            t = persist.tile(list(arr.shape), F32, tag="c_" + cn)
            cst[cn] = t
            nc.sync.dma_start(t[:], dram[cn][:])

        def smc(name):
            return cst[name][:].rearrange("p (a b) -> p a b", a=4)

        hT = persist.tile([128, FC, BL, N], F32, tag="hT")

        def r32(ap):
            return ap

        TT = nc.vector.tensor_tensor
        TS = nc.vector.tensor_scalar
        STT = nc.vector.scalar_tensor_tensor
        CP = nc.vector.tensor_copy
        MM = nc.tensor.matmul

        # ============ patch embed ============
        xpt = persist.tile([12, BL, N], F32, tag="xpt")
        nc.sync.dma_start(xpt[:], dram["xpt"][:].rearrange("b k n -> k b n"))
        pwT = persist.tile([12, C], F32, tag="pwT")
        nc.sync.dma_start(pwT[:], dram["pwT"][:])
        pbias = persist.tile([128, FC, 1], F32, tag="pbias")
        nc.sync.dma_start(pbias[:], dram["pbias"][:].rearrange("f p o -> p f o"))
        posT = persist.tile([128, FC, N], F32, tag="posT")
        nc.sync.dma_start(posT[:], dram["posT"][:].rearrange("f p n -> p f n"))

        for b in range(BL):
            for ch in range(FC):
                ps = ps1.tile([128, 512], F32, tag="ps1")
                MM(ps[:, 0:N], r32(pwT[:, 128 * ch:128 * (ch + 1)]),
                   r32(xpt[:, b]), start=True, stop=True)
                tmp = scr.tile([128, N], F32, tag="ge")
                TS(tmp[:], ps[:, 0:N], pbias[:, ch], None, op0=ALU.add)
                TT(hT[:, ch, b], tmp[:], posT[:, ch], op=ALU.add)

        if nc._dbg:
            nc.sync.dma_start(nc._dbg["d_emb"][:], hT[:])
        # ===================== layers =====================
        for li in range(n_layers):
            wq = wpool.tile([128, FC, C], BF16, tag="wq")
            nc.sync.dma_start(wq[:], dram["wq"][li].rearrange("f p c -> p f c"))
            wproj = wpool.tile([CH, C], BF16, tag="wproj")
            nc.sync.dma_start(wproj[:], dram["wproj"][li])
            hTb = wpool.tile([128, FC, BL, N], BF16, tag="hTb")
            for b_ in range(BL):
                for ch_ in range(FC):
                    CP(hTb[:, ch_, b_], hT[:, ch_, b_])
            bproj = wpool.tile([128, FC, 1], F32, tag="bproj")
            nc.sync.dma_start(bproj[:], dram["bproj"][li].rearrange("f p o -> p f o"))
            lns = wpool.tile([128, FC, 1], F32, tag="lns")
            nc.sync.dma_start(lns[:], dram["lns"][li].rearrange("f p o -> p f o"))
            lnb = wpool.tile([128, FC, 1], F32, tag="lnb")
            nc.sync.dma_start(lnb[:], dram["lnb"][li].rearrange("f p o -> p f o"))
            b1t = wpool.tile([128, 24, 1], F32, tag="b1t")
            nc.sync.dma_start(b1t[:], dram["b1"][li].rearrange("j p o -> p j o"))
            b2t = wpool.tile([128, FC, 1], F32, tag="b2t")
            nc.sync.dma_start(b2t[:], dram["b2"][li].rearrange("f p o -> p f o"))

            for g in range(2):
                bidx = [2 * g, 2 * g + 1]
                Sg = spool.tile([128, GM, 2, N], F32, tag="Sg")
                Vg = spool.tile([128, GM, 2, CH], F32, tag="Vg")
                U0g = spool.tile([128, GM, 2, 1], F32, tag="U0g")
                U0r = spool.tile([128, NG4, N], F32, tag="U0r")

                # ---- v = hT^T Wv per matrix (wv streamed, SBUF-accumulate) --
                Vgb = spool.tile([128, GM, 2, CH], BF16, tag="Vgb")
                for ch in range(FC):
                    wvc = wstr.tile([128, C], BF16, tag="wvc")
                    nc.sync.dma_start(wvc[:], dram["wv"][li, ch])
                    for m in range(GM):
                        b = bidx[m // H]
                        h = m % H
                        for rb in range(2):
                            psv = ps3.tile([128, 256], F32, tag="ps3")
                            MM(psv[:, 0:CH],
                               r32(hTb[:, ch, b, 128 * rb:128 * (rb + 1)]),
                               r32(wvc[:, CH * h:CH * (h + 1)]),
                               start=True, stop=True)
                            if ch == 0:
                                CP(Vg[:, m, rb], psv[:, 0:CH])
                            else:
                                TT(Vg[:, m, rb], Vg[:, m, rb], psv[:, 0:CH],
                                   op=ALU.add)
                for m in range(GM):
                    CP(Vgb[:, m], Vg[:, m])

                # ---- phase A (exp table): logits -> G into Sg ----
                for m in range(GM):
                    b = bidx[m // H]
                    h = m % H
                    qT = scr.tile([CH, N], BF16, tag="qT")
                    psq = ps1.tile([128, 512], F32, tag="ps1")
                    for ch in range(FC):
                        MM(psq[0:CH, 0:N],
                           r32(wq[:, ch, CH * h:CH * (h + 1)]),
                           r32(hTb[:, ch, b]),
                           start=(ch == 0), stop=(ch == FC - 1))
                    CP(qT[:], psq[0:CH, 0:N])
                    psl = ps2.tile([128, 512], F32, tag="ps2")
                    for rb in range(2):
                        MM(psl[:, 256 * rb:256 * (rb + 1)],
                           r32(qT[:, 128 * rb:128 * (rb + 1)]),
                           r32(qT[:]), start=True, stop=True)
                    rm = scr.tile([128, 1], F32, tag="rm")
                    nc.vector.reduce_max(rm[:], psl[:], axis=AX.X)
                    pst = ps3.tile([128, 256], F32, tag="ps3")
                    MM(pst[0:1, 0:128], rm[:], cst["i128"][:], start=True, stop=True)
                    mv = scr.tile([1, 1], F32, tag="mv")
                    nc.vector.reduce_max(mv[:], pst[0:1, 0:128], axis=AX.X)
                    pbc = ps3.tile([128, 256], F32, tag="ps3")
                    MM(pbc[:, 0:1], cst["onesrow"][:], mv[:], start=True, stop=True)
                    negm = scr.tile([128, 1], F32, tag="negm")
                    nc.scalar.mul(negm[:], pbc[:, 0:1], -SCALE_QK)
                    for rb in range(2):
                        ge = scr.tile([128, N], F32, tag="ge")
                        nc.scalar.activation(ge[:], psl[:, 256 * rb:256 * (rb + 1)],
                                             AF.Exp, bias=negm[:, 0:1],
                                             scale=SCALE_QK)
                        TT(Sg[:, m, rb], ge[:], cst["vmask"][:, rb], op=ALU.mult)

                # ---- phase B (sqrt table): G -> Ahat, u0 ----
                for m in range(GM):
                    g4, rr_ = m // 4, m % 4
                    rsum = scr.tile([128, 2], F32, tag="rsum")
                    for rb in range(2):
                        nc.vector.reduce_sum(rsum[:, rb:rb + 1], Sg[:, m, rb],
                                             axis=AX.X)
                    invr = scr.tile([128, 2], F32, tag="invr")
                    nc.vector.reciprocal(invr[:], rsum[:])
                    ptv = ps3.tile([128, 256], F32, tag="ps3")
                    for ob in range(2):
                        for kb in range(2):
                            MM(ptv[:, ob:ob + 1],
                               Sg[:, m, kb, 128 * ob:128 * (ob + 1)],
                               invr[:, kb:kb + 1],
                               start=(kb == 0), stop=(kb == 1))
                    deg = scr.tile([128, 2], F32, tag="deg")
                    TS(deg[:], ptv[:, 0:2], 0.5, 0.5, op0=ALU.mult, op1=ALU.add)
                    sd = scr.tile([128, 2], F32, tag="sd")
                    nc.scalar.activation(sd[:], deg[:], AF.Sqrt)
                    wv_ = scr.tile([128, 2], F32, tag="wv_")
                    nc.vector.reciprocal(wv_[:], sd[:])
                    alpha = scr.tile([128, 2], F32, tag="alpha")
                    TT(alpha[:], wv_[:], invr[:], op=ALU.mult)
                    nc.scalar.mul(alpha[:], alpha[:], 0.5)
                    psn = ps3.tile([128, 256], F32, tag="ps3")
                    for kb in range(2):
                        MM(psn[0:1, 0:1], deg[:, kb:kb + 1], cst["ones128"][:],
                           start=(kb == 0), stop=(kb == 1))
                    nrm = scr.tile([1, 1], F32, tag="nrm")
                    nc.scalar.activation(nrm[:], psn[0:1, 0:1], AF.Sqrt)
                    rn = scr.tile([1, 1], F32, tag="rn")
                    nc.vector.reciprocal(rn[:], nrm[:])
                    pbc = ps3.tile([128, 256], F32, tag="ps3")
                    MM(pbc[:, 0:1], cst["onesrow"][:], rn[:], start=True, stop=True)
                    rnb = scr.tile([128, 1], F32, tag="rnb")
                    CP(rnb[:], pbc[:, 0:1])
                    for kb in range(2):
                        TS(U0g[:, m, kb], sd[:, kb:kb + 1], rnb[:, 0:1], None,
                           op0=ALU.mult)
                    arow = rowp.tile([1, N], F32, tag="arow")
                    brow = rowp.tile([1, N], F32, tag="brow")
                    for (row_t, col_t) in ((arow, alpha), (brow, wv_)):
                        ptr2 = ps3.tile([128, 256], F32, tag="ps3")
                        for kb in range(2):
                            MM(ptr2[0:1, 128 * kb:128 * (kb + 1)],
                               col_t[:, kb:kb + 1], cst["i128"][:],
                               start=True, stop=True)
                        CP(row_t[:], ptr2[0:1, :])
                    ptr3 = ps3.tile([128, 256], F32, tag="ps3")
                    for kb in range(2):
                        MM(ptr3[32 * rr_:32 * rr_ + 1, 128 * kb:128 * (kb + 1)],
                           U0g[:, m, kb], cst["i128"][:],
                           start=True, stop=True, tile_position=(0, 32 * rr_))
                    CP(U0r[32 * rr_:32 * rr_ + 1, g4], ptr3[32 * rr_:32 * rr_ + 1, :])
                    for rb in range(2):
                        pso = ps1.tile([128, 512], F32, tag="ps1")
                        MM(pso[:, 0:N], arow[:, 128 * rb:128 * (rb + 1)],
                           brow[:], start=True, stop=False)
                        MM(pso[:, 0:N], brow[:, 128 * rb:128 * (rb + 1)],
                           arow[:], start=False, stop=True)
                        go = scr.tile([128, N], F32, tag="ge")
                        TT(go[:], Sg[:, m, rb], pso[:, 0:N], op=ALU.mult)
                        STT(Sg[:, m, rb], go[:], -CHEB_S, cst["dmask"][:, rb],
                            op0=ALU.mult, op1=ALU.bypass)
                        STT(Sg[:, m, rb], cst["dmask"][:, rb],
                            CHEB_S * (1.0 - C_CENT), Sg[:, m, rb],
                            op0=ALU.mult, op1=ALU.add)

                if nc._dbg and li == 0 and g == 0:
                    nc.sync.dma_start(nc._dbg["d_S"][:], Sg[:])
                    nc.sync.dma_start(nc._dbg["d_V"][:], Vg[:])
                    nc.sync.dma_start(nc._dbg["d_u0"][:], U0g[:])
                _eigensolve_and_attend(
                    nc, tc, cst, smc, dram, hT, Sg, Vgb, U0g, U0r,
                    wproj, bproj, epool, spool, scr, ps1, ps2, ps3,
                    TT, TS, STT, CP, MM, r32, li, g, bidx)

            if nc._dbg and li == 0:
                nc.sync.dma_start(nc._dbg["d_att"][:], hT[:])
            _mlp_block(nc, tc, cst, dram, hT, lns, lnb, b1t, b2t,
                       wqrt, spool, scr, ps1, ps2, ps3,
                       TT, TS, STT, CP, MM, r32, li)

            if tap_layer is not None and li == tap_layer:
                nc.sync.dma_start(tap[:], hT[:])

        # ============ pool + final LN + head ============
        lns2 = wpool.tile([128, FC, 1], F32, tag="lns")
        nc.sync.dma_start(lns2[:], dram["lns2"][:].rearrange("f p o -> p f o"))
        lnb2 = wpool.tile([128, FC, 1], F32, tag="lnb")
        nc.sync.dma_start(lnb2[:], dram["lnb2"][:].rearrange("f p o -> p f o"))
        whead = wpool.tile([128, FC, NCLS], F32, tag="whead")
        nc.sync.dma_start(whead[:], dram["whead"][:].rearrange("f p c -> p f c"))
        bhead = wpool.tile([1, NCLS], F32, tag="bhead")
        nc.sync.dma_start(bhead[:], dram["bhead"][:])

        for b in range(BL):
            pooled = scr.tile([128, FC], F32, tag="pooled")
            for ch in range(FC):
                nc.vector.reduce_sum(pooled[:, ch:ch + 1], hT[:, ch, b],
                                     axis=AX.X)
            nc.scalar.mul(pooled[:], pooled[:], 1.0 / N)
            psa = ps3.tile([128, 256], F32, tag="ps3")
            for ch in range(FC):
                MM(psa[0:1, 0:1], pooled[:, ch:ch + 1], cst["ones128"][:],
                   start=(ch == 0), stop=(ch == FC - 1))
            sq = scr.tile([128, FC], F32, tag="poolsq")
            nc.scalar.activation(sq[:], pooled[:], AF.Square)
            psb = ps3.tile([128, 256], F32, tag="ps3")
            for ch in range(FC):
                MM(psb[0:1, 0:1], sq[:, ch:ch + 1], cst["ones128"][:],
                   start=(ch == 0), stop=(ch == FC - 1))
            mean = scr.tile([1, 1], F32, tag="fmean")
            nc.scalar.mul(mean[:], psa[0:1, 0:1], 1.0 / C)
            msq = scr.tile([1, 1], F32, tag="fmsq")
            nc.scalar.mul(msq[:], psb[0:1, 0:1], 1.0 / C)
            m2 = scr.tile([1, 1], F32, tag="fm2")
            nc.scalar.activation(m2[:], mean[:], AF.Square)
            var = scr.tile([1, 1], F32, tag="fvar")
            TT(var[:], msq[:], m2[:], op=ALU.subtract)
            TS(var[:], var[:], 1e-5, None, op0=ALU.add)
            sdv = scr.tile([1, 1], F32, tag="fsdv")
            nc.scalar.activation(sdv[:], var[:], AF.Sqrt)
            rstd = scr.tile([1, 1], F32, tag="frstd")
            nc.vector.reciprocal(rstd[:], sdv[:])
            two = scr.tile([1, 2], F32, tag="ftwo")
            CP(two[:, 0:1], mean[:])
            CP(two[:, 1:2], rstd[:])
            psc = ps3.tile([128, 256], F32, tag="ps3")
            MM(psc[:, 0:2], cst["onesrow"][:], two[:], start=True, stop=True)
            meanb = scr.tile([128, 1], F32, tag="fmeanb")
            rstdb = scr.tile([128, 1], F32, tag="frstdb")
            CP(meanb[:], psc[:, 0:1])
            CP(rstdb[:], psc[:, 1:2])
            pnorm = scr.tile([128, FC], F32, tag="pnorm")
            TS(pnorm[:], pooled[:], meanb[:, 0:1], None, op0=ALU.subtract)
            TS(pnorm[:], pnorm[:], rstdb[:, 0:1], None, op0=ALU.mult)
            for ch in range(FC):
                TS(pnorm[:, ch:ch + 1], pnorm[:, ch:ch + 1],
                   lns2[:, ch], lnb2[:, ch], op0=ALU.mult, op1=ALU.add)
            psh = ps3.tile([128, 256], F32, tag="ps3")
            for ch in range(FC):
                MM(psh[0:1, 0:NCLS], pnorm[:, ch:ch + 1], whead[:, ch],
                   start=(ch == 0), stop=(ch == FC - 1))
            ologit = scr.tile([1, NCLS], F32, tag="ologit")
            TT(ologit[:], psh[0:1, 0:NCLS], bhead[:], op=ALU.add)
            nc.sync.dma_start(out[b:b + 1, :], ologit[:])


def _eigensolve_and_attend(nc, tc, cst, smc, dram, hT, Sg, Vg, U0g, U0r,
                           wproj, bproj, epool, spool, scr, ps1, ps2, ps3,
                           TT, TS, STT, CP, MM, r32, li, g, bidx):
    Xg = epool.tile([128, GM, 2, K], F32, tag="Xg")
    for m in range(GM):
        for kb in range(2):
            CP(Xg[:, m, kb], cst["x0c"][:, kb])
    Xt = epool.tile([128, NG4, N], F32, tag="Xt")

    def cheb_round():
        for m in range(GM):
            Tp = epool.tile([128, 2, K], F32, tag="Tp")
            Tc = epool.tile([128, 2, K], F32, tag="Tc")
            for kb in range(2):
                CP(Tp[:, kb], Xg[:, m, kb])
            psx = ps2.tile([128, 512], F32, tag="ps2")
            for ob in range(2):
                for kb in range(2):
                    MM(psx[:, K * ob:K * (ob + 1)],
                       Sg[:, m, kb, 128 * ob:128 * (ob + 1)],
                       Tp[:, kb], start=(kb == 0), stop=(kb == 1))
            for kb in range(2):
                nc.scalar.mul(Tc[:, kb], psx[:, K * kb:K * (kb + 1)], 0.5)
            for _ in range(D_CHEB - 1):
                psy = ps2.tile([128, 512], F32, tag="ps2")
                for ob in range(2):
                    for kb in range(2):
                        MM(psy[:, K * ob:K * (ob + 1)],
                           Sg[:, m, kb, 128 * ob:128 * (ob + 1)],
                           Tc[:, kb], start=(kb == 0), stop=False)
                    MM(psy[:, K * ob:K * (ob + 1)], cst["ni128"][:], Tp[:, ob],
                       start=False, stop=True)
                for kb in range(2):
                    CP(Tp[:, kb], Tc[:, kb])
                    CP(Tc[:, kb], psy[:, K * kb:K * (kb + 1)])
            for kb in range(2):
                CP(Xg[:, m, kb], Tc[:, kb])

    def deflate():
        for m in range(GM):
            g4, r = m // 4, m % 4
            psp = ps3.tile([128, 256], F32, tag="ps3")
            for kb in range(2):
                MM(psp[32 * r:32 * r + 1, 0:K], U0g[:, m, kb], Xg[:, m, kb],
                   start=(kb == 0), stop=(kb == 1), tile_position=(0, 32 * r))
            pr = scr.tile([128, K], F32, tag="pr")
            nc.scalar.mul(pr[32 * r:32 * r + 1, :], psp[32 * r:32 * r + 1, 0:K], -1.0)
            psd = ps2.tile([128, 512], F32, tag="ps2")
            for kb in range(2):
                MM(psd[:, K * kb:K * (kb + 1)], cst["i128"][:], Xg[:, m, kb],
                   start=True, stop=False)
                MM(psd[:, K * kb:K * (kb + 1)],
                   U0r[32 * r:32 * r + 1, g4, 128 * kb:128 * (kb + 1)],
                   pr[32 * r:32 * r + 1, :],
                   start=False, stop=True, tile_position=(32 * r, 0))
            for kb in range(2):
                CP(Xg[:, m, kb], psd[:, K * kb:K * (kb + 1)])

    def small_mm(dst, lhsT, rhs):
        psb = ps2.tile([128, 512], F32, tag="ps2")
        pv = psb[:].rearrange("p (a b) -> p a b", a=4)
        for g4 in range(NG4):
            MM(pv[:, g4], lhsT[:, g4], rhs[:, g4], start=True, stop=True)
        CP(dst[:], pv[:])

    def build_xt():
        for m in range(GM):
            g4, r = m // 4, m % 4
            ptx = ps2.tile([128, 512], F32, tag="ps2")
            for kb in range(2):
                MM(ptx[32 * r:32 * r + K, 128 * kb:128 * (kb + 1)],
                   Xg[:, m, kb], cst["i128"][:],
                   start=True, stop=True, tile_position=(0, 32 * r))
            CP(Xt[32 * r:32 * r + K, g4], ptx[32 * r:32 * r + K, 0:256])

    def bcast_permat(v44):
        ptq = ps3.tile([128, 256], F32, tag="ps3")
        MM(ptq[0:4, 0:4], v44[:], cst["i128"][0:4, 0:4], start=True, stop=True)
        v44t = scr.tile([4, 4], F32, tag="v44t")
        CP(v44t[:], ptq[0:4, 0:4])
        ptw = ps3.tile([128, 256], F32, tag="ps3")
        MM(ptw[:, 0:4], cst["exp32"][:], v44t[:], start=True, stop=True)
        ob = scr.tile([128, 4], F32, tag="permat")
        CP(ob[:], ptw[:, 0:4])
        return ob

    def permat_max(src):
        ptm = ps3.tile([128, 256], F32, tag="ps3")
        MM(ptm[0:4, 0:128], src[:], cst["i128"][:], start=True, stop=True)
        tr = scr.tile([4, 128], F32, tag="tr44")
        CP(tr[:], ptm[0:4, 0:128])
        mx = scr.tile([4, 4], F32, tag="mx44")
        nc.vector.reduce_max(mx[:], tr[:].rearrange("p (a b) -> p a b", a=4),
                             axis=AX.X)
        return mx

    def whiten_pass(eps, steps):
        Gt = epool.tile([128, 4, 128], F32, tag="Gt")
        psg = ps2.tile([128, 512], F32, tag="ps2")
        pvg = psg[:].rearrange("p (a b) -> p a b", a=4)
        for m in range(GM):
            g4, r = m // 4, m % 4
            for kb in range(2):
                MM(psg[32 * r:32 * r + K,
                       128 * g4 + 32 * r:128 * g4 + 32 * r + K],
                   Xg[:, m, kb], Xg[:, m, kb],
                   start=(kb == 0), stop=(kb == 1), tile_position=(0, 32 * r))
        TT(Gt[:], pvg[:], smc("sm_blk"), op=ALU.mult)
        dg = scr.tile([128, 4, 128], F32, tag="smA")
        TT(dg[:], Gt[:], smc("sm_eye"), op=ALU.mult)
        dgt = scr.tile([128, 4], F32, tag="dgt")
        nc.vector.reduce_sum(dgt[:], dg[:], axis=AX.X)
        TS(dgt[:], dgt[:], 1e-12, None, op0=ALU.add)
        sq = scr.tile([128, 4], F32, tag="sq")
        nc.scalar.activation(sq[:], dgt[:], AF.Sqrt)
        srec = scr.tile([128, 4], F32, tag="srec")
        nc.vector.reciprocal(srec[:], sq[:])
        for g4 in range(NG4):
            TS(Gt[:, g4], Gt[:, g4], srec[:, g4:g4 + 1], None, op0=ALU.mult)
        GtT = epool.tile([128, 4, 128], F32, tag="GtT")
        small_mm(GtT, Gt, smc("sm_eye"))
        for g4 in range(NG4):
            TS(GtT[:, g4], GtT[:, g4], srec[:, g4:g4 + 1], None, op0=ALU.mult)
        STT(Gt[:], smc("sm_eye"), eps, GtT[:], op0=ALU.mult, op1=ALU.add)
        brs = scr.tile([128, 4], F32, tag="brs4")
        nc.vector.tensor_reduce(brs[:], Gt[:], axis=AX.X, op=ALU.add,
                                apply_absolute_value=True)
        mx = permat_max(brs)
        rec = scr.tile([4, 4], F32, tag="rec44")
        nc.vector.reciprocal(rec[:], mx[:])
        bre = bcast_permat(rec)
        for g4 in range(NG4):
            TS(Gt[:, g4], Gt[:, g4], bre[:, g4:g4 + 1], None, op0=ALU.mult)
        Yt = epool.tile([128, 4, 128], F32, tag="Yt")
        Zt = epool.tile([128, 4, 128], F32, tag="Zt")
        Tt = epool.tile([128, 4, 128], F32, tag="Tt")
        CP(Yt[:], Gt[:])
        CP(Zt[:], smc("sm_eye"))
        for _ in range(steps):
            psb = ps2.tile([128, 512], F32, tag="ps2")
            pv2 = psb[:].rearrange("p (a b) -> p a b", a=4)
            for g4 in range(NG4):
                MM(pv2[:, g4], Zt[:, g4], Yt[:, g4], start=True, stop=True)
            STT(Tt[:], pv2[:], -0.5, smc("sm_eye15"), op0=ALU.mult, op1=ALU.add)
            small_mm(Yt, Yt, Tt)
            small_mm(Zt, Tt, Zt)
        sqb = scr.tile([4, 4], F32, tag="sqb44")
        nc.scalar.activation(sqb[:], mx[:], AF.Sqrt)
        nc.vector.reciprocal(sqb[:], sqb[:])
        sbe = bcast_permat(sqb)
        for g4 in range(NG4):
            TS(Zt[:, g4], Zt[:, g4], srec[:, g4:g4 + 1], None, op0=ALU.mult)
            TS(Zt[:, g4], Zt[:, g4], sbe[:, g4:g4 + 1], None, op0=ALU.mult)
        build_xt()
        for m in range(GM):
            g4, r = m // 4, m % 4
            psx = ps2.tile([128, 512], F32, tag="ps2")
            for kb in range(2):
                MM(psx[:, K * kb:K * (kb + 1)],
                   Xt[32 * r:32 * r + K, g4, 128 * kb:128 * (kb + 1)],
                   Zt[32 * r:32 * r + K, g4, 32 * r:32 * r + K],
                   start=True, stop=True, tile_position=(32 * r, 0))
            for kb in range(2):
                CP(Xg[:, m, kb], psx[:, K * kb:K * (kb + 1)])

    # ---------------- rounds ----------------
    for rr in range(ROUNDS[li]):
        cheb_round()
        deflate()
        pass  # barrier removed
        whiten_pass(*NS_MID)
        pass  # barrier removed
        for m in range(GM):
            for kb in range(2):
                TT(Xg[:, m, kb], Xg[:, m, kb], cst["r0c"][:, kb], op=ALU.add)
    deflate()
    pass  # barrier removed
    whiten_pass(*NS_F1)
    pass  # barrier removed
    whiten_pass(*NS_F2)
    pass  # barrier removed
    if nc._dbg and li == 0 and g == 0:
        nc.sync.dma_start(nc._dbg["d_X"][:], Xg[:])

    # ---------------- B build ----------------
    Bt = epool.tile([128, 4, 128], F32, tag="Bt")
    Wt = epool.tile([128, 4, 128], F32, tag="Wt")
    psgB = ps2.tile([128, 512], F32, tag="ps2")
    pvB = psgB[:].rearrange("p (a b) -> p a b", a=4)
    for m in range(GM):
        g4, r = m // 4, m % 4
        psz = ps1.tile([128, 512], F32, tag="ps1")
        for ob in range(2):
            for kb in range(2):
                MM(psz[:, K * ob:K * (ob + 1)],
                   Sg[:, m, kb, 128 * ob:128 * (ob + 1)],
                   Xg[:, m, kb], start=(kb == 0), stop=(kb == 1))
        Zc = epool.tile([128, 2, K], F32, tag="Zc")
        for kb in range(2):
            CP(Zc[:, kb], psz[:, K * kb:K * (kb + 1)])
        for kb in range(2):
            MM(psgB[32 * r:32 * r + K,
                    128 * g4 + 32 * r:128 * g4 + 32 * r + K],
               Xg[:, m, kb], Zc[:, kb],
               start=(kb == 0), stop=(kb == 1), tile_position=(0, 32 * r))
    TT(Bt[:], pvB[:], smc("sm_blk"), op=ALU.mult)
    BtT = epool.tile([128, 4, 128], F32, tag="GtT")
    small_mm(BtT, Bt, smc("sm_eye"))
    TT(Bt[:], Bt[:], BtT[:], op=ALU.add)
    nc.scalar.mul(Bt[:], Bt[:], 0.5)
    CP(Wt[:], smc("sm_eye"))
    if nc._dbg and li == 0 and g == 0:
        nc.sync.dma_start(nc._dbg["d_B"][:], Bt[:].rearrange("p a b -> p (a b)"))
    build_xt()

    # ---------------- all-pairs diagonalizer ----------------
    Et = epool.tile([128, 4, 128], F32, tag="Et")
    Qt = epool.tile([128, 4, 128], F32, tag="Qt")
    QtT = epool.tile([128, 4, 128], F32, tag="QtT")
    M1 = epool.tile([128, 4, 128], F32, tag="M1")
    for it in range(DIAG_ITERS):
        dmat = scr.tile([128, 4, 128], F32, tag="smA")
        TT(dmat[:], Bt[:], smc("sm_eye"), op=ALU.mult)
        dcol = scr.tile([128, 4], F32, tag="dcol")
        nc.vector.reduce_sum(dcol[:], dmat[:], axis=AX.X)
        Drow = epool.tile([128, 4, 128], F32, tag="GtT")
        small_mm(Drow, smc("sm_blk"), dmat)
        dd = scr.tile([128, 4, 128], F32, tag="smB")
        for g4 in range(NG4):
            STT(dd[:, g4], Drow[:, g4], dcol[:, g4:g4 + 1],
                smc("sm_blk")[:, g4], op0=ALU.subtract, op1=ALU.mult)
        sgn = scr.tile([128, 4, 128], F32, tag="smC")
        TS(sgn[:], dd[:], 0.0, None, op0=ALU.is_ge)
        STT(dd[:], sgn[:], 2e-9, dd[:], op0=ALU.mult, op1=ALU.add)
        TS(dd[:], dd[:], -1e-9, None, op0=ALU.add)
        nc.vector.reciprocal(dd[:], dd[:])
        TT(Et[:], Bt[:], dd[:], op=ALU.mult)
        TT(Et[:], Et[:], smc("sm_offblk"), op=ALU.mult)
        TS(Et[:], Et[:], DIAG_CAP, None, op0=ALU.min)
        TS(Et[:], Et[:], -DIAG_CAP, None, op0=ALU.max)
        EtT = scr.tile([128, 4, 128], F32, tag="smA")
        small_mm(EtT, Et, smc("sm_eye"))
        TT(Et[:], Et[:], EtT[:], op=ALU.subtract)
        nc.scalar.mul(Et[:], Et[:], 0.5)
        ern = scr.tile([128, 4], F32, tag="ern4")
        nc.vector.tensor_reduce(ern[:], Et[:], axis=AX.X, op=ALU.add,
                                apply_absolute_value=True)
        emx = permat_max(ern)
        TS(emx[:], emx[:], 1e-9, None, op0=ALU.add)
        esc = scr.tile([4, 4], F32, tag="esc44")
        nc.vector.reciprocal(esc[:], emx[:])
        TS(esc[:], esc[:], DIAG_DAMP, 1.0, op0=ALU.mult, op1=ALU.min)
        ebe = bcast_permat(esc)
        for g4 in range(NG4):
            TS(Et[:, g4], Et[:, g4], ebe[:, g4:g4 + 1], None, op0=ALU.mult)
        TT(Qt[:], smc("sm_eye"), Et[:], op=ALU.add)
        TT(QtT[:], smc("sm_eye"), Et[:], op=ALU.subtract)
        for _ in range(2):
            P_ = scr.tile([128, 4, 128], F32, tag="smA")
            small_mm(P_, Qt, Qt)
            STT(M1[:], P_[:], -0.5, smc("sm_eye15"), op0=ALU.mult, op1=ALU.add)
            Qn = scr.tile([128, 4, 128], F32, tag="smB")
            small_mm(Qn, QtT, M1)
            small_mm(QtT, M1, QtT)
            CP(Qt[:], Qn[:])
        BQ = scr.tile([128, 4, 128], F32, tag="smA")
        small_mm(BQ, Bt, Qt)
        small_mm(Bt, Qt, BQ)
        small_mm(Wt, Qt, Wt)

    # ---------------- rank selection -> Ut ----------------
    dmat = scr.tile([128, 4, 128], F32, tag="smA")
    TT(dmat[:], Bt[:], smc("sm_eye"), op=ALU.mult)
    dcol = scr.tile([128, 4], F32, tag="dcol")
    nc.vector.reduce_sum(dcol[:], dmat[:], axis=AX.X)
    TS(dcol[:], dcol[:], cst["sm_tie"][:, 0:1], None, op0=ALU.add)
    dmat2 = scr.tile([128, 4, 128], F32, tag="smB")
    for g4 in range(NG4):
        TS(dmat2[:, g4], smc("sm_eye")[:, g4], dcol[:, g4:g4 + 1], None,
           op0=ALU.mult)
    Drow = epool.tile([128, 4, 128], F32, tag="GtT")
    small_mm(Drow, smc("sm_blk"), dmat2)
    Cc = scr.tile([128, 4, 128], F32, tag="smC")
    for g4 in range(NG4):
        STT(Cc[:, g4], Drow[:, g4], dcol[:, g4:g4 + 1],
            smc("sm_blk")[:, g4], op0=ALU.is_lt, op1=ALU.mult)
    rnk = scr.tile([128, 4], F32, tag="rnk")
    nc.vector.reduce_sum(rnk[:], Cc[:], axis=AX.X)
    Sel = scr.tile([128, 4, 128], F32, tag="smC")
    for g4 in range(NG4):
        TS(Sel[:, g4], smc("sm_iotasel")[:, g4], rnk[:, g4:g4 + 1], None,
           op0=ALU.is_equal)
    Pt = epool.tile([128, 4, 128], F32, tag="M1")
    small_mm(Pt, Wt, Sel)

    Utg = epool.tile([128, NG4, N], F32, tag="Utg")
    for g4 in range(NG4):
        psu = ps1.tile([128, 512], F32, tag="ps1")
        for r in range(4):
            MM(psu[32 * r:32 * r + N_EIGS, 0:N],
               Pt[32 * r:32 * r + K, g4, 32 * r:32 * r + N_EIGS],
               Xt[32 * r:32 * r + K, g4],
               start=True, stop=True, tile_position=(32 * r, 32 * r))
        CP(Utg[:, g4], psu[:, 0:N])

    un = scr.tile([128, NG4], F32, tag="un")
    for g4 in range(NG4):
        usq = scr.tile([128, N], F32, tag="ge")
        nc.scalar.activation(usq[:], Utg[:, g4], AF.Square)
        nc.vector.reduce_sum(un[:, g4:g4 + 1], usq[:], axis=AX.X)
    TS(un[:], un[:], 1e-30, None, op0=ALU.add)
    uns = scr.tile([128, NG4], F32, tag="uns")
    nc.scalar.activation(uns[:], un[:], AF.Sqrt)
    nc.vector.reciprocal(uns[:], uns[:])
    for g4 in range(NG4):
        TS(Utg[:, g4], Utg[:, g4], uns[:, g4:g4 + 1], None, op0=ALU.mult)

    if nc._dbg and li == 0 and g == 0:
        nc.sync.dma_start(nc._dbg["d_Ut"][:], Utg[:])
    # ---------------- group attention + out + proj ----------------
    aoutT = [spool.tile([CH, N], BF16, tag="aoutT" + str(i), name="aoutT" + str(i)) for i in range(2)]
    for m in range(GM):
        g4, r = m // 4, m % 4
        bloc = m // H
        h = m % H
        psga = ps1.tile([128, 512], F32, tag="ps1")
        MM(psga[0:32, 0:N], cst["combs4"][32 * r:32 * r + N_EIGS, :],
           Utg[32 * r:32 * r + N_EIGS, g4],
           start=True, stop=True, tile_position=(32 * r, 0))
        gmx = scr.tile([32, 1], F32, tag="gmx")
        nc.vector.reduce_max(gmx[:], psga[0:32, 0:N], axis=AX.X)
        ngm = scr.tile([32, 1], F32, tag="ngm")
        nc.scalar.mul(ngm[:], gmx[:], -1.0)
        ga = scr.tile([32, N], F32, tag="ga")
        nc.scalar.activation(ga[:], psga[0:32, 0:N], AF.Exp, bias=ngm[:, 0:1])
        gs = scr.tile([32, 1], F32, tag="gs")
        nc.vector.reduce_sum(gs[:], ga[:], axis=AX.X)
        gr = scr.tile([32, 1], F32, tag="gr")
        nc.vector.reciprocal(gr[:], gs[:])
        TS(ga[:], ga[:], gr[:, 0:1], None, op0=ALU.mult)
        gaT = scr.tile([128, 2, 32], BF16, tag="gaT")
        psgt = ps3.tile([128, 256], F32, tag="ps3")
        for kb in range(2):
            MM(psgt[:, 32 * kb:32 * (kb + 1)], ga[:, 128 * kb:128 * (kb + 1)],
               cst["i128"][0:32, 0:32], start=True, stop=True)
            CP(gaT[:, kb], psgt[:, 32 * kb:32 * (kb + 1)])
        psoh = ps3.tile([128, 256], F32, tag="ps3")
        for kb in range(2):
            MM(psoh[0:CH, 0:32], Vg[:, m, kb], gaT[:, kb],
               start=(kb == 0), stop=(kb == 1))
        CP(aoutT[bloc][:, 32 * h:32 * (h + 1)], psoh[0:CH, 0:32])

    for bloc in range(2):
        b = bidx[bloc]
        for ch in range(FC):
            psj = ps1.tile([128, 512], F32, tag="ps1")
            MM(psj[:, 0:N], r32(wproj[:, 128 * ch:128 * (ch + 1)]),
               r32(aoutT[bloc][:]), start=True, stop=True)
            TS(hT[:, ch, b], psj[:, 0:N], bproj[:, ch], None, op0=ALU.add)


def _mlp_block(nc, tc, cst, dram, hT, lns, lnb, b1t, b2t,
               wqrt, spool, scr, ps1, ps2, ps3, TT, TS, STT, CP, MM, r32, li):
    for pair in range(2):
        bb = [2 * pair, 2 * pair + 1]
        hnp = spool.tile([128, FC, 2, N], BF16, tag="Sg")
        for bloc, b in enumerate(bb):
            pss = ps3.tile([128, 256], F32, tag="ps3")
            ps2s = ps3.tile([128, 256], F32, tag="ps3")
            for ch in range(FC):
                MM(pss[0:1, 0:128], cst["ones128"][:], hT[:, ch, b, 0:128],
                   start=(ch == 0), stop=(ch == FC - 1))
            # NOTE: sums are computed in two half-token blocks (psum free 128)
            for ch in range(FC):
                MM(ps2s[0:1, 0:128], cst["ones128"][:], hT[:, ch, b, 128:256],
                   start=(ch == 0), stop=(ch == FC - 1))
            sums = scr.tile([1, N], F32, tag="lsums")
            CP(sums[:, 0:128], pss[0:1, 0:128])
            CP(sums[:, 128:256], ps2s[0:1, 0:128])
            sq1 = scr.tile([128, N], F32, tag="lt1")
            psq1 = ps3.tile([128, 256], F32, tag="ps3")
            psq2 = ps3.tile([128, 256], F32, tag="ps3")
            for ch in range(FC):
                nc.scalar.activation(sq1[:], hT[:, ch, b], AF.Square)
                MM(psq1[0:1, 0:128], cst["ones128"][:], sq1[:, 0:128],
                   start=(ch == 0), stop=(ch == FC - 1))
                MM(psq2[0:1, 0:128], cst["ones128"][:], sq1[:, 128:256],
                   start=(ch == 0), stop=(ch == FC - 1))
            sqs = scr.tile([1, N], F32, tag="lsqs")
            CP(sqs[:, 0:128], psq1[0:1, 0:128])
            CP(sqs[:, 128:256], psq2[0:1, 0:128])
            mean = scr.tile([1, N], F32, tag="lmean")
            nc.scalar.mul(mean[:], sums[:], 1.0 / C)
            msq = scr.tile([1, N], F32, tag="lmsq")
            nc.scalar.mul(msq[:], sqs[:], 1.0 / C)
            m2 = scr.tile([1, N], F32, tag="lm2")
            nc.scalar.activation(m2[:], mean[:], AF.Square)
            var = scr.tile([1, N], F32, tag="lvar")
            TT(var[:], msq[:], m2[:], op=ALU.subtract)
            TS(var[:], var[:], 1e-5, None, op0=ALU.add)
            sdv = scr.tile([1, N], F32, tag="lsdv")
            nc.scalar.activation(sdv[:], var[:], AF.Sqrt)
            rst = scr.tile([1, N], F32, tag="lrst")
            nc.vector.reciprocal(rst[:], sdv[:])
            mrs = scr.tile([1, N], F32, tag="lmrs")
            TT(mrs[:], mean[:], rst[:], op=ALU.mult)
            psb1 = ps1.tile([128, 512], F32, tag="ps1")
            MM(psb1[:, 0:N], cst["onesrow"][:], rst[:], start=True, stop=True)
            MM(psb1[:, 256:512], cst["onesrow"][:], mrs[:], start=True, stop=True)
            rstB = scr.tile([128, N], F32, tag="lrstB")
            mrsB = scr.tile([128, N], F32, tag="lmrsB")
            CP(rstB[:], psb1[:, 0:N])
            CP(mrsB[:], psb1[:, 256:512])
            for ch in range(FC):
                t1 = scr.tile([128, N], F32, tag="lt1")
                TT(t1[:], hT[:, ch, b], rstB[:], op=ALU.mult)
                TT(t1[:], t1[:], mrsB[:], op=ALU.subtract)
                TS(hnp[:, ch, bloc], t1[:], lns[:, ch], lnb[:, ch],
                   op0=ALU.mult, op1=ALU.add)

        # MLP quarters: a1q = gelu(psum-accum) ; mlp2 accumulates into hT
        for q in range(4):
            w1q = wqrt.tile([128, FC, 768], BF16, tag="wqrt")
            nc.sync.dma_start(
                w1q[:], dram["w1"][li, :, :, 768 * q:768 * (q + 1)]
                .rearrange("f p c -> p f c"))
            a1q = spool.tile([128, 6, 2, N], BF16, tag="Vg")
            for jc in range(6):
                jg = 6 * q + jc
                psm = ps1.tile([128, 512], F32, tag="ps1")
                for ch in range(FC):
                    MM(psm[:],
                       r32(w1q[:, ch, 128 * jc:128 * (jc + 1)]),
                       r32(hnp[:, ch].rearrange("p a b -> p (a b)")),
                       start=(ch == 0), stop=(ch == FC - 1))
                nc.scalar.activation(
                    a1q[:, jc].rearrange("p a b -> p (a b)"), psm[:],
                    AF.Gelu, bias=b1t[:, jg])
            w2q = wqrt.tile([128, FC, 768], BF16, tag="wqrt")
            nc.sync.dma_start(
                w2q[:], dram["w2"][li, 6 * q:6 * (q + 1)]
                .rearrange("j p c -> p j c"))
            for mc in range(FC):
                psm2 = ps1.tile([128, 512], F32, tag="ps1")
                for jc in range(6):
                    MM(psm2[:],
                       r32(w2q[:, jc, 128 * mc:128 * (mc + 1)]),
                       r32(a1q[:, jc].rearrange("p a b -> p (a b)")),
                       start=(jc == 0), stop=(jc == 5))
                hv = hT[:, mc, 2 * pair:2 * pair + 2, :].rearrange("p a b -> p (a b)")
                if q == 0:
                    STT(hv, psm2[:], b2t[:, mc], hv, op0=ALU.add, op1=ALU.add)
                else:
                    TT(hv, hv, psm2[:], op=ALU.add)
            # residual base already in hT (attn out); mlp adds on top


FC_A1 = 6


# ====================== host wrapper ======================

_NC_CACHE = {}


def _get_nc(n_layers=D, tap_layer=None):
    key = (n_layers, tap_layer)
    if key not in _NC_CACHE:
        _NC_CACHE[key] = build_nc(n_layers, tap_layer)
    return _NC_CACHE[key]


def kernel(**inputs):
    x = np.asarray(inputs["x"], dtype=np.float32)
    w = _prep_weights(inputs)
    xp = _prep_x(x)                      # [B, 12, 256]
    nc = _get_nc()
    in_maps = []
    for c in range(NCORES):
        im = {}
        im["xpt"] = np.ascontiguousarray(xp[BL * c:BL * (c + 1)])
        for kname, arr in w.items():
            im[kname] = arr
        for cn, arr in CONSTS.items():
            im[cn] = arr
        in_maps.append(im)
    res = run_bass_kernel_spmd(nc, in_maps, core_ids=list(range(NCORES)))
    outs = [res.results[c]["out"] for c in range(NCORES)]
    return np.concatenate(outs, axis=0).astype(np.float32)


if __name__ == "__main__":
    rng = np.random.RandomState(0)
    print("building 1-layer nc for smoke...")
    import time
    t0 = time.time()
    nc = build_nc(n_layers=1, tap_layer=0)
    print("build+compile took", time.time() - t0)



# revision 36
# speedup vs baseline: 2.4907x; 1.1148x over previous
"""Trainium2 Bass kernel for nn_CIFARViT (spectral group-attention ViT).

kernel(**inputs) takes the FULL unsharded inputs (keys as in setup_inputs),
shards the batch over 8 NeuronCores (pure data parallel, 4 images/core),
runs one Bass program per core via run_bass_kernel_spmd, and gathers the
full [32, 10] output.
"""
import sys
import os

if '/opt/trn_rl_repo' not in sys.path:
    sys.path.insert(0, '/opt/trn_rl_repo')
os.environ.setdefault("NEURON_RT_RESET_CORES", "1")

import numpy as np

import concourse.bass as bass
import concourse.mybir as mybir
from concourse import bacc
from concourse.tile import TileContext
from concourse.bass_utils import run_bass_kernel_spmd

F32 = mybir.dt.float32
F32R = mybir.dt.float32r
BF16 = mybir.dt.bfloat16
AF = mybir.ActivationFunctionType
ALU = mybir.AluOpType
AX = mybir.AxisListType

D = 8
C = 768
H = 8
B = 32
BL = 4
NCORES = 8
N = 256
CH = 96
N_EIGS = 5
NCLS = 10
FC = C // 128
SCALE_QK = CH ** -0.5

K = 16
ROUNDS = [3, 3, 1, 1, 1, 1, 1, 1]
if os.environ.get("KERNEL_R1"):
    ROUNDS = [int(os.environ["KERNEL_R1"])] * 8
D_CHEB = 2
A_INT, B_INT = 0.95, 1.35
C_CENT = (B_INT + A_INT) / 2.0
CHEB_S = 4.0 / (B_INT - A_INT)
ETA = 1e-2
NS_MID = (3e-3, 4)
NS_F1 = (1e-3, 6)
NS_F2 = (1e-5, 3)
DIAG_ITERS = 5
DIAG_CAP = 0.45
DIAG_DAMP = 0.4
TIE_D = 1e-5
GM = 16
NG4 = 4

_SEED = 1234


def _host_consts():
    rng = np.random.RandomState(_SEED)
    X0 = rng.randn(N, K).astype(np.float32)
    R0 = (rng.randn(N, K).astype(np.float32) / np.sqrt(N)) * ETA
    cs = {}
    cs["x0c"] = np.ascontiguousarray(X0.reshape(2, 128, K).transpose(1, 0, 2))
    cs["r0c"] = np.ascontiguousarray(R0.reshape(2, 128, K).transpose(1, 0, 2))
    cs["i128"] = np.eye(128, dtype=np.float32)
    cs["ni128"] = -np.eye(128, dtype=np.float32)
    cs["onesrow"] = np.ones((1, 128), dtype=np.float32)
    cs["ones128"] = np.ones((128, 1), dtype=np.float32)
    dm = np.zeros((2, 128, 256), dtype=np.float32)
    for r in range(2):
        for p in range(128):
            dm[r, p, 128 * r + p] = 1.0
    cs["dmask"] = np.ascontiguousarray(dm.transpose(1, 0, 2))
    cs["vmask"] = np.ascontiguousarray(1.0 - dm.transpose(1, 0, 2))
    import itertools
    combs = np.array(list(itertools.product([1.0, -1.0], repeat=N_EIGS)),
                     dtype=np.float32)
    c4 = np.zeros((128, 32), dtype=np.float32)
    for r in range(4):
        c4[32 * r:32 * r + N_EIGS, :] = combs.T
    cs["combs4"] = c4
    eye = np.zeros((128, 512), dtype=np.float32)
    blk = np.zeros((128, 512), dtype=np.float32)
    iot = np.full((128, 512), 1e9, dtype=np.float32)
    for g4 in range(4):
        for r in range(4):
            for i in range(K):
                eye[32 * r + i, 128 * g4 + 32 * r + i] = 1.0
                blk[32 * r + i, 128 * g4 + 32 * r:128 * g4 + 32 * r + K] = 1.0
                for j in range(N_EIGS):
                    iot[32 * r + i, 128 * g4 + 32 * r + j] = float(j)
    cs["sm_eye"] = eye
    cs["sm_eye15"] = 1.5 * eye
    cs["sm_blk"] = blk
    cs["sm_offblk"] = blk - eye
    cs["sm_iotasel"] = iot
    cs["sm_tie"] = (np.arange(128, dtype=np.float32) % 32 * TIE_D).reshape(128, 1)
    e32 = np.zeros((4, 128), dtype=np.float32)
    for r in range(4):
        e32[r, 32 * r:32 * (r + 1)] = 1.0
    cs["exp32"] = e32
    import ml_dtypes
    for k_ in list(cs):
        cs[k_] = np.ascontiguousarray(cs[k_], dtype=np.float32)
    cs["ni128b"] = (-np.eye(128)).astype(ml_dtypes.bfloat16)
    return cs


CONSTS = _host_consts()


def _prep_weights(inputs):
    import ml_dtypes
    bf16 = ml_dtypes.bfloat16
    w = {}
    qkv_w = np.asarray(inputs["qkv_w"], dtype=np.float32)
    w["wq"] = np.ascontiguousarray(qkv_w[:, :, 0:C]).reshape(D, FC, 128, C).astype(bf16)
    w["wv"] = np.ascontiguousarray(qkv_w[:, :, 2 * C:3 * C]).reshape(D, FC, 128, C).astype(bf16)
    w["wproj"] = np.ascontiguousarray(np.asarray(inputs["proj_w"], dtype=np.float32)).astype(bf16)
    w["bproj"] = np.asarray(inputs["proj_b"], dtype=np.float32).reshape(D, FC, 128, 1).copy()
    w["lns"] = np.asarray(inputs["g_ln_s"], dtype=np.float32).reshape(D, FC, 128, 1).copy()
    w["lnb"] = np.asarray(inputs["g_ln_b"], dtype=np.float32).reshape(D, FC, 128, 1).copy()
    w["w1"] = np.ascontiguousarray(np.asarray(inputs["mlp_w1"], dtype=np.float32)).reshape(D, FC, 128, 4 * C).astype(bf16)
    w["b1"] = np.asarray(inputs["mlp_b1"], dtype=np.float32).reshape(D, 24, 128, 1).copy()
    w["w2"] = np.ascontiguousarray(np.asarray(inputs["mlp_w2"], dtype=np.float32)).reshape(D, 24, 128, C).astype(bf16)
    w["b2"] = np.asarray(inputs["mlp_b2"], dtype=np.float32).reshape(D, FC, 128, 1).copy()
    w["lns2"] = np.asarray(inputs["ln_s"], dtype=np.float32).reshape(FC, 128, 1).copy()
    w["lnb2"] = np.asarray(inputs["ln_b"], dtype=np.float32).reshape(FC, 128, 1).copy()
    w["whead"] = np.asarray(inputs["head_w"], dtype=np.float32).reshape(FC, 128, NCLS).copy()
    w["bhead"] = np.asarray(inputs["head_b"], dtype=np.float32).reshape(1, NCLS).copy()
    pw = np.asarray(inputs["patch_w"], dtype=np.float32).reshape(C, 12)
    w["pwT"] = np.ascontiguousarray(pw.T)
    w["pbias"] = np.asarray(inputs["patch_b"], dtype=np.float32).reshape(FC, 128, 1).copy()
    pos = np.asarray(inputs["pos_emb"], dtype=np.float32).reshape(N, C)
    w["posT"] = np.ascontiguousarray(pos.T).reshape(FC, 128, N)
    return w


def _prep_x(x):
    Bb = x.shape[0]
    xp = np.asarray(x, dtype=np.float32).reshape(Bb, 3, 16, 2, 16, 2)
    xp = xp.transpose(0, 2, 4, 1, 3, 5).reshape(Bb, N, 12)
    return np.ascontiguousarray(np.swapaxes(xp, 1, 2))


# ====================== device program ======================

def build_nc(n_layers=D, tap_layer=None):
    nc = bacc.Bacc("TRN2", target_bir_lowering=False, debug=False)
    dram = {}

    def din(name, shape, dt=F32):
        dram[name] = nc.dram_tensor(name, list(shape), dt, kind="ExternalInput")

    din("xpt", (BL, 12, N))
    din("pwT", (12, C))
    din("pbias", (FC, 128, 1))
    din("posT", (FC, 128, N))
    din("wq", (D, FC, 128, C), BF16)
    din("wv", (D, FC, 128, C), BF16)
    din("wproj", (D, CH, C), BF16)
    din("bproj", (D, FC, 128, 1))
    din("lns", (D, FC, 128, 1))
    din("lnb", (D, FC, 128, 1))
    din("w1", (D, FC, 128, 4 * C), BF16)
    din("b1", (D, 24, 128, 1))
    din("w2", (D, 24, 128, C), BF16)
    din("b2", (D, FC, 128, 1))
    din("lns2", (FC, 128, 1))
    din("lnb2", (FC, 128, 1))
    din("whead", (FC, 128, NCLS))
    din("bhead", (1, NCLS))
    for cn, arr in CONSTS.items():
        din(cn, arr.shape, F32 if arr.dtype == np.float32 else BF16)

    out = nc.dram_tensor("out", [BL, NCLS], F32, kind="ExternalOutput")
    tap = None
    if tap_layer is not None:
        tap = nc.dram_tensor("tap", [128, FC, BL, N], F32, kind="ExternalOutput")
    if os.environ.get("KERNEL_DEBUG_TAPS"):
        nc._dbg = {
            "d_emb": nc.dram_tensor("d_emb", [128, FC, BL, N], F32, kind="ExternalOutput"),
            "d_S": nc.dram_tensor("d_S", [128, GM, 2, N], BF16, kind="ExternalOutput"),
            "d_V": nc.dram_tensor("d_V", [128, GM, 2, CH], F32, kind="ExternalOutput"),
            "d_u0": nc.dram_tensor("d_u0", [128, GM, 2, 1], F32, kind="ExternalOutput"),
            "d_X": nc.dram_tensor("d_X", [128, GM, 2, K], F32, kind="ExternalOutput"),
            "d_X1": nc.dram_tensor("d_X1", [128, GM, 2, K], F32, kind="ExternalOutput"),
            "d_X2": nc.dram_tensor("d_X2", [128, GM, 2, K], F32, kind="ExternalOutput"),
            "d_X3": nc.dram_tensor("d_X3", [128, GM, 2, K], F32, kind="ExternalOutput"),
            "d_r0c": nc.dram_tensor("d_r0c", [128, 2, K], F32, kind="ExternalOutput"),
            "d_x0c": nc.dram_tensor("d_x0c", [128, 2, K], F32, kind="ExternalOutput"),
            "d_B": nc.dram_tensor("d_B", [128, 512], F32, kind="ExternalOutput"),
            "d_Ut": nc.dram_tensor("d_Ut", [128, NG4, N], F32, kind="ExternalOutput"),
            "d_att": nc.dram_tensor("d_att", [128, FC, BL, N], F32, kind="ExternalOutput"),
        }
    else:
        nc._dbg = {}

    with TileContext(nc) as tc:
        with nc.allow_low_precision(reason="bf16 eigensolver by design"):
            _emit(nc, tc, dram, out, tap, n_layers, tap_layer)
    nc.compile()
    return nc


def _emit(nc, tc, dram, out, tap, n_layers, tap_layer):
    import contextlib
    es = contextlib.ExitStack()
    with es:
        persist = es.enter_context(tc.tile_pool(name="persist", bufs=1))
        wpool = es.enter_context(tc.tile_pool(name="wpool", bufs=1))
        wstr = es.enter_context(tc.tile_pool(name="wstr", bufs=2))
        wqrt = es.enter_context(tc.tile_pool(name="wqrt", bufs=1))
        spool = es.enter_context(tc.tile_pool(name="spool", bufs=1))
        epool = es.enter_context(tc.tile_pool(name="epool", bufs=1))
        scr = es.enter_context(tc.tile_pool(name="scr", bufs=1))
        rowp = es.enter_context(tc.tile_pool(name="rowp", bufs=1))
        ps1 = es.enter_context(tc.tile_pool(name="ps1", bufs=2, space="PSUM"))
        ps2 = es.enter_context(tc.tile_pool(name="ps2", bufs=2, space="PSUM"))
        ps3 = es.enter_context(tc.tile_pool(name="ps3", bufs=2, space="PSUM"))

        cst = {}
        for cn, arr in CONSTS.items():
            dt_ = F32 if arr.dtype == np.float32 else BF16
            t = persist.tile(list(arr.shape), dt_, tag="c_" + cn)
            cst[cn] = t
            nc.sync.dma_start(t[:], dram[cn][:])

        def smc(name):
            return cst[name][:].rearrange("p (a b) -> p a b", a=4)

        hT = persist.tile([128, FC, BL, N], F32, tag="hT")

        def r32(ap):
            return ap

        TT = nc.vector.tensor_tensor
        TS = nc.vector.tensor_scalar
        STT = nc.vector.scalar_tensor_tensor
        CP = nc.vector.tensor_copy
        MM = nc.tensor.matmul

        # ============ patch embed ============
        xpt = persist.tile([12, BL, N], F32, tag="xpt")
        nc.sync.dma_start(xpt[:], dram["xpt"][:].rearrange("b k n -> k b n"))
        pwT = persist.tile([12, C], F32, tag="pwT")
        nc.sync.dma_start(pwT[:], dram["pwT"][:])
        pbias = persist.tile([128, FC, 1], F32, tag="pbias")
        nc.sync.dma_start(pbias[:], dram["pbias"][:].rearrange("f p o -> p f o"))
        posT = persist.tile([128, FC, N], F32, tag="posT")
        nc.sync.dma_start(posT[:], dram["posT"][:].rearrange("f p n -> p f n"))

        for b in range(BL):
            for ch in range(FC):
                ps = ps1.tile([128, 512], F32, tag="ps1")
                MM(ps[:, 0:N], r32(pwT[:, 128 * ch:128 * (ch + 1)]),
                   r32(xpt[:, b]), start=True, stop=True)
                tmp = scr.tile([128, N], F32, tag="ge")
                TS(tmp[:], ps[:, 0:N], pbias[:, ch], None, op0=ALU.add)
                TT(hT[:, ch, b], tmp[:], posT[:, ch], op=ALU.add)

        if nc._dbg:
            nc.sync.dma_start(nc._dbg["d_emb"][:], hT[:])
        # ===================== layers =====================
        for li in range(n_layers):
            wq = wpool.tile([128, FC, C], BF16, tag="wq")
            nc.sync.dma_start(wq[:], dram["wq"][li].rearrange("f p c -> p f c"))
            wproj = wpool.tile([CH, C], BF16, tag="wproj")
            nc.sync.dma_start(wproj[:], dram["wproj"][li])
            hTb = wpool.tile([128, FC, BL, N], BF16, tag="hTb")
            for b_ in range(BL):
                for ch_ in range(FC):
                    CP(hTb[:, ch_, b_], hT[:, ch_, b_])
            bproj = wpool.tile([128, FC, 1], F32, tag="bproj")
            nc.sync.dma_start(bproj[:], dram["bproj"][li].rearrange("f p o -> p f o"))
            lns = wpool.tile([128, FC, 1], F32, tag="lns")
            nc.sync.dma_start(lns[:], dram["lns"][li].rearrange("f p o -> p f o"))
            lnb = wpool.tile([128, FC, 1], F32, tag="lnb")
            nc.sync.dma_start(lnb[:], dram["lnb"][li].rearrange("f p o -> p f o"))
            b1t = wpool.tile([128, 24, 1], F32, tag="b1t")
            nc.sync.dma_start(b1t[:], dram["b1"][li].rearrange("j p o -> p j o"))
            b2t = wpool.tile([128, FC, 1], F32, tag="b2t")
            nc.sync.dma_start(b2t[:], dram["b2"][li].rearrange("f p o -> p f o"))

            for g in range(2):
                bidx = [2 * g, 2 * g + 1]
                Sg = spool.tile([128, GM, 2, N], BF16, tag="Sg")
                Vg = spool.tile([128, GM, 2, CH], F32, tag="Vg")
                U0g = spool.tile([128, GM, 2, 1], F32, tag="U0g")
                U0r = spool.tile([128, NG4, N], F32, tag="U0r")

                # ---- v = hT^T Wv per matrix (wv streamed, SBUF-accumulate) --
                Vgb = spool.tile([128, GM, 2, CH], BF16, tag="Vgb")
                for ch in range(FC):
                    wvc = wstr.tile([128, C], BF16, tag="wvc")
                    nc.sync.dma_start(wvc[:], dram["wv"][li, ch])
                    for m in range(GM):
                        b = bidx[m // H]
                        h = m % H
                        for rb in range(2):
                            psv = ps3.tile([128, 256], F32, tag="ps3")
                            MM(psv[:, 0:CH],
                               r32(hTb[:, ch, b, 128 * rb:128 * (rb + 1)]),
                               r32(wvc[:, CH * h:CH * (h + 1)]),
                               start=True, stop=True)
                            if ch == 0:
                                CP(Vg[:, m, rb], psv[:, 0:CH])
                            else:
                                TT(Vg[:, m, rb], Vg[:, m, rb], psv[:, 0:CH],
                                   op=ALU.add)
                for m in range(GM):
                    CP(Vgb[:, m], Vg[:, m])

                # ---- phase A (exp table): logits -> G into Sg ----
                for m in range(GM):
                    b = bidx[m // H]
                    h = m % H
                    qT = scr.tile([CH, N], BF16, tag="qT")
                    psq = ps1.tile([128, 512], F32, tag="ps1")
                    for ch in range(FC):
                        MM(psq[0:CH, 0:N],
                           r32(wq[:, ch, CH * h:CH * (h + 1)]),
                           r32(hTb[:, ch, b]),
                           start=(ch == 0), stop=(ch == FC - 1))
                    CP(qT[:], psq[0:CH, 0:N])
                    psl = ps2.tile([128, 512], F32, tag="ps2")
                    for rb in range(2):
                        MM(psl[:, 256 * rb:256 * (rb + 1)],
                           r32(qT[:, 128 * rb:128 * (rb + 1)]),
                           r32(qT[:]), start=True, stop=True)
                    rm = scr.tile([128, 1], F32, tag="rm")
                    nc.vector.reduce_max(rm[:], psl[:], axis=AX.X)
                    pst = ps3.tile([128, 256], F32, tag="ps3")
                    MM(pst[0:1, 0:128], rm[:], cst["i128"][:], start=True, stop=True)
                    mv = scr.tile([1, 1], F32, tag="mv")
                    nc.vector.reduce_max(mv[:], pst[0:1, 0:128], axis=AX.X)
                    pbc = ps3.tile([128, 256], F32, tag="ps3")
                    MM(pbc[:, 0:1], cst["onesrow"][:], mv[:], start=True, stop=True)
                    negm = scr.tile([128, 1], F32, tag="negm")
                    nc.scalar.mul(negm[:], pbc[:, 0:1], -SCALE_QK)
                    for rb in range(2):
                        ge = scr.tile([128, N], F32, tag="ge")
                        nc.scalar.activation(ge[:], psl[:, 256 * rb:256 * (rb + 1)],
                                             AF.Exp, bias=negm[:, 0:1],
                                             scale=SCALE_QK)
                        TT(Sg[:, m, rb], ge[:], cst["vmask"][:, rb], op=ALU.mult)

                # ---- phase B (sqrt table): G -> Ahat, u0 ----
                for m in range(GM):
                    g4, rr_ = m // 4, m % 4
                    rsum = scr.tile([128, 2], F32, tag="rsum")
                    for rb in range(2):
                        nc.vector.reduce_sum(rsum[:, rb:rb + 1], Sg[:, m, rb],
                                             axis=AX.X)
                    invr = scr.tile([128, 2], BF16, tag="invr")
                    nc.vector.reciprocal(invr[:], rsum[:])
                    ptv = ps3.tile([128, 256], F32, tag="ps3")
                    for ob in range(2):
                        for kb in range(2):
                            MM(ptv[:, ob:ob + 1],
                               Sg[:, m, kb, 128 * ob:128 * (ob + 1)],
                               invr[:, kb:kb + 1],
                               start=(kb == 0), stop=(kb == 1))
                    deg = scr.tile([128, 2], F32, tag="deg")
                    TS(deg[:], ptv[:, 0:2], 0.5, 0.5, op0=ALU.mult, op1=ALU.add)
                    sd = scr.tile([128, 2], F32, tag="sd")
                    nc.scalar.activation(sd[:], deg[:], AF.Sqrt)
                    wv_ = scr.tile([128, 2], F32, tag="wv_")
                    nc.vector.reciprocal(wv_[:], sd[:])
                    alpha = scr.tile([128, 2], F32, tag="alpha")
                    TT(alpha[:], wv_[:], invr[:], op=ALU.mult)
                    nc.scalar.mul(alpha[:], alpha[:], 0.5)
                    psn = ps3.tile([128, 256], F32, tag="ps3")
                    for kb in range(2):
                        MM(psn[0:1, 0:1], deg[:, kb:kb + 1], cst["ones128"][:],
                           start=(kb == 0), stop=(kb == 1))
                    nrm = scr.tile([1, 1], F32, tag="nrm")
                    nc.scalar.activation(nrm[:], psn[0:1, 0:1], AF.Sqrt)
                    rn = scr.tile([1, 1], F32, tag="rn")
                    nc.vector.reciprocal(rn[:], nrm[:])
                    pbc = ps3.tile([128, 256], F32, tag="ps3")
                    MM(pbc[:, 0:1], cst["onesrow"][:], rn[:], start=True, stop=True)
                    rnb = scr.tile([128, 1], F32, tag="rnb")
                    CP(rnb[:], pbc[:, 0:1])
                    for kb in range(2):
                        TS(U0g[:, m, kb], sd[:, kb:kb + 1], rnb[:, 0:1], None,
                           op0=ALU.mult)
                    arow = rowp.tile([1, N], F32, tag="arow")
                    brow = rowp.tile([1, N], F32, tag="brow")
                    for (row_t, col_t) in ((arow, alpha), (brow, wv_)):
                        ptr2 = ps3.tile([128, 256], F32, tag="ps3")
                        for kb in range(2):
                            MM(ptr2[0:1, 128 * kb:128 * (kb + 1)],
                               col_t[:, kb:kb + 1], cst["i128"][:],
                               start=True, stop=True)
                        CP(row_t[:], ptr2[0:1, :])
                    ptr3 = ps3.tile([128, 256], F32, tag="ps3")
                    for kb in range(2):
                        MM(ptr3[32 * rr_:32 * rr_ + 1, 128 * kb:128 * (kb + 1)],
                           U0g[:, m, kb], cst["i128"][:],
                           start=True, stop=True, tile_position=(0, 32 * rr_))
                    CP(U0r[32 * rr_:32 * rr_ + 1, g4], ptr3[32 * rr_:32 * rr_ + 1, :])
                    for rb in range(2):
                        pso = ps1.tile([128, 512], F32, tag="ps1")
                        MM(pso[:, 0:N], arow[:, 128 * rb:128 * (rb + 1)],
                           brow[:], start=True, stop=False)
                        MM(pso[:, 0:N], brow[:, 128 * rb:128 * (rb + 1)],
                           arow[:], start=False, stop=True)
                        go = scr.tile([128, N], F32, tag="ge")
                        TT(go[:], Sg[:, m, rb], pso[:, 0:N], op=ALU.mult)
                        STT(Sg[:, m, rb], go[:], -CHEB_S, cst["dmask"][:, rb],
                            op0=ALU.mult, op1=ALU.bypass)
                        STT(Sg[:, m, rb], cst["dmask"][:, rb],
                            CHEB_S * (1.0 - C_CENT), Sg[:, m, rb],
                            op0=ALU.mult, op1=ALU.add)

                if nc._dbg and li == 0 and g == 0:
                    nc.sync.dma_start(nc._dbg["d_S"][:], Sg[:])
                    nc.sync.dma_start(nc._dbg["d_V"][:], Vg[:])
                    nc.sync.dma_start(nc._dbg["d_u0"][:], U0g[:])
                _eigensolve_and_attend(
                    nc, tc, cst, smc, dram, hT, Sg, Vgb, U0g, U0r,
                    wproj, bproj, epool, spool, scr, ps1, ps2, ps3,
                    TT, TS, STT, CP, MM, r32, li, g, bidx)

            if nc._dbg and li == 0:
                nc.sync.dma_start(nc._dbg["d_att"][:], hT[:])
            _mlp_block(nc, tc, cst, dram, hT, lns, lnb, b1t, b2t,
                       wqrt, spool, scr, ps1, ps2, ps3,
                       TT, TS, STT, CP, MM, r32, li)

            if tap_layer is not None and li == tap_layer:
                nc.sync.dma_start(tap[:], hT[:])

        # ============ pool + final LN + head ============
        lns2 = wpool.tile([128, FC, 1], F32, tag="lns")
        nc.sync.dma_start(lns2[:], dram["lns2"][:].rearrange("f p o -> p f o"))
        lnb2 = wpool.tile([128, FC, 1], F32, tag="lnb")
        nc.sync.dma_start(lnb2[:], dram["lnb2"][:].rearrange("f p o -> p f o"))
        whead = wpool.tile([128, FC, NCLS], F32, tag="whead")
        nc.sync.dma_start(whead[:], dram["whead"][:].rearrange("f p c -> p f c"))
        bhead = wpool.tile([1, NCLS], F32, tag="bhead")
        nc.sync.dma_start(bhead[:], dram["bhead"][:])

        for b in range(BL):
            pooled = scr.tile([128, FC], F32, tag="pooled")
            for ch in range(FC):
                nc.vector.reduce_sum(pooled[:, ch:ch + 1], hT[:, ch, b],
                                     axis=AX.X)
            nc.scalar.mul(pooled[:], pooled[:], 1.0 / N)
            psa = ps3.tile([128, 256], F32, tag="ps3")
            for ch in range(FC):
                MM(psa[0:1, 0:1], pooled[:, ch:ch + 1], cst["ones128"][:],
                   start=(ch == 0), stop=(ch == FC - 1))
            sq = scr.tile([128, FC], F32, tag="poolsq")
            nc.scalar.activation(sq[:], pooled[:], AF.Square)
            psb = ps3.tile([128, 256], F32, tag="ps3")
            for ch in range(FC):
                MM(psb[0:1, 0:1], sq[:, ch:ch + 1], cst["ones128"][:],
                   start=(ch == 0), stop=(ch == FC - 1))
            mean = scr.tile([1, 1], F32, tag="fmean")
            nc.scalar.mul(mean[:], psa[0:1, 0:1], 1.0 / C)
            msq = scr.tile([1, 1], F32, tag="fmsq")
            nc.scalar.mul(msq[:], psb[0:1, 0:1], 1.0 / C)
            m2 = scr.tile([1, 1], F32, tag="fm2")
            nc.scalar.activation(m2[:], mean[:], AF.Square)
            var = scr.tile([1, 1], F32, tag="fvar")
            TT(var[:], msq[:], m2[:], op=ALU.subtract)
            TS(var[:], var[:], 1e-5, None, op0=ALU.add)
            sdv = scr.tile([1, 1], F32, tag="fsdv")
            nc.scalar.activation(sdv[:], var[:], AF.Sqrt)
            rstd = scr.tile([1, 1], F32, tag="frstd")
            nc.vector.reciprocal(rstd[:], sdv[:])
            two = scr.tile([1, 2], F32, tag="ftwo")
            CP(two[:, 0:1], mean[:])
            CP(two[:, 1:2], rstd[:])
            psc = ps3.tile([128, 256], F32, tag="ps3")
            MM(psc[:, 0:2], cst["onesrow"][:], two[:], start=True, stop=True)
            meanb = scr.tile([128, 1], F32, tag="fmeanb")
            rstdb = scr.tile([128, 1], F32, tag="frstdb")
            CP(meanb[:], psc[:, 0:1])
            CP(rstdb[:], psc[:, 1:2])
            pnorm = scr.tile([128, FC], F32, tag="pnorm")
            TS(pnorm[:], pooled[:], meanb[:, 0:1], None, op0=ALU.subtract)
            TS(pnorm[:], pnorm[:], rstdb[:, 0:1], None, op0=ALU.mult)
            for ch in range(FC):
                TS(pnorm[:, ch:ch + 1], pnorm[:, ch:ch + 1],
                   lns2[:, ch], lnb2[:, ch], op0=ALU.mult, op1=ALU.add)
            psh = ps3.tile([128, 256], F32, tag="ps3")
            for ch in range(FC):
                MM(psh[0:1, 0:NCLS], pnorm[:, ch:ch + 1], whead[:, ch],
                   start=(ch == 0), stop=(ch == FC - 1))
            ologit = scr.tile([1, NCLS], F32, tag="ologit")
            TT(ologit[:], psh[0:1, 0:NCLS], bhead[:], op=ALU.add)
            nc.sync.dma_start(out[b:b + 1, :], ologit[:])


def _eigensolve_and_attend(nc, tc, cst, smc, dram, hT, Sg, Vg, U0g, U0r,
                           wproj, bproj, epool, spool, scr, ps1, ps2, ps3,
                           TT, TS, STT, CP, MM, r32, li, g, bidx):
    Xg = epool.tile([128, GM, 2, K], F32, tag="Xg")
    for m in range(GM):
        for kb in range(2):
            CP(Xg[:, m, kb], cst["x0c"][:, kb])
    Xt = epool.tile([128, NG4, N], F32, tag="Xt")

    def cheb_round():
        for m in range(GM):
            Tp = epool.tile([128, 2, K], BF16, tag="Tp")
            Tc = epool.tile([128, 2, K], BF16, tag="Tc")
            for kb in range(2):
                CP(Tp[:, kb], Xg[:, m, kb])
            psx = ps2.tile([128, 512], F32, tag="ps2")
            for ob in range(2):
                for kb in range(2):
                    MM(psx[:, K * ob:K * (ob + 1)],
                       Sg[:, m, kb, 128 * ob:128 * (ob + 1)],
                       Tp[:, kb], start=(kb == 0), stop=(kb == 1))
            for kb in range(2):
                nc.scalar.mul(Tc[:, kb], psx[:, K * kb:K * (kb + 1)], 0.5)
            for _ in range(D_CHEB - 1):
                psy = ps2.tile([128, 512], F32, tag="ps2")
                for ob in range(2):
                    for kb in range(2):
                        MM(psy[:, K * ob:K * (ob + 1)],
                           Sg[:, m, kb, 128 * ob:128 * (ob + 1)],
                           Tc[:, kb], start=(kb == 0), stop=False)
                    MM(psy[:, K * ob:K * (ob + 1)], cst["ni128b"][:], Tp[:, ob],
                       start=False, stop=True)
                for kb in range(2):
                    CP(Tp[:, kb], Tc[:, kb])
                    CP(Tc[:, kb], psy[:, K * kb:K * (kb + 1)])
            for kb in range(2):
                CP(Xg[:, m, kb], Tc[:, kb])

    def deflate():
        for m in range(GM):
            g4, r = m // 4, m % 4
            psp = ps3.tile([128, 256], F32, tag="ps3")
            for kb in range(2):
                MM(psp[32 * r:32 * r + 1, 0:K], U0g[:, m, kb], Xg[:, m, kb],
                   start=(kb == 0), stop=(kb == 1), tile_position=(0, 32 * r))
            pr = scr.tile([128, K], F32, tag="pr")
            nc.scalar.mul(pr[32 * r:32 * r + 1, :], psp[32 * r:32 * r + 1, 0:K], -1.0)
            psd = ps2.tile([128, 512], F32, tag="ps2")
            for kb in range(2):
                MM(psd[:, K * kb:K * (kb + 1)], cst["i128"][:], Xg[:, m, kb],
                   start=True, stop=False)
                MM(psd[:, K * kb:K * (kb + 1)],
                   U0r[32 * r:32 * r + 1, g4, 128 * kb:128 * (kb + 1)],
                   pr[32 * r:32 * r + 1, :],
                   start=False, stop=True, tile_position=(32 * r, 0))
            for kb in range(2):
                CP(Xg[:, m, kb], psd[:, K * kb:K * (kb + 1)])

    def small_mm(dst, lhsT, rhs):
        psb = ps2.tile([128, 512], F32, tag="ps2")
        pv = psb[:].rearrange("p (a b) -> p a b", a=4)
        for g4 in range(NG4):
            MM(pv[:, g4], lhsT[:, g4], rhs[:, g4], start=True, stop=True)
        CP(dst[:], pv[:])

    def build_xt():
        for m in range(GM):
            g4, r = m // 4, m % 4
            ptx = ps2.tile([128, 512], F32, tag="ps2")
            for kb in range(2):
                MM(ptx[32 * r:32 * r + K, 128 * kb:128 * (kb + 1)],
                   Xg[:, m, kb], cst["i128"][:],
                   start=True, stop=True, tile_position=(0, 32 * r))
            CP(Xt[32 * r:32 * r + K, g4], ptx[32 * r:32 * r + K, 0:256])

    def bcast_permat(v44):
        ptq = ps3.tile([128, 256], F32, tag="ps3")
        MM(ptq[0:4, 0:4], v44[:], cst["i128"][0:4, 0:4], start=True, stop=True)
        v44t = scr.tile([4, 4], F32, tag="v44t")
        CP(v44t[:], ptq[0:4, 0:4])
        ptw = ps3.tile([128, 256], F32, tag="ps3")
        MM(ptw[:, 0:4], cst["exp32"][:], v44t[:], start=True, stop=True)
        ob = scr.tile([128, 4], F32, tag="permat")
        CP(ob[:], ptw[:, 0:4])
        return ob

    def permat_max(src):
        ptm = ps3.tile([128, 256], F32, tag="ps3")
        MM(ptm[0:4, 0:128], src[:], cst["i128"][:], start=True, stop=True)
        tr = scr.tile([4, 128], F32, tag="tr44")
        CP(tr[:], ptm[0:4, 0:128])
        mx = scr.tile([4, 4], F32, tag="mx44")
        nc.vector.reduce_max(mx[:], tr[:].rearrange("p (a b) -> p a b", a=4),
                             axis=AX.X)
        return mx

    def whiten_pass(eps, steps):
        Gt = epool.tile([128, 4, 128], F32, tag="Gt")
        psg = ps2.tile([128, 512], F32, tag="ps2")
        pvg = psg[:].rearrange("p (a b) -> p a b", a=4)
        for m in range(GM):
            g4, r = m // 4, m % 4
            for kb in range(2):
                MM(psg[32 * r:32 * r + K,
                       128 * g4 + 32 * r:128 * g4 + 32 * r + K],
                   Xg[:, m, kb], Xg[:, m, kb],
                   start=(kb == 0), stop=(kb == 1), tile_position=(0, 32 * r))
        TT(Gt[:], pvg[:], smc("sm_blk"), op=ALU.mult)
        dg = scr.tile([128, 4, 128], F32, tag="smA")
        TT(dg[:], Gt[:], smc("sm_eye"), op=ALU.mult)
        dgt = scr.tile([128, 4], F32, tag="dgt")
        nc.vector.reduce_sum(dgt[:], dg[:], axis=AX.X)
        TS(dgt[:], dgt[:], 1e-12, None, op0=ALU.add)
        sq = scr.tile([128, 4], F32, tag="sq")
        nc.scalar.activation(sq[:], dgt[:], AF.Sqrt)
        srec = scr.tile([128, 4], F32, tag="srec")
        nc.vector.reciprocal(srec[:], sq[:])
        for g4 in range(NG4):
            TS(Gt[:, g4], Gt[:, g4], srec[:, g4:g4 + 1], None, op0=ALU.mult)
        GtT = epool.tile([128, 4, 128], F32, tag="GtT")
        small_mm(GtT, Gt, smc("sm_eye"))
        for g4 in range(NG4):
            TS(GtT[:, g4], GtT[:, g4], srec[:, g4:g4 + 1], None, op0=ALU.mult)
        STT(Gt[:], smc("sm_eye"), eps, GtT[:], op0=ALU.mult, op1=ALU.add)
        brs = scr.tile([128, 4], F32, tag="brs4")
        nc.vector.tensor_reduce(brs[:], Gt[:], axis=AX.X, op=ALU.add,
                                apply_absolute_value=True)
        mx = permat_max(brs)
        rec = scr.tile([4, 4], F32, tag="rec44")
        nc.vector.reciprocal(rec[:], mx[:])
        bre = bcast_permat(rec)
        for g4 in range(NG4):
            TS(Gt[:, g4], Gt[:, g4], bre[:, g4:g4 + 1], None, op0=ALU.mult)
        Yt = epool.tile([128, 4, 128], F32, tag="Yt")
        Zt = epool.tile([128, 4, 128], F32, tag="Zt")
        Tt = epool.tile([128, 4, 128], F32, tag="Tt")
        CP(Yt[:], Gt[:])
        CP(Zt[:], smc("sm_eye"))
        for _ in range(steps):
            psb = ps2.tile([128, 512], F32, tag="ps2")
            pv2 = psb[:].rearrange("p (a b) -> p a b", a=4)
            for g4 in range(NG4):
                MM(pv2[:, g4], Zt[:, g4], Yt[:, g4], start=True, stop=True)
            STT(Tt[:], pv2[:], -0.5, smc("sm_eye15"), op0=ALU.mult, op1=ALU.add)
            small_mm(Yt, Yt, Tt)
            small_mm(Zt, Tt, Zt)
        sqb = scr.tile([4, 4], F32, tag="sqb44")
        nc.scalar.activation(sqb[:], mx[:], AF.Sqrt)
        nc.vector.reciprocal(sqb[:], sqb[:])
        sbe = bcast_permat(sqb)
        for g4 in range(NG4):
            TS(Zt[:, g4], Zt[:, g4], srec[:, g4:g4 + 1], None, op0=ALU.mult)
            TS(Zt[:, g4], Zt[:, g4], sbe[:, g4:g4 + 1], None, op0=ALU.mult)
        build_xt()
        for m in range(GM):
            g4, r = m // 4, m % 4
            psx = ps2.tile([128, 512], F32, tag="ps2")
            for kb in range(2):
                MM(psx[:, K * kb:K * (kb + 1)],
                   Xt[32 * r:32 * r + K, g4, 128 * kb:128 * (kb + 1)],
                   Zt[32 * r:32 * r + K, g4, 32 * r:32 * r + K],
                   start=True, stop=True, tile_position=(32 * r, 0))
            for kb in range(2):
                CP(Xg[:, m, kb], psx[:, K * kb:K * (kb + 1)])

    # ---------------- rounds ----------------
    for rr in range(ROUNDS[li]):
        cheb_round()
        deflate()
        whiten_pass(*NS_MID)
        if nc._dbg and li == 0 and g == 0 and rr == ROUNDS[li] - 1:
            nc.sync.dma_start(nc._dbg["d_X1"][:], Xg[:])
        for m in range(GM):
            for kb in range(2):
                TT(Xg[:, m, kb], Xg[:, m, kb], cst["r0c"][:, kb], op=ALU.add)
        if nc._dbg and li == 0 and g == 0 and rr == ROUNDS[li] - 1:
            nc.sync.dma_start(nc._dbg["d_X2"][:], Xg[:])
            nc.sync.dma_start(nc._dbg["d_r0c"][:], cst["r0c"][:])
            nc.sync.dma_start(nc._dbg["d_x0c"][:], cst["x0c"][:])
    deflate()
    if nc._dbg and li == 0 and g == 0:
        nc.sync.dma_start(nc._dbg["d_X3"][:], Xg[:])
    whiten_pass(*NS_F1)
    whiten_pass(*NS_F2)
    if nc._dbg and li == 0 and g == 0:
        nc.sync.dma_start(nc._dbg["d_X"][:], Xg[:])

    # ---------------- B build ----------------
    Bt = epool.tile([128, 4, 128], F32, tag="Bt")
    Wt = epool.tile([128, 4, 128], F32, tag="Wt")
    psgB = ps2.tile([128, 512], F32, tag="ps2")
    pvB = psgB[:].rearrange("p (a b) -> p a b", a=4)
    for m in range(GM):
        g4, r = m // 4, m % 4
        Xgb = epool.tile([128, 2, K], BF16, tag="Tp")
        for kb in range(2):
            CP(Xgb[:, kb], Xg[:, m, kb])
        psz = ps1.tile([128, 512], F32, tag="ps1")
        for ob in range(2):
            for kb in range(2):
                MM(psz[:, K * ob:K * (ob + 1)],
                   Sg[:, m, kb, 128 * ob:128 * (ob + 1)],
                   Xgb[:, kb], start=(kb == 0), stop=(kb == 1))
        Zc = epool.tile([128, 2, K], BF16, tag="Zc")
        for kb in range(2):
            CP(Zc[:, kb], psz[:, K * kb:K * (kb + 1)])
        for kb in range(2):
            MM(psgB[32 * r:32 * r + K,
                    128 * g4 + 32 * r:128 * g4 + 32 * r + K],
               Xgb[:, kb], Zc[:, kb],
               start=(kb == 0), stop=(kb == 1), tile_position=(0, 32 * r))
    TT(Bt[:], pvB[:], smc("sm_blk"), op=ALU.mult)
    BtT = epool.tile([128, 4, 128], F32, tag="GtT")
    small_mm(BtT, Bt, smc("sm_eye"))
    TT(Bt[:], Bt[:], BtT[:], op=ALU.add)
    nc.scalar.mul(Bt[:], Bt[:], 0.5)
    CP(Wt[:], smc("sm_eye"))
    if nc._dbg and li == 0 and g == 0:
        nc.sync.dma_start(nc._dbg["d_B"][:], Bt[:].rearrange("p a b -> p (a b)"))
    build_xt()

    # ---------------- all-pairs diagonalizer ----------------
    Et = epool.tile([128, 4, 128], F32, tag="Et")
    Qt = epool.tile([128, 4, 128], F32, tag="Qt")
    QtT = epool.tile([128, 4, 128], F32, tag="QtT")
    M1 = epool.tile([128, 4, 128], F32, tag="M1")
    for it in range(DIAG_ITERS):
        dmat = scr.tile([128, 4, 128], F32, tag="smA")
        TT(dmat[:], Bt[:], smc("sm_eye"), op=ALU.mult)
        dcol = scr.tile([128, 4], F32, tag="dcol")
        nc.vector.reduce_sum(dcol[:], dmat[:], axis=AX.X)
        Drow = epool.tile([128, 4, 128], F32, tag="GtT")
        small_mm(Drow, smc("sm_blk"), dmat)
        dd = scr.tile([128, 4, 128], F32, tag="smB")
        for g4 in range(NG4):
            STT(dd[:, g4], Drow[:, g4], dcol[:, g4:g4 + 1],
                smc("sm_blk")[:, g4], op0=ALU.subtract, op1=ALU.mult)
        sgn = scr.tile([128, 4, 128], F32, tag="smC")
        TS(sgn[:], dd[:], 0.0, None, op0=ALU.is_ge)
        STT(dd[:], sgn[:], 2e-9, dd[:], op0=ALU.mult, op1=ALU.add)
        TS(dd[:], dd[:], -1e-9, None, op0=ALU.add)
        nc.vector.reciprocal(dd[:], dd[:])
        TT(Et[:], Bt[:], dd[:], op=ALU.mult)
        TT(Et[:], Et[:], smc("sm_offblk"), op=ALU.mult)
        TS(Et[:], Et[:], DIAG_CAP, None, op0=ALU.min)
        TS(Et[:], Et[:], -DIAG_CAP, None, op0=ALU.max)
        EtT = scr.tile([128, 4, 128], F32, tag="smA")
        small_mm(EtT, Et, smc("sm_eye"))
        TT(Et[:], Et[:], EtT[:], op=ALU.subtract)
        nc.scalar.mul(Et[:], Et[:], 0.5)
        ern = scr.tile([128, 4], F32, tag="ern4")
        nc.vector.tensor_reduce(ern[:], Et[:], axis=AX.X, op=ALU.add,
                                apply_absolute_value=True)
        emx = permat_max(ern)
        TS(emx[:], emx[:], 1e-9, None, op0=ALU.add)
        esc = scr.tile([4, 4], F32, tag="esc44")
        nc.vector.reciprocal(esc[:], emx[:])
        TS(esc[:], esc[:], DIAG_DAMP, 1.0, op0=ALU.mult, op1=ALU.min)
        ebe = bcast_permat(esc)
        for g4 in range(NG4):
            TS(Et[:, g4], Et[:, g4], ebe[:, g4:g4 + 1], None, op0=ALU.mult)
        TT(Qt[:], smc("sm_eye"), Et[:], op=ALU.add)
        TT(QtT[:], smc("sm_eye"), Et[:], op=ALU.subtract)
        for _ in range(2):
            P_ = scr.tile([128, 4, 128], F32, tag="smA")
            small_mm(P_, Qt, Qt)
            STT(M1[:], P_[:], -0.5, smc("sm_eye15"), op0=ALU.mult, op1=ALU.add)
            Qn = scr.tile([128, 4, 128], F32, tag="smB")
            small_mm(Qn, QtT, M1)
            small_mm(QtT, M1, QtT)
            CP(Qt[:], Qn[:])
        BQ = scr.tile([128, 4, 128], F32, tag="smA")
        small_mm(BQ, Bt, Qt)
        small_mm(Bt, Qt, BQ)
        small_mm(Wt, Qt, Wt)

    # ---------------- rank selection -> Ut ----------------
    dmat = scr.tile([128, 4, 128], F32, tag="smA")
    TT(dmat[:], Bt[:], smc("sm_eye"), op=ALU.mult)
    dcol = scr.tile([128, 4], F32, tag="dcol")
    nc.vector.reduce_sum(dcol[:], dmat[:], axis=AX.X)
    TS(dcol[:], dcol[:], cst["sm_tie"][:, 0:1], None, op0=ALU.add)
    dmat2 = scr.tile([128, 4, 128], F32, tag="smB")
    for g4 in range(NG4):
        TS(dmat2[:, g4], smc("sm_eye")[:, g4], dcol[:, g4:g4 + 1], None,
           op0=ALU.mult)
    Drow = epool.tile([128, 4, 128], F32, tag="GtT")
    small_mm(Drow, smc("sm_blk"), dmat2)
    Cc = scr.tile([128, 4, 128], F32, tag="smC")
    for g4 in range(NG4):
        STT(Cc[:, g4], Drow[:, g4], dcol[:, g4:g4 + 1],
            smc("sm_blk")[:, g4], op0=ALU.is_lt, op1=ALU.mult)
    rnk = scr.tile([128, 4], F32, tag="rnk")
    nc.vector.reduce_sum(rnk[:], Cc[:], axis=AX.X)
    Sel = scr.tile([128, 4, 128], F32, tag="smC")
    for g4 in range(NG4):
        TS(Sel[:, g4], smc("sm_iotasel")[:, g4], rnk[:, g4:g4 + 1], None,
           op0=ALU.is_equal)
    Pt = epool.tile([128, 4, 128], F32, tag="M1")
    small_mm(Pt, Wt, Sel)

    Utg = epool.tile([128, NG4, N], F32, tag="Utg")
    for g4 in range(NG4):
        psu = ps1.tile([128, 512], F32, tag="ps1")
        for r in range(4):
            MM(psu[32 * r:32 * r + N_EIGS, 0:N],
               Pt[32 * r:32 * r + K, g4, 32 * r:32 * r + N_EIGS],
               Xt[32 * r:32 * r + K, g4],
               start=True, stop=True, tile_position=(32 * r, 32 * r))
        CP(Utg[:, g4], psu[:, 0:N])

    un = scr.tile([128, NG4], F32, tag="un")
    for g4 in range(NG4):
        usq = scr.tile([128, N], F32, tag="ge")
        nc.scalar.activation(usq[:], Utg[:, g4], AF.Square)
        nc.vector.reduce_sum(un[:, g4:g4 + 1], usq[:], axis=AX.X)
    TS(un[:], un[:], 1e-30, None, op0=ALU.add)
    uns = scr.tile([128, NG4], F32, tag="uns")
    nc.scalar.activation(uns[:], un[:], AF.Sqrt)
    nc.vector.reciprocal(uns[:], uns[:])
    for g4 in range(NG4):
        TS(Utg[:, g4], Utg[:, g4], uns[:, g4:g4 + 1], None, op0=ALU.mult)

    if nc._dbg and li == 0 and g == 0:
        nc.sync.dma_start(nc._dbg["d_Ut"][:], Utg[:])
    # ---------------- group attention + out + proj ----------------
    aoutT = [spool.tile([CH, N], BF16, tag="aoutT" + str(i), name="aoutT" + str(i)) for i in range(2)]
    for m in range(GM):
        g4, r = m // 4, m % 4
        bloc = m // H
        h = m % H
        psga = ps1.tile([128, 512], F32, tag="ps1")
        MM(psga[0:32, 0:N], cst["combs4"][32 * r:32 * r + N_EIGS, :],
           Utg[32 * r:32 * r + N_EIGS, g4],
           start=True, stop=True, tile_position=(32 * r, 0))
        gmx = scr.tile([32, 1], F32, tag="gmx")
        nc.vector.reduce_max(gmx[:], psga[0:32, 0:N], axis=AX.X)
        ngm = scr.tile([32, 1], F32, tag="ngm")
        nc.scalar.mul(ngm[:], gmx[:], -1.0)
        ga = scr.tile([32, N], F32, tag="ga")
        nc.scalar.activation(ga[:], psga[0:32, 0:N], AF.Exp, bias=ngm[:, 0:1])
        gs = scr.tile([32, 1], F32, tag="gs")
        nc.vector.reduce_sum(gs[:], ga[:], axis=AX.X)
        gr = scr.tile([32, 1], F32, tag="gr")
        nc.vector.reciprocal(gr[:], gs[:])
        TS(ga[:], ga[:], gr[:, 0:1], None, op0=ALU.mult)
        gaT = scr.tile([128, 2, 32], BF16, tag="gaT")
        psgt = ps3.tile([128, 256], F32, tag="ps3")
        for kb in range(2):
            MM(psgt[:, 32 * kb:32 * (kb + 1)], ga[:, 128 * kb:128 * (kb + 1)],
               cst["i128"][0:32, 0:32], start=True, stop=True)
            CP(gaT[:, kb], psgt[:, 32 * kb:32 * (kb + 1)])
        psoh = ps3.tile([128, 256], F32, tag="ps3")
        for kb in range(2):
            MM(psoh[0:CH, 0:32], Vg[:, m, kb], gaT[:, kb],
               start=(kb == 0), stop=(kb == 1))
        CP(aoutT[bloc][:, 32 * h:32 * (h + 1)], psoh[0:CH, 0:32])

    for bloc in range(2):
        b = bidx[bloc]
        for ch in range(FC):
            psj = ps1.tile([128, 512], F32, tag="ps1")
            MM(psj[:, 0:N], r32(wproj[:, 128 * ch:128 * (ch + 1)]),
               r32(aoutT[bloc][:]), start=True, stop=True)
            TS(hT[:, ch, b], psj[:, 0:N], bproj[:, ch], None, op0=ALU.add)


def _mlp_block(nc, tc, cst, dram, hT, lns, lnb, b1t, b2t,
               wqrt, spool, scr, ps1, ps2, ps3, TT, TS, STT, CP, MM, r32, li):
    for pair in range(2):
        bb = [2 * pair, 2 * pair + 1]
        hnp = spool.tile([128, FC, 2, N], BF16, tag="Sg")
        for bloc, b in enumerate(bb):
            pss = ps3.tile([128, 256], F32, tag="ps3")
            ps2s = ps3.tile([128, 256], F32, tag="ps3")
            for ch in range(FC):
                MM(pss[0:1, 0:128], cst["ones128"][:], hT[:, ch, b, 0:128],
                   start=(ch == 0), stop=(ch == FC - 1))
            # NOTE: sums are computed in two half-token blocks (psum free 128)
            for ch in range(FC):
                MM(ps2s[0:1, 0:128], cst["ones128"][:], hT[:, ch, b, 128:256],
                   start=(ch == 0), stop=(ch == FC - 1))
            sums = scr.tile([1, N], F32, tag="lsums")
            CP(sums[:, 0:128], pss[0:1, 0:128])
            CP(sums[:, 128:256], ps2s[0:1, 0:128])
            sq1 = scr.tile([128, N], F32, tag="lt1")
            psq1 = ps3.tile([128, 256], F32, tag="ps3")
            psq2 = ps3.tile([128, 256], F32, tag="ps3")
            for ch in range(FC):
                nc.scalar.activation(sq1[:], hT[:, ch, b], AF.Square)
                MM(psq1[0:1, 0:128], cst["ones128"][:], sq1[:, 0:128],
                   start=(ch == 0), stop=(ch == FC - 1))
                MM(psq2[0:1, 0:128], cst["ones128"][:], sq1[:, 128:256],
                   start=(ch == 0), stop=(ch == FC - 1))
            sqs = scr.tile([1, N], F32, tag="lsqs")
            CP(sqs[:, 0:128], psq1[0:1, 0:128])
            CP(sqs[:, 128:256], psq2[0:1, 0:128])
            mean = scr.tile([1, N], F32, tag="lmean")
            nc.scalar.mul(mean[:], sums[:], 1.0 / C)
            msq = scr.tile([1, N], F32, tag="lmsq")
            nc.scalar.mul(msq[:], sqs[:], 1.0 / C)
            m2 = scr.tile([1, N], F32, tag="lm2")
            nc.scalar.activation(m2[:], mean[:], AF.Square)
            var = scr.tile([1, N], F32, tag="lvar")
            TT(var[:], msq[:], m2[:], op=ALU.subtract)
            TS(var[:], var[:], 1e-5, None, op0=ALU.add)
            sdv = scr.tile([1, N], F32, tag="lsdv")
            nc.scalar.activation(sdv[:], var[:], AF.Sqrt)
            rst = scr.tile([1, N], F32, tag="lrst")
            nc.vector.reciprocal(rst[:], sdv[:])
            mrs = scr.tile([1, N], F32, tag="lmrs")
            TT(mrs[:], mean[:], rst[:], op=ALU.mult)
            psb1 = ps1.tile([128, 512], F32, tag="ps1")
            MM(psb1[:, 0:N], cst["onesrow"][:], rst[:], start=True, stop=True)
            MM(psb1[:, 256:512], cst["onesrow"][:], mrs[:], start=True, stop=True)
            rstB = scr.tile([128, N], F32, tag="lrstB")
            mrsB = scr.tile([128, N], F32, tag="lmrsB")
            CP(rstB[:], psb1[:, 0:N])
            CP(mrsB[:], psb1[:, 256:512])
            for ch in range(FC):
                t1 = scr.tile([128, N], F32, tag="lt1")
                TT(t1[:], hT[:, ch, b], rstB[:], op=ALU.mult)
                TT(t1[:], t1[:], mrsB[:], op=ALU.subtract)
                TS(hnp[:, ch, bloc], t1[:], lns[:, ch], lnb[:, ch],
                   op0=ALU.mult, op1=ALU.add)

        # MLP quarters: a1q = gelu(psum-accum) ; mlp2 accumulates into hT
        for q in range(4):
            w1q = wqrt.tile([128, FC, 768], BF16, tag="wqrt")
            nc.sync.dma_start(
                w1q[:], dram["w1"][li, :, :, 768 * q:768 * (q + 1)]
                .rearrange("f p c -> p f c"))
            a1q = spool.tile([128, 6, 2, N], BF16, tag="Vg")
            for jc in range(6):
                jg = 6 * q + jc
                psm = ps1.tile([128, 512], F32, tag="ps1")
                for ch in range(FC):
                    MM(psm[:],
                       r32(w1q[:, ch, 128 * jc:128 * (jc + 1)]),
                       r32(hnp[:, ch].rearrange("p a b -> p (a b)")),
                       start=(ch == 0), stop=(ch == FC - 1))
                nc.scalar.activation(
                    a1q[:, jc].rearrange("p a b -> p (a b)"), psm[:],
                    AF.Gelu, bias=b1t[:, jg])
            w2q = wqrt.tile([128, FC, 768], BF16, tag="wqrt")
            nc.sync.dma_start(
                w2q[:], dram["w2"][li, 6 * q:6 * (q + 1)]
                .rearrange("j p c -> p j c"))
            for mc in range(FC):
                psm2 = ps1.tile([128, 512], F32, tag="ps1")
                for jc in range(6):
                    MM(psm2[:],
                       r32(w2q[:, jc, 128 * mc:128 * (mc + 1)]),
                       r32(a1q[:, jc].rearrange("p a b -> p (a b)")),
                       start=(jc == 0), stop=(jc == 5))
                hv = hT[:, mc, 2 * pair:2 * pair + 2, :].rearrange("p a b -> p (a b)")
                if q == 0:
                    STT(hv, psm2[:], b2t[:, mc], hv, op0=ALU.add, op1=ALU.add)
                else:
                    TT(hv, hv, psm2[:], op=ALU.add)
            # residual base already in hT (attn out); mlp adds on top


FC_A1 = 6


# ====================== host wrapper ======================

_NC_CACHE = {}


def _get_nc(n_layers=D, tap_layer=None):
    key = (n_layers, tap_layer)
    if key not in _NC_CACHE:
        _NC_CACHE[key] = build_nc(n_layers, tap_layer)
    return _NC_CACHE[key]


def kernel(**inputs):
    x = np.asarray(inputs["x"], dtype=np.float32)
    w = _prep_weights(inputs)
    xp = _prep_x(x)                      # [B, 12, 256]
    nc = _get_nc()
    in_maps = []
    for c in range(NCORES):
        im = {}
        im["xpt"] = np.ascontiguousarray(xp[BL * c:BL * (c + 1)])
        for kname, arr in w.items():
            im[kname] = arr
        for cn, arr in CONSTS.items():
            im[cn] = arr
        in_maps.append(im)
    res = run_bass_kernel_spmd(nc, in_maps, core_ids=list(range(NCORES)))
    outs = [res.results[c]["out"] for c in range(NCORES)]
    return np.concatenate(outs, axis=0).astype(np.float32)


if __name__ == "__main__":
    rng = np.random.RandomState(0)
    print("building 1-layer nc for smoke...")
    import time
    t0 = time.time()
    nc = build_nc(n_layers=1, tap_layer=0)
    print("build+compile took", time.time() - t0)

